# revision 1
# baseline (speedup 1.0000x reference)
"""CGCNN message-passing kernel for 8 Trainium2 NeuronCores (Bass/Tile).

Strategy (data-parallel by dst-node range):
- Nodes split into 8 contiguous shards of 3750 (padded to 3840). Edges are
  assigned to the core owning their dst node, grouped by 128-node dst window,
  padded so every (core, window) has the same chunk count (SPMD-uniform).
- Per layer: each core projects its own h shard (Q_dst windows on the fly),
  all-gathers the full node table hT [64, 30720] f32 into SBUF, then per
  128-edge chunk: src rows via gpsimd ap_gather from the SBUF table,
  dst contribution via onehot^T matmul against the window projection,
  edge-attr contribution via matmul (bias folded in a ones-row), gated
  nonlinearity sigmoid*softplus computed as ln/exp (one ACT table set),
  segment-mean via onehot matmul with 1/cnt folded into the onehot.
- BatchNorm batch stats via a tiny AllReduce; residual+relu updates the own
  shard only; the next layer's AllGather redistributes.
- Global mean-pool via onehot matmul (1/gcnt folded), partial sums
  AllReduced, tiny head MLP computed redundantly on every core.
"""
import numpy as np
import ml_dtypes

N = 30000
E = 480000
NF = 92
EF = 50
D1 = 64
D2 = 64
L = 3
FC = 2
G = 256
EPS = 1e-5
NCORES = 8
SHARD = N // NCORES            # 3750
SHARD_P = 3840                 # padded shard (30 windows of 128)
NWIN = SHARD_P // 128          # 30
TBL = NCORES * SHARD_P         # 30720 table entries

_CACHE = {}


def _build_nc(cw):
    """Build the SPMD bass module. cw = chunks per dst window (uniform)."""
    import concourse.mybir as mybir
    from concourse import bacc
    from concourse.tile import TileContext

    f32 = mybir.dt.float32
    bf16 = mybir.dt.bfloat16
    AF = mybir.ActivationFunctionType
    OP = mybir.AluOpType

    nchunk = NWIN * cw                 # chunks per layer
    etot = nchunk * 128                # padded edges per core
    ntile = (etot + 1023) // 1024      # gather/ea tiles of up to 1024 edges

    import concourse.hw_specs as _hw
    import concourse.bacc as _bacc_mod
    _real_tables = _hw.get_activation_tables("gen3")
    _combined = "natural_log_exp_and_others"
    if _combined in _real_tables:
        _patched = {
            k: (v if k == _combined else (v - {AF.Exp, AF.Ln}))
            for k, v in _real_tables.items()
        }
        _bacc_mod.get_activation_tables = lambda arch: _patched

    nc = bacc.Bacc(None, target_bir_lowering=False)

    # ---- inputs (per core) ----
    xT = nc.dram_tensor("xT", [NF, SHARD_P], f32, kind="ExternalInput")
    eaT = nc.dram_tensor("eaT", [EF + 1, etot], bf16, kind="ExternalInput")
    sidx = nc.dram_tensor("sidx", [128, etot // 32], mybir.dt.int16, kind="ExternalInput")
    dstloc_p = nc.dram_tensor("dstloc_p", [128, nchunk], f32, kind="ExternalInput")
    dstloc_f = nc.dram_tensor("dstloc_f", [1, etot], f32, kind="ExternalInput")
    rc_p = nc.dram_tensor("rc_p", [128, nchunk], f32, kind="ExternalInput")
    batchloc = nc.dram_tensor("batchloc", [128, NWIN], f32, kind="ExternalInput")
    rgc_pn = nc.dram_tensor("rgc_pn", [128, NWIN], f32, kind="ExternalInput")
    # weights (replicated)
    lin0w = nc.dram_tensor("lin0w", [NF, D1], f32, kind="ExternalInput")
    lin0b = nc.dram_tensor("lin0b", [D1, 1], f32, kind="ExternalInput")
    wdst = nc.dram_tensor("wdst", [D1, L * 128], f32, kind="ExternalInput")
    wsrc = nc.dram_tensor("wsrc", [2 * D1, L * 128], f32, kind="ExternalInput")
    wea = nc.dram_tensor("wea", [EF + 1, L * 128], bf16, kind="ExternalInput")
    bng = nc.dram_tensor("bng", [D1, L], f32, kind="ExternalInput")
    bnb = nc.dram_tensor("bnb", [D1, L], f32, kind="ExternalInput")
    lin1w = nc.dram_tensor("lin1w", [D1, D2], f32, kind="ExternalInput")
    lin1b = nc.dram_tensor("lin1b", [D2, 1], f32, kind="ExternalInput")
    fcw = nc.dram_tensor("fcw", [D2, FC * D2], f32, kind="ExternalInput")
    fcb = nc.dram_tensor("fcb", [D2, FC], f32, kind="ExternalInput")
    lin2w = nc.dram_tensor("lin2w", [D2, 1], f32, kind="ExternalInput")
    lin2b = nc.dram_tensor("lin2b", [1, 1], f32, kind="ExternalInput")
    iota128 = nc.dram_tensor("iota128", [128, 128], f32, kind="ExternalInput")
    iota256 = nc.dram_tensor("iota256", [128, G], f32, kind="ExternalInput")
    iotap = nc.dram_tensor("iotap", [128, 1], f32, kind="ExternalInput")
    ident = nc.dram_tensor("ident", [128, 128], f32, kind="ExternalInput")

    yout = nc.dram_tensor("y", [1, G], f32, kind="ExternalOutput")

    # ---- collective bounce buffers ----
    ag_in = nc.dram_tensor("ag_in", [D1, SHARD_P], f32)
    ag_out = nc.dram_tensor("ag_out", [NCORES * D1, SHARD_P], f32, addr_space="Shared")
    ar_in = nc.dram_tensor("ar_in", [D1, 16], f32)
    ar_out = nc.dram_tensor("ar_out", [D1, 16], f32, addr_space="Shared")
    pl_in = nc.dram_tensor("pl_in", [D1, G], f32)
    aggr_d = nc.dram_tensor("aggr_d", [D1, SHARD_P], f32)
    pl_out = nc.dram_tensor("pl_out", [D1, G], f32, addr_space="Shared")

    rg = [list(range(NCORES))]

    with TileContext(nc) as tc:
        with (
            tc.tile_pool(name="const", bufs=1) as cp,
            tc.tile_pool(name="big", bufs=1) as bigp,
            tc.tile_pool(name="work", bufs=2) as wp,
            tc.tile_pool(name="oh", bufs=2) as ohp,
            tc.tile_pool(name="nl", bufs=1) as nlp,
            tc.tile_pool(name="psum", bufs=2, space="PSUM") as pp,
            tc.tile_pool(name="psA", bufs=2, space="PSUM") as ppA,
            tc.tile_pool(name="psS", bufs=2, space="PSUM") as ppS,

        ):
            # ---------- load constants ----------
            def load_const(t, dram, shape, dtype=f32):
                tt = cp.tile(shape, dtype, tag=t)
                nc.sync.dma_start(out=tt[:], in_=dram)
                return tt

            io128 = load_const("io128", iota128[:, :], [128, 128])
            io256 = load_const("io256", iota256[:, :], [128, G])
            iop = load_const("iop", iotap[:, :], [128, 1])
            idn = load_const("idn", ident[:, :], [128, 128])
            l0w = load_const("l0w", lin0w[:, :], [NF, D1])
            l0b = load_const("l0b", lin0b[:, :], [D1, 1])
            wd = load_const("wd", wdst[:, :], [D1, L * 128])
            ws = load_const("ws", wsrc[:, :], [2 * D1, L * 128])
            we = load_const("we", wea[:, :], [EF + 1, L * 128], bf16)
            gmt = load_const("gmt", bng[:, :], [D1, L])
            bbt = load_const("bbt", bnb[:, :], [D1, L])
            l1w = load_const("l1w", lin1w[:, :], [D1, D2])
            l1b = load_const("l1b", lin1b[:, :], [D2, 1])
            fw = load_const("fw", fcw[:, :], [D2, FC * D2])
            fb = load_const("fb", fcb[:, :], [D2, FC])
            l2w = load_const("l2w", lin2w[:, :], [D2, 1])
            l2b = load_const("l2b", lin2b[:, :], [1, 1])
            dlp = load_const("dlp", dstloc_p[:, :], [128, nchunk])
            rcp = load_const("rcp", rc_p[:, :], [128, nchunk])
            blc = load_const("blc", batchloc[:, :], [128, NWIN])
            rgp = load_const("rgp", rgc_pn[:, :], [128, NWIN])

            # ---------- resident state ----------
            hT_own = bigp.tile([D1, SHARD_P], f32, tag="hown")      # own shard
            hT_full = bigp.tile([128, TBL], f32, tag="hfull")       # gather table (x2 replica)

            # ---------- lin0: hT_own = relu(lin0w.T @ xT + b) ----------
            for j in range(8):
                sl = slice(j * 480, (j + 1) * 480)
                xt = wp.tile([NF, 480], f32, tag="xt")
                nc.sync.dma_start(out=xt[:], in_=xT[:, sl])
                ph = pp.tile([D1, 480], f32, tag="pre")
                nc.tensor.matmul(out=ph[:], lhsT=l0w[:], rhs=xt[:],
                                 start=True, stop=True)
                nc.scalar.activation(out=hT_own[:, sl], in_=ph[:],
                                     func=AF.Relu, bias=l0b[:], scale=1.0)

            # ---------- layers ----------
            for l in range(L):
                # --- AllGather h table ---
                nc.sync.dma_start(out=ag_in[:, :], in_=hT_own[:])
                nc.gpsimd.collective_compute(
                    "AllGather", OP.bypass, replica_groups=rg,
                    ins=[ag_in.ap().opt()], outs=[ag_out.ap().opt()])
                for s_ in range(NCORES):
                    nc.sync.dma_start(
                        out=hT_full[0:D1, s_ * SHARD_P:(s_ + 1) * SHARD_P],
                        in_=ag_out[s_ * D1:(s_ + 1) * D1, :])
                    nc.sync.dma_start(
                        out=hT_full[D1:2 * D1, s_ * SHARD_P:(s_ + 1) * SHARD_P],
                        in_=ag_out[s_ * D1:(s_ + 1) * D1, :])

                wd_l = wd[:, l * 128:(l + 1) * 128]
                ws_lA = ws[0:D1, l * 128:(l + 1) * 128]
                ws_lB = ws[D1:2 * D1, l * 128:(l + 1) * 128]
                we_l = we[:, l * 128:(l + 1) * 128]

                # --- edge pipeline ---
                state = {}
                s1p = wp.tile([D1, 8], f32, tag="s1p")
                s2p = wp.tile([D1, 8], f32, tag="s2p")
                for tt in range(0, ntile, 2):
                    tls = [t for t in (tt, tt + 1) if t < ntile]
                    st_d = {}
                    s_edges = sum(min(1024, etot - t * 1024) for t in tls)
                    s_half = s_edges // 2            # edges per partition-half
                    hchunks = s_half // 128          # chunks per half
                    sixt = ohp.tile([128, 64], mybir.dt.int16, tag="sixt")
                    nc.sync.dma_start(out=sixt[:, :s_half // 16],
                                      in_=sidx[:, tt * 32: tt * 32 + s_half // 16])
                    gt = ohp.tile([128, 1024, 1], f32, tag="gt")
                    nc.gpsimd.ap_gather(
                        gt[:, :s_half, :],
                        hT_full[:].rearrange("p (n o) -> p n o", o=1),
                        sixt[:, :s_half // 16],
                        channels=128, num_elems=TBL, d=1, num_idxs=s_half)
                    for t in tls:
                        te = min(1024, etot - t * 1024)   # edges this tile
                        nch = te // 128
                        et = ohp.tile([EF + 1, 1024], bf16, tag="et")
                        nc.sync.dma_start(out=et[:, :te],
                                          in_=eaT[:, t * 1024: t * 1024 + te])

                        pre = pp.tile([128, 1024], f32, tag="pre")
                        for c in range(nch):
                            gc = t * 8 + c          # global chunk id
                            w = gc // cw            # dst window
                            if gc % cw == 0:
                                qp = ppA.tile([128, 128], f32, tag="qp")
                                nc.tensor.matmul(
                                    out=qp[:],
                                    lhsT=hT_own[:, w * 128:(w + 1) * 128],
                                    rhs=wd_l, start=True, stop=True)
                                qd = wp.tile([128, 128], f32, tag="qd")
                                nc.vector.tensor_copy(out=qd[:], in_=qp[:])
                                state["qd"] = qd
                            qd = state["qd"]
                            o01 = ohp.tile([128, 128], f32, tag="o01")
                            nc.vector.tensor_scalar(
                                out=o01[:], in0=io128[:],
                                scalar1=dlp[:, gc:gc + 1], scalar2=None,
                                op0=OP.is_equal)
                            ohTp = ppA.tile([128, 128], f32, tag="qp")
                            nc.tensor.transpose(out=ohTp[:], in_=o01[:], identity=idn[:])
                            ohT = ohp.tile([128, 128], f32, tag="ohT")
                            nc.vector.tensor_copy(out=ohT[:], in_=ohTp[:])
                            csl = slice(c * 128, (c + 1) * 128)
                            nc.tensor.matmul(out=pre[:, csl], lhsT=ohT[:], rhs=qd[:],
                                             start=True, stop=False)
                            sc = (t - tt) * 8 + c     # chunk within super-tile
                            gsl = (slice(0, D1), slice(sc * 128, (sc + 1) * 128)) if sc < hchunks \
                                else (slice(D1, 2 * D1), slice((sc - hchunks) * 128, (sc - hchunks + 1) * 128))
                            nc.tensor.matmul(out=pre[:, csl],
                                             lhsT=gt[gsl[0], gsl[1], 0:1].rearrange("p e o -> p (e o)"),
                                             rhs=ws_lA if sc < hchunks else ws_lB,
                                             start=False, stop=False)
                            nc.tensor.matmul(out=pre[:, csl], lhsT=et[:, csl],
                                             rhs=we_l, start=False, stop=True)
                        st_d[t] = (pre, te, nch)

                    # ACT grouped by table set across the super-tile
                    for t in tls:
                        pre, te, nch = st_d[t]
                        pre3 = pre[:].rearrange("p (c g) -> p c g", g=128)
                        t1 = nlp.tile([128, 512], f32, tag="t1" + str(t % 2))
                        t13 = t1[:].rearrange("p (c g) -> p c g", g=64)
                        nc.scalar.activation(out=t13[:, :nch, :], in_=pre3[:, :nch, 0:64],
                                             func=AF.Exp, scale=-1.0)
                        st_d[t] = (pre, te, nch, t1)
                    for t in tls:
                        pre, te, nch, t1 = st_d[t]
                        pre3 = pre[:].rearrange("p (c g) -> p c g", g=128)
                        t2 = nlp.tile([128, 512], f32, tag="t2" + str(t % 2))
                        t23 = t2[:].rearrange("p (c g) -> p c g", g=64)
                        nc.scalar.activation(out=t23[:, :nch, :], in_=pre3[:, :nch, 64:128],
                                             func=AF.Exp, scale=1.0)
                        sp = nlp.tile([128, 512], f32, tag="sp" + str(t % 2))
                        nc.scalar.activation(out=sp[:, :nch * 64], in_=t2[:, :nch * 64],
                                             func=AF.Ln, bias=1.0, scale=1.0)
                        st_d[t] = (te, nch, t1, sp)

                    for t in tls:
                        te, nch, t1, sp = st_d[t]
                        nc.vector.tensor_scalar(out=t1[:, :nch * 64], in0=t1[:, :nch * 64],
                                                scalar1=1.0, scalar2=None, op0=OP.add)
                        nc.vector.reciprocal(out=t1[:, :nch * 64], in_=t1[:, :nch * 64])
                        m = sp
                        nc.vector.tensor_tensor(out=m[:, :nch * 64], in0=sp[:, :nch * 64],
                                                in1=t1[:, :nch * 64], op=OP.mult)
                        for c in range(nch):
                            gc = t * 8 + c
                            w = gc // cw
                            grp = w // 4
                            if gc % (4 * cw) == 0:
                                pa = ppS.tile([64, 512], f32, tag="pa")
                                state["pa"] = pa
                            pa = state["pa"]
                            ohS = ohp.tile([128, 128], f32, tag="ohS")
                            nc.vector.tensor_scalar(
                                out=ohS[:], in0=io128[:],
                                scalar1=dlp[:, gc:gc + 1], scalar2=rcp[:, gc:gc + 1],
                                op0=OP.is_equal, op1=OP.mult)
                            nc.tensor.matmul(
                                out=pa[:, (w % 4) * 128:(w % 4 + 1) * 128],
                                lhsT=m[:, c * 64:(c + 1) * 64],
                                rhs=ohS[:],
                                start=(gc % cw == 0), stop=(gc % cw == cw - 1))
                            if gc % (4 * cw) == 4 * cw - 1 or gc == nchunk - 1:
                                lo = grp * 512
                                hi = min(lo + 512, SHARD_P)
                                asb = wp.tile([D1, 512], f32, tag="asb")
                                nc.vector.tensor_copy(out=asb[:, :hi - lo],
                                                      in_=pa[:, :hi - lo])
                                nc.sync.dma_start(out=aggr_d[:, lo:hi],
                                                  in_=asb[:, :hi - lo])
                                nc.vector.reduce_sum(out=s1p[:, grp:grp + 1],
                                                     in_=asb[:, :hi - lo],
                                                     axis=mybir.AxisListType.X)
                                sqt = wp.tile([D1, 512], f32, tag="sqt")
                                nc.vector.tensor_tensor(out=sqt[:, :hi - lo],
                                                        in0=asb[:, :hi - lo],
                                                        in1=asb[:, :hi - lo], op=OP.mult)
                                nc.vector.reduce_sum(out=s2p[:, grp:grp + 1],
                                                     in_=sqt[:, :hi - lo],
                                                     axis=mybir.AxisListType.X)

                # --- BN stats + AllReduce ---
                st = wp.tile([D1, 16], f32, tag="st")
                nc.vector.reduce_sum(out=st[:, 0:1], in_=s1p[:], axis=mybir.AxisListType.X)
                nc.vector.reduce_sum(out=st[:, 1:2], in_=s2p[:], axis=mybir.AxisListType.X)
                nc.sync.dma_start(out=ar_in[:, :], in_=st[:])
                nc.gpsimd.collective_compute(
                    "AllReduce", OP.add, replica_groups=rg,
                    ins=[ar_in.ap().opt()], outs=[ar_out.ap().opt()])
                stg = wp.tile([D1, 16], f32, tag="stg")
                nc.sync.dma_start(out=stg[:], in_=ar_out[:, :])
                mu = wp.tile([D1, 1], f32, tag="mu")
                nc.vector.tensor_scalar(out=mu[:], in0=stg[:, 0:1],
                                        scalar1=1.0 / N, scalar2=None, op0=OP.mult)
                ex2 = wp.tile([D1, 1], f32, tag="ex2")
                nc.vector.tensor_scalar(out=ex2[:], in0=stg[:, 1:2],
                                        scalar1=1.0 / N, scalar2=None, op0=OP.mult)
                mu2 = wp.tile([D1, 1], f32, tag="mu2")
                nc.vector.tensor_tensor(out=mu2[:], in0=mu[:], in1=mu[:], op=OP.mult)
                var = wp.tile([D1, 1], f32, tag="var")
                nc.vector.tensor_tensor(out=var[:], in0=ex2[:], in1=mu2[:], op=OP.subtract)
                # isd = exp(-0.5*ln(var+eps))
                ve = wp.tile([D1, 1], f32, tag="ve")
                nc.vector.tensor_scalar(out=ve[:], in0=var[:], scalar1=EPS,
                                        scalar2=None, op0=OP.add)
                lv = wp.tile([D1, 1], f32, tag="lv")
                nc.scalar.activation(out=lv[:], in_=ve[:], func=AF.Ln, scale=1.0)
                isd = wp.tile([D1, 1], f32, tag="isd")
                nc.scalar.activation(out=isd[:], in_=lv[:], func=AF.Exp, scale=-0.5)
                scale = wp.tile([D1, 1], f32, tag="scale")
                nc.vector.tensor_tensor(out=scale[:], in0=isd[:], in1=gmt[:, l:l + 1], op=OP.mult)
                mshift = wp.tile([D1, 1], f32, tag="mshift")
                nc.vector.tensor_tensor(out=mshift[:], in0=mu[:], in1=scale[:], op=OP.mult)
                shift = wp.tile([D1, 1], f32, tag="shift")
                nc.vector.tensor_tensor(out=shift[:], in0=bbt[:, l:l + 1], in1=mshift[:], op=OP.subtract)
                # h = relu(h + aggr*scale + shift), streamed in 512 groups
                for j in range(8):
                    lo, hi = j * 512, min(j * 512 + 512, SHARD_P)
                    asb = wp.tile([D1, 512], f32, tag="asb")
                    nc.sync.dma_start(out=asb[:, :hi - lo], in_=aggr_d[:, lo:hi])
                    nc.vector.tensor_scalar(out=asb[:, :hi - lo], in0=asb[:, :hi - lo],
                                            scalar1=scale[:], scalar2=shift[:],
                                            op0=OP.mult, op1=OP.add)
                    nc.vector.tensor_tensor(out=asb[:, :hi - lo], in0=asb[:, :hi - lo],
                                            in1=hT_own[:, lo:hi], op=OP.add)
                    nc.vector.tensor_scalar(out=hT_own[:, lo:hi], in0=asb[:, :hi - lo],
                                            scalar1=0.0, scalar2=None, op0=OP.max)

            # ---------- global mean pool ----------
            pool_ps = ppS.tile([D1, G], f32, tag="pa")
            for w in range(NWIN):
                # rows of h for this window: transpose hT_own slice
                tp = ppA.tile([128, D1], f32, tag="qp")
                nc.tensor.transpose(out=tp[:], in_=hT_own[:, w * 128:(w + 1) * 128],
                                    identity=idn[0:D1, 0:D1])
                rows = wp.tile([128, D1], f32, tag="rows")
                nc.vector.tensor_copy(out=rows[:], in_=tp[:])
                ohg = ohp.tile([128, G], f32, tag="ohg")
                nc.vector.tensor_scalar(
                    out=ohg[:], in0=io256[:],
                    scalar1=blc[:, w:w + 1], scalar2=rgp[:, w:w + 1],
                    op0=OP.is_equal, op1=OP.mult)
                nc.tensor.matmul(out=pool_ps[:], lhsT=rows[:], rhs=ohg[:],
                                 start=(w == 0), stop=(w == NWIN - 1))
            poolT = wp.tile([D1, G], f32, tag="poolT")
            nc.vector.tensor_copy(out=poolT[:], in_=pool_ps[:])
            nc.sync.dma_start(out=pl_in[:, :], in_=poolT[:])
            nc.gpsimd.collective_compute(
                "AllReduce", OP.add, replica_groups=rg,
                ins=[pl_in.ap().opt()], outs=[pl_out.ap().opt()])
            pg = wp.tile([D1, G], f32, tag="pg")
            nc.sync.dma_start(out=pg[:], in_=pl_out[:, :])

            # ---------- head ----------
            a = pg
            hw_ = [(l1w[:], l1b[:]), (fw[:, 0:D2], fb[:, 0:1]), (fw[:, D2:2 * D2], fb[:, 1:2])]
            for (wt, bt) in hw_:
                ps = pp.tile([D2, G], f32, tag="pre")
                nc.tensor.matmul(out=ps[:], lhsT=wt, rhs=a[:], start=True, stop=True)
                an = wp.tile([D2, G], f32, tag="an")
                nc.scalar.activation(out=an[:], in_=ps[:], func=AF.Relu,
                                     bias=bt, scale=1.0)
                a = an
            ps = pp.tile([1, G], f32, tag="pre")
            nc.tensor.matmul(out=ps[:], lhsT=l2w[:], rhs=a[:], start=True, stop=True)
            yt = wp.tile([1, G], f32, tag="yt")
            nc.scalar.activation(out=yt[:], in_=ps[:], func=AF.Identity,
                                 bias=l2b[:], scale=1.0)
            nc.sync.dma_start(out=yout[:, :], in_=yt[:])

    nc.compile()
    return nc


def _wrap_idxs128(idx):
    """Per 2048-edge super-tile: first half -> partitions 0:63 (gpsimd cores
    0-3), second half -> partitions 64:127 (cores 4-7)."""
    n = idx.shape[0]
    cols = []
    for o in range(0, n, 2048):
        te = min(2048, n - o)
        h = te // 2
        a = idx[o:o + h].reshape(h // 16, 16).T.astype(np.int16)
        b = idx[o + h:o + te].reshape(h // 16, 16).T.astype(np.int16)
        cols.append(np.vstack([np.tile(a, (4, 1)), np.tile(b, (4, 1))]))
    return np.concatenate(cols, axis=1)


def _preprocess(inputs):
    x = np.asarray(inputs["x"], np.float32)
    ea = np.asarray(inputs["edge_attr"], np.float32)
    ei = np.asarray(inputs["edge_index"]).astype(np.int64)
    batch = np.asarray(inputs["batch"]).astype(np.int64)
    src, dst = ei[0], ei[1]

    cnt = np.bincount(dst, minlength=N).astype(np.float32)
    rc_node = 1.0 / np.maximum(cnt, 1.0)
    gcnt = np.bincount(batch, minlength=G).astype(np.float32)
    rgc = 1.0 / np.maximum(gcnt, 1.0)

    core = dst // SHARD
    order = np.argsort(dst, kind="stable")
    src_s, dst_s, ea_idx = src[order], dst[order], order
    core_s = core[order]

    # window id within core: local node // 128
    loc = dst_s - core_s * SHARD
    win = loc // 128

    # group edges by (core, window): edges are dst-sorted, so each (core,
    # window) is a contiguous run; find boundaries with searchsorted.
    bounds = []
    for c in range(NCORES):
        for w in range(NWIN):
            bounds.append(c * SHARD + min(w * 128, SHARD))
    bounds.append(N)
    bpos = np.searchsorted(dst_s, np.asarray(bounds), side="left")
    percw = {}
    maxcnt = 0
    k = 0
    for c in range(NCORES):
        for w in range(NWIN):
            lo, hi = bpos[k], bpos[k + 1]
            percw[(c, w)] = np.arange(lo, hi)
            maxcnt = max(maxcnt, hi - lo)
            k += 1
    cw = max(1, (maxcnt + 127) // 128)
    etot = NWIN * cw * 128

    per_core = []
    for c in range(NCORES):
        src_t = np.zeros(etot, np.int64)
        dl = np.full(etot, -1.0, np.float32)
        rc_e = np.ones(etot, np.float32)
        ea_e = np.zeros((etot, EF), np.float32)
        for w in range(NWIN):
            idxs = percw[(c, w)]
            o = w * cw * 128
            k = len(idxs)
            s = src_s[idxs]
            src_t[o:o + k] = (s // SHARD) * SHARD_P + (s % SHARD)
            dl[o:o + k] = (dst_s[idxs] - c * SHARD - w * 128).astype(np.float32)
            rc_e[o:o + k] = rc_node[dst_s[idxs]]
            ea_e[o:o + k] = ea[ea_idx[idxs]]
        eaT = np.ones((EF + 1, etot), np.float32)
        eaT[:EF] = ea_e.T
        eaT[EF, dl < 0] = 0.0
        nch = etot // 128
        d = {
            "sidx": _wrap_idxs128(src_t),
            "dstloc_p": dl.reshape(nch, 128).T.copy(),
            "dstloc_f": dl.reshape(1, etot),
            "rc_p": rc_e.reshape(nch, 128).T.copy(),
            "eaT": eaT.astype(ml_dtypes.bfloat16),
        }
        # node-side tables
        xp = np.zeros((NF, SHARD_P), np.float32)
        xp[:, :SHARD] = x[c * SHARD:(c + 1) * SHARD].T
        d["xT"] = xp
        bl = np.full(SHARD_P, -1.0, np.float32)
        bl[:SHARD] = batch[c * SHARD:(c + 1) * SHARD].astype(np.float32)
        rg_n = np.zeros(SHARD_P, np.float32)
        rg_n[:SHARD] = rgc[batch[c * SHARD:(c + 1) * SHARD]]
        d["batchloc"] = bl.reshape(NWIN, 128).T.copy()
        d["rgc_pn"] = rg_n.reshape(NWIN, 128).T.copy()
        per_core.append(d)

    # replicated weights
    wf = np.asarray(inputs["conv_wf"], np.float32)
    wsv = np.asarray(inputs["conv_ws"], np.float32)
    bf = np.asarray(inputs["conv_bf"], np.float32)
    bs = np.asarray(inputs["conv_bs"], np.float32)
    wdst = np.concatenate([wf[:, 0:D1, :], wsv[:, 0:D1, :]], axis=2)
    wsrc = np.concatenate([wf[:, D1:2 * D1, :], wsv[:, D1:2 * D1, :]], axis=2)
    wea = np.concatenate([wf[:, 2 * D1:, :], wsv[:, 2 * D1:, :]], axis=2)
    bias = np.concatenate([bf, bs], axis=1)[:, None, :]
    wea = np.concatenate([wea, bias], axis=1)
    shared = {
        "lin0w": np.asarray(inputs["lin0_w"], np.float32),
        "lin0b": np.asarray(inputs["lin0_b"], np.float32).reshape(D1, 1),
        "wdst": np.transpose(wdst, (1, 0, 2)).reshape(D1, L * 128).astype(np.float32),
        "wsrc": np.tile(np.transpose(wsrc, (1, 0, 2)).reshape(D1, L * 128).astype(np.float32), (2, 1)),
        "wea": np.transpose(wea, (1, 0, 2)).reshape(EF + 1, L * 128).astype(ml_dtypes.bfloat16),
        "bng": np.asarray(inputs["bn_gamma"], np.float32).T.copy(),
        "bnb": np.asarray(inputs["bn_beta"], np.float32).T.copy(),
        "lin1w": np.asarray(inputs["lin1_w"], np.float32),
        "lin1b": np.asarray(inputs["lin1_b"], np.float32).reshape(D2, 1),
        "fcw": np.transpose(np.asarray(inputs["fc_w"], np.float32), (1, 0, 2)).reshape(D2, FC * D2),
        "fcb": np.asarray(inputs["fc_b"], np.float32).T.copy(),
        "lin2w": np.asarray(inputs["lin2_w"], np.float32).reshape(D2, 1),
        "lin2b": np.asarray(inputs["lin2_b"], np.float32).reshape(1, 1),
        "iota128": np.broadcast_to(np.arange(128, dtype=np.float32)[None, :],
                                   (128, 128)).copy(),
        "iota256": np.broadcast_to(np.arange(G, dtype=np.float32)[None, :],
                                   (128, G)).copy(),
        "iotap": np.arange(128, dtype=np.float32).reshape(128, 1),
        "ident": np.eye(128, dtype=np.float32),
    }
    in_maps = [dict(shared, **pc) for pc in per_core]
    return in_maps, cw


def kernel(**inputs):
    from concourse.bass_utils import run_bass_kernel_spmd

    in_maps, cw = _preprocess(inputs)
    key = ("nc", cw)
    if key not in _CACHE:
        _CACHE[key] = _build_nc(cw)
    nc = _CACHE[key]
    res = run_bass_kernel_spmd(nc, in_maps, core_ids=list(range(NCORES)))
    return res.results[0]["y"].reshape(G).astype(np.float32)



# revision 12
# speedup vs baseline: 3.3060x; 3.3060x over previous
"""CGCNN message-passing kernel for 8 Trainium2 NeuronCores (Bass/Tile), v2.

Strategy (data-parallel by dst-node range, gather-based edge pipeline):
- Nodes split into 8 shards of 3750 (padded 3840 = 30 windows x 128). Edges
  assigned to the core owning dst, grouped by 128-node dst window, padded to a
  uniform chunks-per-window count cw (SPMD-uniform program).
- Per layer, per core:
  * Qd table (own shard)  = h_own  @ Wdst  -> DRAM [3840, 128] bf16
  * AllGather h (bf16), then Qs table (all nodes) = h_full @ Wsrc
    -> DRAM [30720, 128] bf16 (partition-major row order for fat DMA runs)
  * per 1024-edge tile: dma_gather Qd rows + Qs rows (1024 descriptors each),
    Qe = ea @ Wea as matmul, summed in PSUM via identity-matmul adds.
    f-gate columns are negated at preprocessing, so one joint exp pass gives
    u=[e^-a | e^b]; v=ln(1+u)=[sp(-a) | sp(b)]; sigmoid(a)=e^(-sp(-a));
    m = sigmoid * softplus (bf16).
  * segment-mean via onehot matmul (is_equal(iota,dst)*1/cnt, bf16) into PSUM
    accumulated per dst window; BatchNorm batch stats via tiny AllReduce;
    residual + relu on the own shard.
- Global mean pool via onehot matmul, partial sums AllReduced, tiny head MLP
  computed redundantly on every core.
"""
import numpy as np
import ml_dtypes

N = 30000
E = 480000
NF = 92
EF = 50
D1 = 64
D2 = 64
L = 3
FC = 2
G = 256
EPS = 1e-5
NCORES = 8
SHARD = N // NCORES            # 3750
SHARD_P = 3840                 # padded shard (30 windows of 128)
NWIN = SHARD_P // 128          # 30
NWING = NCORES * NWIN          # 240 global windows
TBL = NCORES * SHARD_P         # 30720 table rows

_CACHE = {}


def _build_nc(cw):
    """Build the SPMD bass module. cw = chunks per dst window (uniform)."""
    import concourse.mybir as mybir
    from concourse import bacc
    from concourse.tile import TileContext

    f32 = mybir.dt.float32
    bf16 = mybir.dt.bfloat16
    i16 = mybir.dt.int16
    AF = mybir.ActivationFunctionType
    OP = mybir.AluOpType

    nchunk = NWIN * cw                 # chunks per layer per core
    etot = nchunk * 128                # padded edges per core
    ntile = (nchunk + 7) // 8          # 8-chunk (1024-edge) tiles

    import concourse.hw_specs as _hw
    import concourse.bacc as _bacc_mod
    _real_tables = _hw.get_activation_tables("gen3")
    _combined = "natural_log_exp_and_others"
    if _combined in _real_tables:
        _patched = {
            k: (v if k == _combined else (v - {AF.Exp, AF.Ln}))
            for k, v in _real_tables.items()
        }
        _bacc_mod.get_activation_tables = lambda arch: _patched

    nc = bacc.Bacc(None, target_bir_lowering=False)

    # ---- inputs (per core) ----
    xT = nc.dram_tensor("xT", [NF, SHARD_P], bf16, kind="ExternalInput")
    eaT = nc.dram_tensor("eaT", [EF + 1, etot], bf16, kind="ExternalInput")
    qs_idxD = nc.dram_tensor("qs_idxD", [128, etot // 16], i16, kind="ExternalInput")
    qd_idxD = nc.dram_tensor("qd_idxD", [128, etot // 16], i16, kind="ExternalInput")
    dstloc_p = nc.dram_tensor("dstloc_p", [128, nchunk], f32, kind="ExternalInput")
    rc_p = nc.dram_tensor("rc_p", [128, nchunk], f32, kind="ExternalInput")
    batchloc = nc.dram_tensor("batchloc", [128, NWIN], f32, kind="ExternalInput")
    rgc_pn = nc.dram_tensor("rgc_pn", [128, NWIN], f32, kind="ExternalInput")
    # weights (replicated; f-gate halves pre-negated)
    lin0w = nc.dram_tensor("lin0w", [NF, D1], bf16, kind="ExternalInput")
    lin0b = nc.dram_tensor("lin0b", [D1, 1], f32, kind="ExternalInput")
    wdst = nc.dram_tensor("wdst", [D1, L * 128], bf16, kind="ExternalInput")
    wsrc = nc.dram_tensor("wsrc", [D1, L * 128], bf16, kind="ExternalInput")
    wea = nc.dram_tensor("wea", [EF + 1, L * 128], bf16, kind="ExternalInput")
    bng = nc.dram_tensor("bng", [D1, L], f32, kind="ExternalInput")
    bnb = nc.dram_tensor("bnb", [D1, L], f32, kind="ExternalInput")
    lin1w = nc.dram_tensor("lin1w", [D1, D2], f32, kind="ExternalInput")
    lin1b = nc.dram_tensor("lin1b", [D2, 1], f32, kind="ExternalInput")
    fcw = nc.dram_tensor("fcw", [D2, FC * D2], f32, kind="ExternalInput")
    fcb = nc.dram_tensor("fcb", [D2, FC], f32, kind="ExternalInput")
    lin2w = nc.dram_tensor("lin2w", [D2, 1], f32, kind="ExternalInput")
    lin2b = nc.dram_tensor("lin2b", [1, 1], f32, kind="ExternalInput")
    iota128 = nc.dram_tensor("iota128", [128, 128], bf16, kind="ExternalInput")
    iota256 = nc.dram_tensor("iota256", [128, G], bf16, kind="ExternalInput")
    ident = nc.dram_tensor("ident", [128, 128], f32, kind="ExternalInput")
    identb = nc.dram_tensor("identb", [128, 128], bf16, kind="ExternalInput")

    yout = nc.dram_tensor("y", [1, G], f32, kind="ExternalOutput")

    # ---- DRAM scratch ----
    QdD = nc.dram_tensor("QdD", [SHARD_P, 128], bf16)      # row p*NWIN+w
    QsD = nc.dram_tensor("QsD", [TBL, 128], bf16)          # row p*NWING+W
    ag_in = nc.dram_tensor("ag_in", [D1, SHARD_P], bf16)
    ag_out = nc.dram_tensor("ag_out", [NCORES * D1, SHARD_P], bf16,
                            addr_space="Shared")
    ar_in = nc.dram_tensor("ar_in", [D1, 16], f32)
    ar_out = nc.dram_tensor("ar_out", [D1, 16], f32, addr_space="Shared")
    pl_in = nc.dram_tensor("pl_in", [D1, G], f32)
    pl_out = nc.dram_tensor("pl_out", [D1, G], f32, addr_space="Shared")

    rg = [list(range(NCORES))]
    QdD3 = QdD[:, :].rearrange("(p w) f -> p w f", p=128)   # [128, NWIN, 128]
    QsD3 = QsD[:, :].rearrange("(p w) f -> p w f", p=128)   # [128, NWING, 128]

    with TileContext(nc) as tc:
        with (
            tc.tile_pool(name="const", bufs=1) as cp,
            tc.tile_pool(name="big", bufs=1) as bigp,
            tc.tile_pool(name="work", bufs=2) as wp,
            tc.tile_pool(name="gat", bufs=3) as gp,
            tc.tile_pool(name="nl", bufs=2) as nlp,
            tc.tile_pool(name="oh", bufs=2) as ohp,
            tc.tile_pool(name="st", bufs=2) as stp,
            tc.tile_pool(name="scr", bufs=1) as scp,
            tc.tile_pool(name="pre", bufs=2, space="PSUM") as pp,
            tc.tile_pool(name="psB", bufs=2, space="PSUM") as ppB,
            tc.tile_pool(name="psA", bufs=2, space="PSUM") as ppA,
        ):
            # ---------- constants ----------
            def load_const(t, dram, shape, dtype=f32):
                tt = cp.tile(shape, dtype, tag=t)
                nc.sync.dma_start(out=tt[:], in_=dram)
                return tt

            io128 = load_const("io128", iota128[:, :], [128, 128], bf16)
            io256 = load_const("io256", iota256[:, :], [128, G], bf16)
            idn = load_const("idn", ident[:, :], [128, 128])
            idnb = load_const("idnb", identb[:, :], [128, 128], bf16)
            l0w = load_const("l0w", lin0w[:, :], [NF, D1], bf16)
            l0b = load_const("l0b", lin0b[:, :], [D1, 1])
            wd = load_const("wd", wdst[:, :], [D1, L * 128], bf16)
            ws = load_const("ws", wsrc[:, :], [D1, L * 128], bf16)
            we = load_const("we", wea[:, :], [EF + 1, L * 128], bf16)
            gmt = load_const("gmt", bng[:, :], [D1, L])
            bbt = load_const("bbt", bnb[:, :], [D1, L])
            l1w = load_const("l1w", lin1w[:, :], [D1, D2])
            l1b = load_const("l1b", lin1b[:, :], [D2, 1])
            fw = load_const("fw", fcw[:, :], [D2, FC * D2])
            fb = load_const("fb", fcb[:, :], [D2, FC])
            l2w = load_const("l2w", lin2w[:, :], [D2, 1])
            l2b = load_const("l2b", lin2b[:, :], [1, 1])
            dlp = load_const("dlp", dstloc_p[:, :], [128, nchunk])
            rcp = load_const("rcp", rc_p[:, :], [128, nchunk])
            blc = load_const("blc", batchloc[:, :], [128, NWIN])
            rgp = load_const("rgp", rgc_pn[:, :], [128, NWIN])
            qsix = load_const("qsix", qs_idxD[:, :], [128, etot // 16], i16)
            qdix = load_const("qdix", qd_idxD[:, :], [128, etot // 16], i16)

            # ---------- resident state ----------
            hT_own = bigp.tile([D1, SHARD_P], f32, tag="hown")
            hb_own = bigp.tile([D1, SHARD_P], bf16, tag="hbown")
            aggr_sb = bigp.tile([D1, SHARD_P], f32, tag="aggr")

            # ---------- lin0: hT_own = relu(lin0w.T @ xT + b) ----------
            for j in range(8):
                sl = slice(j * 480, (j + 1) * 480)
                xt = wp.tile([NF, 480], bf16, tag="xt")
                nc.sync.dma_start(out=xt[:], in_=xT[:, sl])
                ph = ppB.tile([D1, 480], f32, tag="bld")
                nc.tensor.matmul(out=ph[:], lhsT=l0w[:], rhs=xt[:],
                                 start=True, stop=True)
                nc.scalar.activation(out=hT_own[:, sl], in_=ph[:],
                                     func=AF.Relu, bias=l0b[:], scale=1.0)

            # ---------- layers ----------
            for l in range(L):
                wd_l = wd[:, l * 128:(l + 1) * 128]
                ws_l = ws[:, l * 128:(l + 1) * 128]
                we_l = we[:, l * 128:(l + 1) * 128]

                # --- bf16 copy of own h ---
                nc.scalar.activation(out=hb_own[:], in_=hT_own[:],
                                     func=AF.Identity, scale=1.0)

                # --- Qd table build (own shard) -> QdD ---
                for w0 in range(0, NWIN, 4):
                    kk = min(4, NWIN - w0)
                    qp = ppB.tile([128, 512], f32, tag="bld")
                    for k in range(kk):
                        w = w0 + k
                        nc.tensor.matmul(
                            out=qp[:, k * 128:(k + 1) * 128],
                            lhsT=hb_own[:, w * 128:(w + 1) * 128],
                            rhs=wd_l, start=True, stop=True)
                    sg_t = stp.tile([128, 4, 128], bf16, tag="qdst")
                    nc.scalar.activation(
                        out=sg_t[:, :kk, :].rearrange("p a b -> p (a b)"),
                        in_=qp[:, :kk * 128],
                        func=AF.Identity, scale=1.0)
                    nc.sync.dma_start(out=QdD3[:, w0:w0 + kk, :],
                                      in_=sg_t[:, :kk, :])

                # --- AllGather h (bf16) ---
                nc.sync.dma_start(out=ag_in[:, :], in_=hb_own[:])
                nc.gpsimd.collective_compute(
                    "AllGather", OP.bypass, replica_groups=rg,
                    ins=[ag_in.ap().opt()], outs=[ag_out.ap().opt()])

                # --- Qs table build (all nodes, per gathered shard) -> QsD ---
                for s_ in range(NCORES):
                    hb_sh = stp.tile([D1, SHARD_P], bf16, tag="hbsh")
                    nc.sync.dma_start(out=hb_sh[:],
                                      in_=ag_out[s_ * D1:(s_ + 1) * D1, :])
                    for w0 in range(0, NWIN, 4):
                        kk = min(4, NWIN - w0)
                        qp = ppB.tile([128, 512], f32, tag="bld")
                        for k in range(kk):
                            w = w0 + k
                            nc.tensor.matmul(
                                out=qp[:, k * 128:(k + 1) * 128],
                                lhsT=hb_sh[:, w * 128:(w + 1) * 128],
                                rhs=ws_l, start=True, stop=True)
                        sg_t = stp.tile([128, 4, 128], bf16, tag="qsst")
                        nc.scalar.activation(
                            out=sg_t[:, :kk, :].rearrange("p a b -> p (a b)"),
                            in_=qp[:, :kk * 128],
                            func=AF.Identity, scale=1.0)
                        W0 = s_ * NWIN + w0
                        nc.sync.dma_start(out=QsD3[:, W0:W0 + kk, :],
                                          in_=sg_t[:, :kk, :])

                # --- edge pipeline ---
                agg = None
                for t in range(ntile):
                    te = min(8, nchunk - t * 8)          # chunks this tile
                    ne = te * 128                        # edges this tile
                    qs_g = gp.tile([128, 8, 128], bf16, tag="qsg")
                    nc.gpsimd.dma_gather(
                        qs_g[:, :te, :], QsD[:, :],
                        qsix[:, t * 64: t * 64 + te * 8], ne, ne, 128)
                    qd_g = gp.tile([128, 8, 128], bf16, tag="qdg")
                    nc.gpsimd.dma_gather(
                        qd_g[:, :te, :], QdD[:, :],
                        qdix[:, t * 64: t * 64 + te * 8], ne, ne, 128)
                    et = gp.tile([EF + 1, 1024], bf16, tag="et")
                    nc.sync.dma_start(out=et[:, :ne],
                                      in_=eaT[:, t * 1024: t * 1024 + ne])

                    pre = pp.tile([128, 1024], f32, tag="pre")
                    qs_f = qs_g[:].rearrange("p a b -> p (a b)")
                    qd_f = qd_g[:].rearrange("p a b -> p (a b)")
                    for c in range(te):
                        csl = slice(c * 128, (c + 1) * 128)
                        nc.tensor.matmul(out=pre[:, csl], lhsT=et[:, csl],
                                         rhs=we_l, start=True, stop=False)
                        nc.tensor.matmul(out=pre[:, csl], lhsT=idnb[:],
                                         rhs=qs_f[:, csl], start=False, stop=False)
                        nc.tensor.matmul(out=pre[:, csl], lhsT=idnb[:],
                                         rhs=qd_f[:, csl], start=False, stop=True)

                    u = nlp.tile([128, 1024], f32, tag="u")
                    nc.scalar.activation(out=u[:, :ne], in_=pre[:, :ne],
                                         func=AF.Exp, scale=1.0)
                    v = nlp.tile([128, 8, 128], f32, tag="v")
                    nc.scalar.activation(
                        out=v[:, :te, :].rearrange("p a b -> p (a b)"),
                        in_=u[:, :ne], func=AF.Ln, bias=1.0, scale=1.0)
                    sg = nlp.tile([128, 8, 64], f32, tag="sg")
                    nc.scalar.activation(out=sg[:, :te, :], in_=v[:, :te, 0:64],
                                         func=AF.Exp, scale=-1.0)
                    m = nlp.tile([128, 8, 64], bf16, tag="m")
                    nc.vector.tensor_tensor(out=m[:, :te, :], in0=sg[:, :te, :],
                                            in1=v[:, :te, 64:128], op=OP.mult)

                    for c in range(te):
                        gc = t * 8 + c
                        w = gc // cw
                        if gc % (4 * cw) == 0:
                            agg = ppA.tile([D1, 512], f32, tag="agg")
                        ohS = ohp.tile([128, 128], bf16, tag="ohS")
                        nc.vector.tensor_scalar(
                            out=ohS[:], in0=io128[:],
                            scalar1=dlp[:, gc:gc + 1], scalar2=rcp[:, gc:gc + 1],
                            op0=OP.is_equal, op1=OP.mult)
                        nc.tensor.matmul(
                            out=agg[:, (w % 4) * 128:(w % 4 + 1) * 128],
                            lhsT=m[:, c, :], rhs=ohS[:],
                            start=(gc % cw == 0), stop=(gc % cw == cw - 1))
                        if gc % (4 * cw) == 4 * cw - 1 or gc == nchunk - 1:
                            grp = w // 4
                            lo = grp * 512
                            hi = min(lo + 512, SHARD_P)
                            nc.scalar.activation(
                                out=aggr_sb[:, lo:hi], in_=agg[:, :hi - lo],
                                func=AF.Identity, scale=1.0)

                # --- BN stats + AllReduce ---
                st = wp.tile([D1, 16], f32, tag="st")
                nc.vector.reduce_sum(out=st[:, 0:1], in_=aggr_sb[:],
                                     axis=mybir.AxisListType.X)
                sq = scp.tile([D1, SHARD_P], f32, tag="sq")
                nc.vector.tensor_tensor(out=sq[:], in0=aggr_sb[:],
                                        in1=aggr_sb[:], op=OP.mult)
                nc.vector.reduce_sum(out=st[:, 1:2], in_=sq[:],
                                     axis=mybir.AxisListType.X)
                nc.sync.dma_start(out=ar_in[:, :], in_=st[:])
                nc.gpsimd.collective_compute(
                    "AllReduce", OP.add, replica_groups=rg,
                    ins=[ar_in.ap().opt()], outs=[ar_out.ap().opt()])
                stg = wp.tile([D1, 16], f32, tag="stg")
                nc.sync.dma_start(out=stg[:], in_=ar_out[:, :])
                mu = wp.tile([D1, 1], f32, tag="mu")
                nc.vector.tensor_scalar(out=mu[:], in0=stg[:, 0:1],
                                        scalar1=1.0 / N, scalar2=None, op0=OP.mult)
                ex2 = wp.tile([D1, 1], f32, tag="ex2")
                nc.vector.tensor_scalar(out=ex2[:], in0=stg[:, 1:2],
                                        scalar1=1.0 / N, scalar2=None, op0=OP.mult)
                mu2 = wp.tile([D1, 1], f32, tag="mu2")
                nc.vector.tensor_tensor(out=mu2[:], in0=mu[:], in1=mu[:], op=OP.mult)
                var = wp.tile([D1, 1], f32, tag="var")
                nc.vector.tensor_tensor(out=var[:], in0=ex2[:], in1=mu2[:],
                                        op=OP.subtract)
                ve = wp.tile([D1, 1], f32, tag="ve")
                nc.vector.tensor_scalar(out=ve[:], in0=var[:], scalar1=EPS,
                                        scalar2=None, op0=OP.add)
                lv = wp.tile([D1, 1], f32, tag="lv")
                nc.scalar.activation(out=lv[:], in_=ve[:], func=AF.Ln, scale=1.0)
                isd = wp.tile([D1, 1], f32, tag="isd")
                nc.scalar.activation(out=isd[:], in_=lv[:], func=AF.Exp, scale=-0.5)
                scale = wp.tile([D1, 1], f32, tag="scale")
                nc.vector.tensor_tensor(out=scale[:], in0=isd[:],
                                        in1=gmt[:, l:l + 1], op=OP.mult)
                mshift = wp.tile([D1, 1], f32, tag="mshift")
                nc.vector.tensor_tensor(out=mshift[:], in0=mu[:], in1=scale[:],
                                        op=OP.mult)
                shift = wp.tile([D1, 1], f32, tag="shift")
                nc.vector.tensor_tensor(out=shift[:], in0=bbt[:, l:l + 1],
                                        in1=mshift[:], op=OP.subtract)
                # h = relu(h + aggr*scale + shift)
                asb = scp.tile([D1, SHARD_P], f32, tag="asb")
                nc.vector.tensor_scalar(out=asb[:], in0=aggr_sb[:],
                                        scalar1=scale[:], scalar2=shift[:],
                                        op0=OP.mult, op1=OP.add)
                nc.vector.tensor_tensor(out=asb[:], in0=asb[:], in1=hT_own[:],
                                        op=OP.add)
                nc.vector.tensor_scalar(out=hT_own[:], in0=asb[:],
                                        scalar1=0.0, scalar2=None, op0=OP.max)

            # ---------- global mean pool ----------
            pool_ps = pp.tile([D1, G], f32, tag="pre")
            for w in range(NWIN):
                tp = ppB.tile([128, D1], f32, tag="bld")
                nc.tensor.transpose(out=tp[:], in_=hT_own[:, w * 128:(w + 1) * 128],
                                    identity=idn[0:D1, 0:D1])
                rows = wp.tile([128, D1], bf16, tag="rows")
                nc.vector.tensor_copy(out=rows[:], in_=tp[:])
                ohg = ohp.tile([128, G], bf16, tag="ohg")
                nc.vector.tensor_scalar(
                    out=ohg[:], in0=io256[:],
                    scalar1=blc[:, w:w + 1], scalar2=rgp[:, w:w + 1],
                    op0=OP.is_equal, op1=OP.mult)
                nc.tensor.matmul(out=pool_ps[:], lhsT=rows[:], rhs=ohg[:],
                                 start=(w == 0), stop=(w == NWIN - 1))
            poolT = wp.tile([D1, G], f32, tag="poolT")
            nc.vector.tensor_copy(out=poolT[:], in_=pool_ps[:])
            nc.sync.dma_start(out=pl_in[:, :], in_=poolT[:])
            nc.gpsimd.collective_compute(
                "AllReduce", OP.add, replica_groups=rg,
                ins=[pl_in.ap().opt()], outs=[pl_out.ap().opt()])
            pg = wp.tile([D1, G], f32, tag="pg")
            nc.sync.dma_start(out=pg[:], in_=pl_out[:, :])

            # ---------- head ----------
            a = pg
            hw_ = [(l1w[:], l1b[:]), (fw[:, 0:D2], fb[:, 0:1]), (fw[:, D2:2 * D2], fb[:, 1:2])]
            for (wt, bt) in hw_:
                ps = ppB.tile([D2, G], f32, tag="bld")
                nc.tensor.matmul(out=ps[:, 0:G], lhsT=wt, rhs=a[:], start=True, stop=True)
                an = wp.tile([D2, G], f32, tag="an")
                nc.scalar.activation(out=an[:], in_=ps[:, 0:G], func=AF.Relu,
                                     bias=bt, scale=1.0)
                a = an
            ps = ppB.tile([1, G], f32, tag="bld")
            nc.tensor.matmul(out=ps[:, 0:G], lhsT=l2w[:], rhs=a[:], start=True, stop=True)
            yt = wp.tile([1, G], f32, tag="yt")
            nc.scalar.activation(out=yt[:], in_=ps[:, 0:G], func=AF.Identity,
                                 bias=l2b[:], scale=1.0)
            nc.sync.dma_start(out=yout[:, :], in_=yt[:])

    nc.compile()
    return nc


def _wrap16(idx):
    """Flat idx list -> [128, n/16] int16: slot i at [i%16, i//16], replicated
    across the 8 Q7 cores."""
    a = idx.reshape(-1, 16).T.astype(np.int16)
    return np.tile(a, (8, 1))


def _preprocess(inputs):
    x = np.asarray(inputs["x"], np.float32)
    ea = np.asarray(inputs["edge_attr"], np.float32)
    ei = np.asarray(inputs["edge_index"]).astype(np.int64)
    batch = np.asarray(inputs["batch"]).astype(np.int64)
    src, dst = ei[0], ei[1]

    cnt = np.bincount(dst, minlength=N).astype(np.float32)
    rc_node = 1.0 / np.maximum(cnt, 1.0)
    gcnt = np.bincount(batch, minlength=G).astype(np.float32)
    rgc = 1.0 / np.maximum(gcnt, 1.0)

    order = np.argsort(dst, kind="stable")
    src_s, dst_s, ea_idx = src[order], dst[order], order
    core_s = dst_s // SHARD

    bounds = []
    for c in range(NCORES):
        for w in range(NWIN):
            bounds.append(c * SHARD + min(w * 128, SHARD))
    bounds.append(N)
    bpos = np.searchsorted(dst_s, np.asarray(bounds), side="left")
    percw = {}
    maxcnt = 0
    k = 0
    for c in range(NCORES):
        for w in range(NWIN):
            lo, hi = bpos[k], bpos[k + 1]
            percw[(c, w)] = np.arange(lo, hi)
            maxcnt = max(maxcnt, hi - lo)
            k += 1
    cw = max(1, (maxcnt + 127) // 128)
    etot = NWIN * cw * 128

    per_core = []
    for c in range(NCORES):
        qs_idx = np.zeros(etot, np.int64)
        qd_idx = np.zeros(etot, np.int64)
        dl = np.full(etot, -1.0, np.float32)
        rc_e = np.ones(etot, np.float32)
        ea_e = np.zeros((etot, EF), np.float32)
        for w in range(NWIN):
            idxs = percw[(c, w)]
            o = w * cw * 128
            k = len(idxs)
            s = src_s[idxs]
            g = (s // SHARD) * SHARD_P + (s % SHARD)   # padded global id
            qs_idx[o:o + k] = (g % 128) * NWING + (g // 128)
            loc = dst_s[idxs] - c * SHARD              # 0..3749
            qd_idx[o:o + k] = (loc % 128) * NWIN + (loc // 128)
            dl[o:o + k] = (loc - w * 128).astype(np.float32)
            rc_e[o:o + k] = rc_node[dst_s[idxs]]
            ea_e[o:o + k] = ea[ea_idx[idxs]]
        eaT = np.ones((EF + 1, etot), np.float32)
        eaT[:EF] = ea_e.T
        eaT[EF, dl < 0] = 0.0
        nch = etot // 128
        d = {
            "qs_idxD": _wrap16(qs_idx),
            "qd_idxD": _wrap16(qd_idx),
            "dstloc_p": dl.reshape(nch, 128).T.copy(),
            "rc_p": rc_e.reshape(nch, 128).T.copy(),
            "eaT": eaT.astype(ml_dtypes.bfloat16),
        }
        xp = np.zeros((NF, SHARD_P), np.float32)
        xp[:, :SHARD] = x[c * SHARD:(c + 1) * SHARD].T
        d["xT"] = xp.astype(ml_dtypes.bfloat16)
        bl = np.full(SHARD_P, -1.0, np.float32)
        bl[:SHARD] = batch[c * SHARD:(c + 1) * SHARD].astype(np.float32)
        rg_n = np.zeros(SHARD_P, np.float32)
        rg_n[:SHARD] = rgc[batch[c * SHARD:(c + 1) * SHARD]]
        d["batchloc"] = bl.reshape(NWIN, 128).T.copy()
        d["rgc_pn"] = rg_n.reshape(NWIN, 128).T.copy()
        per_core.append(d)

    # replicated weights; f-gate half negated so pre = [-a | b]
    wf = np.asarray(inputs["conv_wf"], np.float32)
    wsv = np.asarray(inputs["conv_ws"], np.float32)
    bf = np.asarray(inputs["conv_bf"], np.float32)
    bs = np.asarray(inputs["conv_bs"], np.float32)
    wdst = np.concatenate([-wf[:, 0:D1, :], wsv[:, 0:D1, :]], axis=2)
    wsrc = np.concatenate([-wf[:, D1:2 * D1, :], wsv[:, D1:2 * D1, :]], axis=2)
    wea = np.concatenate([-wf[:, 2 * D1:, :], wsv[:, 2 * D1:, :]], axis=2)
    bias = np.concatenate([-bf, bs], axis=1)[:, None, :]
    wea = np.concatenate([wea, bias], axis=1)
    shared = {
        "lin0w": np.asarray(inputs["lin0_w"], np.float32).astype(ml_dtypes.bfloat16),
        "lin0b": np.asarray(inputs["lin0_b"], np.float32).reshape(D1, 1),
        "wdst": np.transpose(wdst, (1, 0, 2)).reshape(D1, L * 128).astype(ml_dtypes.bfloat16),
        "wsrc": np.transpose(wsrc, (1, 0, 2)).reshape(D1, L * 128).astype(ml_dtypes.bfloat16),
        "wea": np.transpose(wea, (1, 0, 2)).reshape(EF + 1, L * 128).astype(ml_dtypes.bfloat16),
        "bng": np.asarray(inputs["bn_gamma"], np.float32).T.copy(),
        "bnb": np.asarray(inputs["bn_beta"], np.float32).T.copy(),
        "lin1w": np.asarray(inputs["lin1_w"], np.float32),
        "lin1b": np.asarray(inputs["lin1_b"], np.float32).reshape(D2, 1),
        "fcw": np.transpose(np.asarray(inputs["fc_w"], np.float32), (1, 0, 2)).reshape(D2, FC * D2),
        "fcb": np.asarray(inputs["fc_b"], np.float32).T.copy(),
        "lin2w": np.asarray(inputs["lin2_w"], np.float32).reshape(D2, 1),
        "lin2b": np.asarray(inputs["lin2_b"], np.float32).reshape(1, 1),
        "iota128": np.broadcast_to(np.arange(128, dtype=np.float32)[None, :],
                                   (128, 128)).astype(ml_dtypes.bfloat16),
        "iota256": np.broadcast_to(np.arange(G, dtype=np.float32)[None, :],
                                   (128, G)).astype(ml_dtypes.bfloat16),
        "ident": np.eye(128, dtype=np.float32),
        "identb": np.eye(128, dtype=np.float32).astype(ml_dtypes.bfloat16),
    }
    in_maps = [dict(shared, **pc) for pc in per_core]
    return in_maps, cw


def kernel(**inputs):
    from concourse.bass_utils import run_bass_kernel_spmd

    in_maps, cw = _preprocess(inputs)
    key = ("nc", cw)
    if key not in _CACHE:
        _CACHE[key] = _build_nc(cw)
    nc = _CACHE[key]
    res = run_bass_kernel_spmd(nc, in_maps, core_ids=list(range(NCORES)))
    return res.results[0]["y"].reshape(G).astype(np.float32)


# revision 15
# speedup vs baseline: 3.4011x; 1.0288x over previous
"""CGCNN message-passing kernel for 8 Trainium2 NeuronCores (Bass/Tile), v2.

Strategy (data-parallel by dst-node range, gather-based edge pipeline):
- Nodes split into 8 shards of 3750 (padded 3840 = 30 windows x 128). Edges
  assigned to the core owning dst, grouped by 128-node dst window, padded to a
  uniform chunks-per-window count cw (SPMD-uniform program).
- Per layer, per core:
  * Qd table (own shard)  = h_own  @ Wdst  -> DRAM [3840, 128] bf16
  * AllGather h (bf16), then Qs table (all nodes) = h_full @ Wsrc
    -> DRAM [30720, 128] bf16 (partition-major row order for fat DMA runs)
  * per 1024-edge tile: dma_gather Qd rows + Qs rows (1024 descriptors each),
    Qe = ea @ Wea as matmul, summed in PSUM via identity-matmul adds.
    f-gate columns are negated at preprocessing, so one joint exp pass gives
    u=[e^-a | e^b]; v=ln(1+u)=[sp(-a) | sp(b)]; sigmoid(a)=e^(-sp(-a));
    m = sigmoid * softplus (bf16).
  * segment-mean via onehot matmul (is_equal(iota,dst)*1/cnt, bf16) into PSUM
    accumulated per dst window; BatchNorm batch stats via tiny AllReduce;
    residual + relu on the own shard.
- Global mean pool via onehot matmul, partial sums AllReduced, tiny head MLP
  computed redundantly on every core.
"""
import numpy as np
import ml_dtypes

N = 30000
E = 480000
NF = 92
EF = 50
D1 = 64
D2 = 64
L = 3
FC = 2
G = 256
EPS = 1e-5
NCORES = 8
SHARD = N // NCORES            # 3750
SHARD_P = 3840                 # padded shard (30 windows of 128)
NWIN = SHARD_P // 128          # 30
NWING = NCORES * NWIN          # 240 global windows
TBL = NCORES * SHARD_P         # 30720 table rows

_CACHE = {}


def _build_nc(cw):
    """Build the SPMD bass module. cw = chunks per dst window (uniform)."""
    import concourse.mybir as mybir
    from concourse import bacc
    from concourse.tile import TileContext

    f32 = mybir.dt.float32
    bf16 = mybir.dt.bfloat16
    i16 = mybir.dt.int16
    AF = mybir.ActivationFunctionType
    OP = mybir.AluOpType

    nchunk = NWIN * cw                 # chunks per layer per core
    etot = nchunk * 128                # padded edges per core
    ntile = (nchunk + 7) // 8          # 8-chunk (1024-edge) tiles

    import concourse.hw_specs as _hw
    import concourse.bacc as _bacc_mod
    _real_tables = _hw.get_activation_tables("gen3")
    _combined = "natural_log_exp_and_others"
    if _combined in _real_tables:
        _patched = {
            k: (v if k == _combined else (v - {AF.Exp, AF.Ln}))
            for k, v in _real_tables.items()
        }
        _bacc_mod.get_activation_tables = lambda arch: _patched

    nc = bacc.Bacc(None, target_bir_lowering=False)

    # ---- inputs (per core) ----
    xT = nc.dram_tensor("xT", [NF, SHARD_P], bf16, kind="ExternalInput")
    eaT = nc.dram_tensor("eaT", [EF + 1, etot], bf16, kind="ExternalInput")
    qs_idxD = nc.dram_tensor("qs_idxD", [128, etot // 16], i16, kind="ExternalInput")
    dlfD = nc.dram_tensor("dlfD", [1, etot], bf16, kind="ExternalInput")
    onesD = nc.dram_tensor("onesD", [1, 128], bf16, kind="ExternalInput")
    iotapD = nc.dram_tensor("iotapD", [128, 1], f32, kind="ExternalInput")
    dstloc_p = nc.dram_tensor("dstloc_p", [128, nchunk], f32, kind="ExternalInput")
    rc_p = nc.dram_tensor("rc_p", [128, nchunk], f32, kind="ExternalInput")
    batchloc = nc.dram_tensor("batchloc", [128, NWIN], f32, kind="ExternalInput")
    rgc_pn = nc.dram_tensor("rgc_pn", [128, NWIN], f32, kind="ExternalInput")
    # weights (replicated; f-gate halves pre-negated)
    lin0w = nc.dram_tensor("lin0w", [NF, D1], bf16, kind="ExternalInput")
    lin0b = nc.dram_tensor("lin0b", [D1, 1], f32, kind="ExternalInput")
    wdst = nc.dram_tensor("wdst", [D1, L * 128], bf16, kind="ExternalInput")
    wsrc = nc.dram_tensor("wsrc", [D1, L * 128], bf16, kind="ExternalInput")
    wea = nc.dram_tensor("wea", [EF + 1, L * 128], bf16, kind="ExternalInput")
    bng = nc.dram_tensor("bng", [D1, L], f32, kind="ExternalInput")
    bnb = nc.dram_tensor("bnb", [D1, L], f32, kind="ExternalInput")
    lin1w = nc.dram_tensor("lin1w", [D1, D2], f32, kind="ExternalInput")
    lin1b = nc.dram_tensor("lin1b", [D2, 1], f32, kind="ExternalInput")
    fcw = nc.dram_tensor("fcw", [D2, FC * D2], f32, kind="ExternalInput")
    fcb = nc.dram_tensor("fcb", [D2, FC], f32, kind="ExternalInput")
    lin2w = nc.dram_tensor("lin2w", [D2, 1], f32, kind="ExternalInput")
    lin2b = nc.dram_tensor("lin2b", [1, 1], f32, kind="ExternalInput")
    iota128 = nc.dram_tensor("iota128", [128, 128], bf16, kind="ExternalInput")
    iota256 = nc.dram_tensor("iota256", [128, G], bf16, kind="ExternalInput")
    ident = nc.dram_tensor("ident", [128, 128], f32, kind="ExternalInput")
    identb = nc.dram_tensor("identb", [128, 128], bf16, kind="ExternalInput")

    yout = nc.dram_tensor("y", [1, G], f32, kind="ExternalOutput")

    # ---- DRAM scratch ----
    QsD = nc.dram_tensor("QsD", [TBL, 128], bf16)          # row p*NWING+W
    ag_in = nc.dram_tensor("ag_in", [D1, SHARD_P], bf16)
    ag_out = nc.dram_tensor("ag_out", [NCORES * D1, SHARD_P], bf16,
                            addr_space="Shared")
    ar_in = nc.dram_tensor("ar_in", [D1, 16], f32)
    ar_out = nc.dram_tensor("ar_out", [D1, 16], f32, addr_space="Shared")
    pl_in = nc.dram_tensor("pl_in", [D1, G], f32)
    pl_out = nc.dram_tensor("pl_out", [D1, G], f32, addr_space="Shared")

    rg = [list(range(NCORES))]
    QsD3 = QsD[:, :].rearrange("(p w) f -> p w f", p=128)   # [128, NWING, 128]

    with TileContext(nc) as tc:
        with (
            tc.tile_pool(name="const", bufs=1) as cp,
            tc.tile_pool(name="big", bufs=1) as bigp,
            tc.tile_pool(name="work", bufs=2) as wp,
            tc.tile_pool(name="gat", bufs=3) as gp,
            tc.tile_pool(name="nl", bufs=2) as nlp,
            tc.tile_pool(name="oh", bufs=2) as ohp,
            tc.tile_pool(name="st", bufs=2) as stp,
            tc.tile_pool(name="scr", bufs=1) as scp,
            tc.tile_pool(name="pre", bufs=2, space="PSUM") as pp,
            tc.tile_pool(name="psB", bufs=2, space="PSUM") as ppB,
            tc.tile_pool(name="psA", bufs=2, space="PSUM") as ppA,
        ):
            # ---------- constants ----------
            def load_const(t, dram, shape, dtype=f32):
                tt = cp.tile(shape, dtype, tag=t)
                nc.sync.dma_start(out=tt[:], in_=dram)
                return tt

            io128 = load_const("io128", iota128[:, :], [128, 128], bf16)
            io256 = load_const("io256", iota256[:, :], [128, G], bf16)
            idn = load_const("idn", ident[:, :], [128, 128])
            idnb = load_const("idnb", identb[:, :], [128, 128], bf16)
            l0w = load_const("l0w", lin0w[:, :], [NF, D1], bf16)
            l0b = load_const("l0b", lin0b[:, :], [D1, 1])
            wd = load_const("wd", wdst[:, :], [D1, L * 128], bf16)
            ws = load_const("ws", wsrc[:, :], [D1, L * 128], bf16)
            we = load_const("we", wea[:, :], [EF + 1, L * 128], bf16)
            gmt = load_const("gmt", bng[:, :], [D1, L])
            bbt = load_const("bbt", bnb[:, :], [D1, L])
            l1w = load_const("l1w", lin1w[:, :], [D1, D2])
            l1b = load_const("l1b", lin1b[:, :], [D2, 1])
            fw = load_const("fw", fcw[:, :], [D2, FC * D2])
            fb = load_const("fb", fcb[:, :], [D2, FC])
            l2w = load_const("l2w", lin2w[:, :], [D2, 1])
            l2b = load_const("l2b", lin2b[:, :], [1, 1])
            dlp = load_const("dlp", dstloc_p[:, :], [128, nchunk])
            rcp = load_const("rcp", rc_p[:, :], [128, nchunk])
            blc = load_const("blc", batchloc[:, :], [128, NWIN])
            rgp = load_const("rgp", rgc_pn[:, :], [128, NWIN])
            qsix = load_const("qsix", qs_idxD[:, :], [128, etot // 16], i16)
            onesb = load_const("onesb", onesD[:, :], [1, 128], bf16)
            iotap = load_const("iotap", iotapD[:, :], [128, 1])

            # ---------- resident state ----------
            hT_own = bigp.tile([D1, SHARD_P], f32, tag="hown")
            hb_own = bigp.tile([D1, SHARD_P], bf16, tag="hbown")
            aggr_sb = bigp.tile([D1, SHARD_P], f32, tag="aggr")
            qd_sb = bigp.tile([128, NWIN, 128], bf16, tag="qdsb")

            # ---------- lin0: hT_own = relu(lin0w.T @ xT + b) ----------
            for j in range(8):
                sl = slice(j * 480, (j + 1) * 480)
                xt = wp.tile([NF, 480], bf16, tag="xt")
                nc.sync.dma_start(out=xt[:], in_=xT[:, sl])
                ph = ppB.tile([D1, 480], f32, tag="bld")
                nc.tensor.matmul(out=ph[:], lhsT=l0w[:], rhs=xt[:],
                                 start=True, stop=True)
                nc.scalar.activation(out=hT_own[:, sl], in_=ph[:],
                                     func=AF.Relu, bias=l0b[:], scale=1.0)

            # ---------- layers ----------
            for l in range(L):
                wd_l = wd[:, l * 128:(l + 1) * 128]
                ws_l = ws[:, l * 128:(l + 1) * 128]
                we_l = we[:, l * 128:(l + 1) * 128]

                # --- bf16 copy of own h ---
                nc.scalar.activation(out=hb_own[:], in_=hT_own[:],
                                     func=AF.Identity, scale=1.0)

                # --- Qd table build (own shard) -> QdD ---
                for w0 in range(0, NWIN, 4):
                    kk = min(4, NWIN - w0)
                    qp = ppB.tile([128, 512], f32, tag="bld")
                    for k in range(kk):
                        w = w0 + k
                        nc.tensor.matmul(
                            out=qp[:, k * 128:(k + 1) * 128],
                            lhsT=hb_own[:, w * 128:(w + 1) * 128],
                            rhs=wd_l, start=True, stop=True)
                    nc.scalar.activation(
                        out=qd_sb[:, w0:w0 + kk, :].rearrange("p a b -> p (a b)"),
                        in_=qp[:, :kk * 128],
                        func=AF.Identity, scale=1.0)

                # --- AllGather h (bf16) ---
                nc.sync.dma_start(out=ag_in[:, :], in_=hb_own[:])
                nc.gpsimd.collective_compute(
                    "AllGather", OP.bypass, replica_groups=rg,
                    ins=[ag_in.ap().opt()], outs=[ag_out.ap().opt()])

                # --- Qs table build (all nodes, per gathered shard) -> QsD ---
                for s_ in range(NCORES):
                    hb_sh = stp.tile([D1, SHARD_P], bf16, tag="hbsh")
                    nc.sync.dma_start(out=hb_sh[:],
                                      in_=ag_out[s_ * D1:(s_ + 1) * D1, :])
                    for w0 in range(0, NWIN, 4):
                        kk = min(4, NWIN - w0)
                        qp = ppB.tile([128, 512], f32, tag="bld")
                        for k in range(kk):
                            w = w0 + k
                            nc.tensor.matmul(
                                out=qp[:, k * 128:(k + 1) * 128],
                                lhsT=hb_sh[:, w * 128:(w + 1) * 128],
                                rhs=ws_l, start=True, stop=True)
                        sg_t = stp.tile([128, 4, 128], bf16, tag="qsst")
                        nc.scalar.activation(
                            out=sg_t[:, :kk, :].rearrange("p a b -> p (a b)"),
                            in_=qp[:, :kk * 128],
                            func=AF.Identity, scale=1.0)
                        W0 = s_ * NWIN + w0
                        nc.sync.dma_start(out=QsD3[:, W0:W0 + kk, :],
                                          in_=sg_t[:, :kk, :])

                # --- edge pipeline ---
                agg = None
                for t in range(ntile):
                    te = min(8, nchunk - t * 8)          # chunks this tile
                    ne = te * 128                        # edges this tile
                    qs_g = gp.tile([128, 8, 128], bf16, tag="qsg")
                    nc.gpsimd.dma_gather(
                        qs_g[:, :te, :], QsD[:, :],
                        qsix[:, t * 64: t * 64 + te * 8], ne, ne, 128)
                    dlf = gp.tile([1, 1024], bf16, tag="dlf")
                    nc.sync.dma_start(out=dlf[:, :ne],
                                      in_=dlfD[:, t * 1024: t * 1024 + ne])
                    ohT = ohp.tile([128, 1024], bf16, tag="ohT")
                    for h0 in range(0, ne, 512):
                        hn = min(512, ne - h0)
                        bc = ppB.tile([128, 512], f32, tag="bld")
                        nc.tensor.matmul(out=bc[:, :hn], lhsT=onesb[:],
                                         rhs=dlf[:, h0:h0 + hn],
                                         start=True, stop=True)
                        nc.vector.tensor_scalar(
                            out=ohT[:, h0:h0 + hn], in0=bc[:, :hn],
                            scalar1=iotap[:], scalar2=None, op0=OP.is_equal)
                    et = gp.tile([EF + 1, 1024], bf16, tag="et")
                    nc.sync.dma_start(out=et[:, :ne],
                                      in_=eaT[:, t * 1024: t * 1024 + ne])

                    pre = pp.tile([128, 1024], f32, tag="pre")
                    qs_f = qs_g[:].rearrange("p a b -> p (a b)")
                    for c in range(te):
                        gc = t * 8 + c
                        w = gc // cw
                        csl = slice(c * 128, (c + 1) * 128)
                        nc.tensor.matmul(out=pre[:, csl], lhsT=et[:, csl],
                                         rhs=we_l, start=True, stop=False)
                        nc.tensor.matmul(out=pre[:, csl], lhsT=idnb[:],
                                         rhs=qs_f[:, csl], start=False, stop=False)
                        nc.tensor.matmul(out=pre[:, csl], lhsT=ohT[:, csl],
                                         rhs=qd_sb[:, w, :], start=False, stop=True)

                    u = nlp.tile([128, 1024], f32, tag="u")
                    nc.scalar.activation(out=u[:, :ne], in_=pre[:, :ne],
                                         func=AF.Exp, scale=1.0)
                    v = nlp.tile([128, 8, 128], f32, tag="v")
                    nc.scalar.activation(
                        out=v[:, :te, :].rearrange("p a b -> p (a b)"),
                        in_=u[:, :ne], func=AF.Ln, bias=1.0, scale=1.0)
                    sg = nlp.tile([128, 8, 64], f32, tag="sg")
                    nc.scalar.activation(out=sg[:, :te, :], in_=v[:, :te, 0:64],
                                         func=AF.Exp, scale=-1.0)
                    m = nlp.tile([128, 8, 64], bf16, tag="m")
                    nc.vector.tensor_tensor(out=m[:, :te, :], in0=sg[:, :te, :],
                                            in1=v[:, :te, 64:128], op=OP.mult)

                    for c in range(te):
                        gc = t * 8 + c
                        w = gc // cw
                        if gc % (4 * cw) == 0:
                            agg = ppA.tile([D1, 512], f32, tag="agg")
                        ohS = ohp.tile([128, 128], bf16, tag="ohS")
                        nc.vector.tensor_scalar(
                            out=ohS[:], in0=io128[:],
                            scalar1=dlp[:, gc:gc + 1], scalar2=rcp[:, gc:gc + 1],
                            op0=OP.is_equal, op1=OP.mult)
                        nc.tensor.matmul(
                            out=agg[:, (w % 4) * 128:(w % 4 + 1) * 128],
                            lhsT=m[:, c, :], rhs=ohS[:],
                            start=(gc % cw == 0), stop=(gc % cw == cw - 1))
                        if gc % (4 * cw) == 4 * cw - 1 or gc == nchunk - 1:
                            grp = w // 4
                            lo = grp * 512
                            hi = min(lo + 512, SHARD_P)
                            nc.scalar.activation(
                                out=aggr_sb[:, lo:hi], in_=agg[:, :hi - lo],
                                func=AF.Identity, scale=1.0)

                # --- BN stats + AllReduce ---
                st = wp.tile([D1, 16], f32, tag="st")
                nc.vector.reduce_sum(out=st[:, 0:1], in_=aggr_sb[:],
                                     axis=mybir.AxisListType.X)
                sq = scp.tile([D1, SHARD_P], f32, tag="sq")
                nc.vector.tensor_tensor(out=sq[:], in0=aggr_sb[:],
                                        in1=aggr_sb[:], op=OP.mult)
                nc.vector.reduce_sum(out=st[:, 1:2], in_=sq[:],
                                     axis=mybir.AxisListType.X)
                nc.sync.dma_start(out=ar_in[:, :], in_=st[:])
                nc.gpsimd.collective_compute(
                    "AllReduce", OP.add, replica_groups=rg,
                    ins=[ar_in.ap().opt()], outs=[ar_out.ap().opt()])
                stg = wp.tile([D1, 16], f32, tag="stg")
                nc.sync.dma_start(out=stg[:], in_=ar_out[:, :])
                mu = wp.tile([D1, 1], f32, tag="mu")
                nc.vector.tensor_scalar(out=mu[:], in0=stg[:, 0:1],
                                        scalar1=1.0 / N, scalar2=None, op0=OP.mult)
                ex2 = wp.tile([D1, 1], f32, tag="ex2")
                nc.vector.tensor_scalar(out=ex2[:], in0=stg[:, 1:2],
                                        scalar1=1.0 / N, scalar2=None, op0=OP.mult)
                mu2 = wp.tile([D1, 1], f32, tag="mu2")
                nc.vector.tensor_tensor(out=mu2[:], in0=mu[:], in1=mu[:], op=OP.mult)
                var = wp.tile([D1, 1], f32, tag="var")
                nc.vector.tensor_tensor(out=var[:], in0=ex2[:], in1=mu2[:],
                                        op=OP.subtract)
                ve = wp.tile([D1, 1], f32, tag="ve")
                nc.vector.tensor_scalar(out=ve[:], in0=var[:], scalar1=EPS,
                                        scalar2=None, op0=OP.add)
                lv = wp.tile([D1, 1], f32, tag="lv")
                nc.scalar.activation(out=lv[:], in_=ve[:], func=AF.Ln, scale=1.0)
                isd = wp.tile([D1, 1], f32, tag="isd")
                nc.scalar.activation(out=isd[:], in_=lv[:], func=AF.Exp, scale=-0.5)
                scale = wp.tile([D1, 1], f32, tag="scale")
                nc.vector.tensor_tensor(out=scale[:], in0=isd[:],
                                        in1=gmt[:, l:l + 1], op=OP.mult)
                mshift = wp.tile([D1, 1], f32, tag="mshift")
                nc.vector.tensor_tensor(out=mshift[:], in0=mu[:], in1=scale[:],
                                        op=OP.mult)
                shift = wp.tile([D1, 1], f32, tag="shift")
                nc.vector.tensor_tensor(out=shift[:], in0=bbt[:, l:l + 1],
                                        in1=mshift[:], op=OP.subtract)
                # h = relu(h + aggr*scale + shift)
                asb = scp.tile([D1, SHARD_P], f32, tag="asb")
                nc.vector.tensor_scalar(out=asb[:], in0=aggr_sb[:],
                                        scalar1=scale[:], scalar2=shift[:],
                                        op0=OP.mult, op1=OP.add)
                nc.vector.tensor_tensor(out=asb[:], in0=asb[:], in1=hT_own[:],
                                        op=OP.add)
                nc.vector.tensor_scalar(out=hT_own[:], in0=asb[:],
                                        scalar1=0.0, scalar2=None, op0=OP.max)

            # ---------- global mean pool ----------
            pool_ps = pp.tile([D1, G], f32, tag="pre")
            for w in range(NWIN):
                tp = ppB.tile([128, D1], f32, tag="bld")
                nc.tensor.transpose(out=tp[:], in_=hT_own[:, w * 128:(w + 1) * 128],
                                    identity=idn[0:D1, 0:D1])
                rows = wp.tile([128, D1], bf16, tag="rows")
                nc.vector.tensor_copy(out=rows[:], in_=tp[:])
                ohg = ohp.tile([128, G], bf16, tag="ohg")
                nc.vector.tensor_scalar(
                    out=ohg[:], in0=io256[:],
                    scalar1=blc[:, w:w + 1], scalar2=rgp[:, w:w + 1],
                    op0=OP.is_equal, op1=OP.mult)
                nc.tensor.matmul(out=pool_ps[:], lhsT=rows[:], rhs=ohg[:],
                                 start=(w == 0), stop=(w == NWIN - 1))
            poolT = wp.tile([D1, G], f32, tag="poolT")
            nc.vector.tensor_copy(out=poolT[:], in_=pool_ps[:])
            nc.sync.dma_start(out=pl_in[:, :], in_=poolT[:])
            nc.gpsimd.collective_compute(
                "AllReduce", OP.add, replica_groups=rg,
                ins=[pl_in.ap().opt()], outs=[pl_out.ap().opt()])
            pg = wp.tile([D1, G], f32, tag="pg")
            nc.sync.dma_start(out=pg[:], in_=pl_out[:, :])

            # ---------- head ----------
            a = pg
            hw_ = [(l1w[:], l1b[:]), (fw[:, 0:D2], fb[:, 0:1]), (fw[:, D2:2 * D2], fb[:, 1:2])]
            for (wt, bt) in hw_:
                ps = ppB.tile([D2, G], f32, tag="bld")
                nc.tensor.matmul(out=ps[:, 0:G], lhsT=wt, rhs=a[:], start=True, stop=True)
                an = wp.tile([D2, G], f32, tag="an")
                nc.scalar.activation(out=an[:], in_=ps[:, 0:G], func=AF.Relu,
                                     bias=bt, scale=1.0)
                a = an
            ps = ppB.tile([1, G], f32, tag="bld")
            nc.tensor.matmul(out=ps[:, 0:G], lhsT=l2w[:], rhs=a[:], start=True, stop=True)
            yt = wp.tile([1, G], f32, tag="yt")
            nc.scalar.activation(out=yt[:], in_=ps[:, 0:G], func=AF.Identity,
                                 bias=l2b[:], scale=1.0)
            nc.sync.dma_start(out=yout[:, :], in_=yt[:])

    nc.compile()
    return nc


def _wrap16(idx):
    """Flat idx list -> [128, n/16] int16: slot i at [i%16, i//16], replicated
    across the 8 Q7 cores."""
    a = idx.reshape(-1, 16).T.astype(np.int16)
    return np.tile(a, (8, 1))


def _preprocess(inputs):
    x = np.asarray(inputs["x"], np.float32)
    ea = np.asarray(inputs["edge_attr"], np.float32)
    ei = np.asarray(inputs["edge_index"]).astype(np.int64)
    batch = np.asarray(inputs["batch"]).astype(np.int64)
    src, dst = ei[0], ei[1]

    cnt = np.bincount(dst, minlength=N).astype(np.float32)
    rc_node = 1.0 / np.maximum(cnt, 1.0)
    gcnt = np.bincount(batch, minlength=G).astype(np.float32)
    rgc = 1.0 / np.maximum(gcnt, 1.0)

    order = np.argsort(dst, kind="stable")
    src_s, dst_s, ea_idx = src[order], dst[order], order
    core_s = dst_s // SHARD

    bounds = []
    for c in range(NCORES):
        for w in range(NWIN):
            bounds.append(c * SHARD + min(w * 128, SHARD))
    bounds.append(N)
    bpos = np.searchsorted(dst_s, np.asarray(bounds), side="left")
    percw = {}
    maxcnt = 0
    k = 0
    for c in range(NCORES):
        for w in range(NWIN):
            lo, hi = bpos[k], bpos[k + 1]
            percw[(c, w)] = np.arange(lo, hi)
            maxcnt = max(maxcnt, hi - lo)
            k += 1
    cw = max(1, (maxcnt + 127) // 128)
    etot = NWIN * cw * 128

    per_core = []
    for c in range(NCORES):
        qs_idx = np.zeros(etot, np.int64)
        dl = np.full(etot, -1.0, np.float32)
        rc_e = np.ones(etot, np.float32)
        ea_e = np.zeros((etot, EF), np.float32)
        for w in range(NWIN):
            idxs = percw[(c, w)]
            o = w * cw * 128
            k = len(idxs)
            s = src_s[idxs]
            g = (s // SHARD) * SHARD_P + (s % SHARD)   # padded global id
            qs_idx[o:o + k] = (g % 128) * NWING + (g // 128)
            loc = dst_s[idxs] - c * SHARD              # 0..3749
            dl[o:o + k] = (loc - w * 128).astype(np.float32)
            rc_e[o:o + k] = rc_node[dst_s[idxs]]
            ea_e[o:o + k] = ea[ea_idx[idxs]]
        eaT = np.ones((EF + 1, etot), np.float32)
        eaT[:EF] = ea_e.T
        eaT[EF, dl < 0] = 0.0
        nch = etot // 128
        d = {
            "qs_idxD": _wrap16(qs_idx),
            "dlfD": dl.reshape(1, etot).astype(ml_dtypes.bfloat16),
            "dstloc_p": dl.reshape(nch, 128).T.copy(),
            "rc_p": rc_e.reshape(nch, 128).T.copy(),
            "eaT": eaT.astype(ml_dtypes.bfloat16),
        }
        xp = np.zeros((NF, SHARD_P), np.float32)
        xp[:, :SHARD] = x[c * SHARD:(c + 1) * SHARD].T
        d["xT"] = xp.astype(ml_dtypes.bfloat16)
        bl = np.full(SHARD_P, -1.0, np.float32)
        bl[:SHARD] = batch[c * SHARD:(c + 1) * SHARD].astype(np.float32)
        rg_n = np.zeros(SHARD_P, np.float32)
        rg_n[:SHARD] = rgc[batch[c * SHARD:(c + 1) * SHARD]]
        d["batchloc"] = bl.reshape(NWIN, 128).T.copy()
        d["rgc_pn"] = rg_n.reshape(NWIN, 128).T.copy()
        per_core.append(d)

    # replicated weights; f-gate half negated so pre = [-a | b]
    wf = np.asarray(inputs["conv_wf"], np.float32)
    wsv = np.asarray(inputs["conv_ws"], np.float32)
    bf = np.asarray(inputs["conv_bf"], np.float32)
    bs = np.asarray(inputs["conv_bs"], np.float32)
    wdst = np.concatenate([-wf[:, 0:D1, :], wsv[:, 0:D1, :]], axis=2)
    wsrc = np.concatenate([-wf[:, D1:2 * D1, :], wsv[:, D1:2 * D1, :]], axis=2)
    wea = np.concatenate([-wf[:, 2 * D1:, :], wsv[:, 2 * D1:, :]], axis=2)
    bias = np.concatenate([-bf, bs], axis=1)[:, None, :]
    wea = np.concatenate([wea, bias], axis=1)
    shared = {
        "lin0w": np.asarray(inputs["lin0_w"], np.float32).astype(ml_dtypes.bfloat16),
        "lin0b": np.asarray(inputs["lin0_b"], np.float32).reshape(D1, 1),
        "wdst": np.transpose(wdst, (1, 0, 2)).reshape(D1, L * 128).astype(ml_dtypes.bfloat16),
        "wsrc": np.transpose(wsrc, (1, 0, 2)).reshape(D1, L * 128).astype(ml_dtypes.bfloat16),
        "wea": np.transpose(wea, (1, 0, 2)).reshape(EF + 1, L * 128).astype(ml_dtypes.bfloat16),
        "bng": np.asarray(inputs["bn_gamma"], np.float32).T.copy(),
        "bnb": np.asarray(inputs["bn_beta"], np.float32).T.copy(),
        "lin1w": np.asarray(inputs["lin1_w"], np.float32),
        "lin1b": np.asarray(inputs["lin1_b"], np.float32).reshape(D2, 1),
        "fcw": np.transpose(np.asarray(inputs["fc_w"], np.float32), (1, 0, 2)).reshape(D2, FC * D2),
        "fcb": np.asarray(inputs["fc_b"], np.float32).T.copy(),
        "lin2w": np.asarray(inputs["lin2_w"], np.float32).reshape(D2, 1),
        "lin2b": np.asarray(inputs["lin2_b"], np.float32).reshape(1, 1),
        "iota128": np.broadcast_to(np.arange(128, dtype=np.float32)[None, :],
                                   (128, 128)).astype(ml_dtypes.bfloat16),
        "iota256": np.broadcast_to(np.arange(G, dtype=np.float32)[None, :],
                                   (128, G)).astype(ml_dtypes.bfloat16),
        "ident": np.eye(128, dtype=np.float32),
        "identb": np.eye(128, dtype=np.float32).astype(ml_dtypes.bfloat16),
        "onesD": np.ones((1, 128), np.float32).astype(ml_dtypes.bfloat16),
        "iotapD": np.arange(128, dtype=np.float32).reshape(128, 1),
    }
    in_maps = [dict(shared, **pc) for pc in per_core]
    return in_maps, cw


def kernel(**inputs):
    from concourse.bass_utils import run_bass_kernel_spmd

    in_maps, cw = _preprocess(inputs)
    key = ("nc", cw)
    if key not in _CACHE:
        _CACHE[key] = _build_nc(cw)
    nc = _CACHE[key]
    res = run_bass_kernel_spmd(nc, in_maps, core_ids=list(range(NCORES)))
    return res.results[0]["y"].reshape(G).astype(np.float32)


# revision 18
# speedup vs baseline: 3.9112x; 1.1500x over previous
"""CGCNN message-passing kernel for 8 Trainium2 NeuronCores (Bass/Tile), v2.

Strategy (data-parallel by dst-node range, gather-based edge pipeline):
- Nodes split into 8 shards of 3750 (padded 3840 = 30 windows x 128). Edges
  assigned to the core owning dst, grouped by 128-node dst window, padded to a
  uniform chunks-per-window count cw (SPMD-uniform program).
- Per layer, per core:
  * Qd table (own shard)  = h_own  @ Wdst  -> DRAM [3840, 128] bf16
  * AllGather h (bf16), then Qs table (all nodes) = h_full @ Wsrc
    -> DRAM [30720, 128] bf16 (partition-major row order for fat DMA runs)
  * per 1024-edge tile: dma_gather Qd rows + Qs rows (1024 descriptors each),
    Qe = ea @ Wea as matmul, summed in PSUM via identity-matmul adds.
    f-gate columns are negated at preprocessing, so one joint exp pass gives
    u=[e^-a | e^b]; v=ln(1+u)=[sp(-a) | sp(b)]; sigmoid(a)=e^(-sp(-a));
    m = sigmoid * softplus (bf16).
  * segment-mean via onehot matmul (is_equal(iota,dst)*1/cnt, bf16) into PSUM
    accumulated per dst window; BatchNorm batch stats via tiny AllReduce;
    residual + relu on the own shard.
- Global mean pool via onehot matmul, partial sums AllReduced, tiny head MLP
  computed redundantly on every core.
"""
import numpy as np
import ml_dtypes

N = 30000
E = 480000
NF = 92
EF = 50
D1 = 64
D2 = 64
L = 3
FC = 2
G = 256
EPS = 1e-5
NCORES = 8
SHARD = N // NCORES            # 3750
SHARD_P = 3840                 # padded shard (30 windows of 128)
NWIN = SHARD_P // 128          # 30
NWING = NCORES * NWIN          # 240 global windows
TBL = NCORES * SHARD_P         # 30720 table rows

_CACHE = {}


def _build_nc(cw):
    """Build the SPMD bass module. cw = chunks per dst window (uniform)."""
    import concourse.mybir as mybir
    from concourse import bacc
    from concourse.tile import TileContext

    f32 = mybir.dt.float32
    bf16 = mybir.dt.bfloat16
    i16 = mybir.dt.int16
    AF = mybir.ActivationFunctionType
    OP = mybir.AluOpType

    nchunk = NWIN * cw                 # chunks per layer per core
    etot = nchunk * 128                # padded edges per core
    ntile = (nchunk + 7) // 8          # 8-chunk (1024-edge) tiles

    import concourse.hw_specs as _hw
    import concourse.bacc as _bacc_mod
    _real_tables = _hw.get_activation_tables("gen3")
    _combined = "natural_log_exp_and_others"
    if _combined in _real_tables:
        _patched = {
            k: (v if k == _combined else (v - {AF.Exp, AF.Ln}))
            for k, v in _real_tables.items()
        }
        _bacc_mod.get_activation_tables = lambda arch: _patched

    nc = bacc.Bacc(None, target_bir_lowering=False)

    # ---- inputs (per core) ----
    xT = nc.dram_tensor("xT", [NF, SHARD_P], bf16, kind="ExternalInput")
    eaT = nc.dram_tensor("eaT", [64, etot], bf16, kind="ExternalInput")
    qs_idxD = nc.dram_tensor("qs_idxD", [128, etot // 16], i16, kind="ExternalInput")
    onesD = nc.dram_tensor("onesD", [1, 128], bf16, kind="ExternalInput")
    iotapD = nc.dram_tensor("iotapD", [128, 1], f32, kind="ExternalInput")
    dstloc_p = nc.dram_tensor("dstloc_p", [128, nchunk], f32, kind="ExternalInput")
    rc_p = nc.dram_tensor("rc_p", [128, nchunk], f32, kind="ExternalInput")
    batchloc = nc.dram_tensor("batchloc", [128, NWIN], f32, kind="ExternalInput")
    rgc_pn = nc.dram_tensor("rgc_pn", [128, NWIN], f32, kind="ExternalInput")
    # weights (replicated; f-gate halves pre-negated)
    lin0w = nc.dram_tensor("lin0w", [NF, D1], bf16, kind="ExternalInput")
    lin0b = nc.dram_tensor("lin0b", [D1, 1], f32, kind="ExternalInput")
    wdst = nc.dram_tensor("wdst", [D1, L * 128], bf16, kind="ExternalInput")
    wsrc = nc.dram_tensor("wsrc", [D1, L * 128], bf16, kind="ExternalInput")
    wea = nc.dram_tensor("wea", [64, L * 128], bf16, kind="ExternalInput")
    bng = nc.dram_tensor("bng", [D1, L], f32, kind="ExternalInput")
    bnb = nc.dram_tensor("bnb", [D1, L], f32, kind="ExternalInput")
    lin1w = nc.dram_tensor("lin1w", [D1, D2], f32, kind="ExternalInput")
    lin1b = nc.dram_tensor("lin1b", [D2, 1], f32, kind="ExternalInput")
    fcw = nc.dram_tensor("fcw", [D2, FC * D2], f32, kind="ExternalInput")
    fcb = nc.dram_tensor("fcb", [D2, FC], f32, kind="ExternalInput")
    lin2w = nc.dram_tensor("lin2w", [D2, 1], f32, kind="ExternalInput")
    lin2b = nc.dram_tensor("lin2b", [1, 1], f32, kind="ExternalInput")
    iota128 = nc.dram_tensor("iota128", [128, 128], bf16, kind="ExternalInput")
    iota256 = nc.dram_tensor("iota256", [128, G], bf16, kind="ExternalInput")
    ident = nc.dram_tensor("ident", [128, 128], f32, kind="ExternalInput")
    identb = nc.dram_tensor("identb", [128, 128], bf16, kind="ExternalInput")

    yout = nc.dram_tensor("y", [1, G], f32, kind="ExternalOutput")

    # ---- DRAM scratch ----
    QsD = nc.dram_tensor("QsD", [TBL, 128], bf16)          # row p*NWING+W
    ag_in = nc.dram_tensor("ag_in", [D1, SHARD_P], bf16)
    ag_out = nc.dram_tensor("ag_out", [NCORES * D1, SHARD_P], bf16,
                            addr_space="Shared")
    ar_in = nc.dram_tensor("ar_in", [D1, 2], f32)
    ar_out = nc.dram_tensor("ar_out", [NCORES * D1, 2], f32, addr_space="Shared")
    pl_in = nc.dram_tensor("pl_in", [D1, G], f32)
    pl_out = nc.dram_tensor("pl_out", [D1, G], f32, addr_space="Shared")

    rg = [list(range(NCORES))]
    QsD3 = QsD[:, :].rearrange("(p w) f -> p w f", p=128)   # [128, NWING, 128]

    with TileContext(nc) as tc:
        with (
            tc.tile_pool(name="const", bufs=1) as cp,
            tc.tile_pool(name="big", bufs=1) as bigp,
            tc.tile_pool(name="work", bufs=2) as wp,
            tc.tile_pool(name="gat", bufs=3) as gp,
            tc.tile_pool(name="nl", bufs=2) as nlp,
            tc.tile_pool(name="oh", bufs=2) as ohp,
            tc.tile_pool(name="st", bufs=2) as stp,
            tc.tile_pool(name="scr", bufs=1) as scp,
            tc.tile_pool(name="pre", bufs=2, space="PSUM") as pp,
            tc.tile_pool(name="psB", bufs=2, space="PSUM") as ppB,
            tc.tile_pool(name="psA", bufs=2, space="PSUM") as ppA,
        ):
            # ---------- constants ----------
            def load_const(t, dram, shape, dtype=f32):
                tt = cp.tile(shape, dtype, tag=t)
                nc.sync.dma_start(out=tt[:], in_=dram)
                return tt

            io128 = load_const("io128", iota128[:, :], [128, 128], bf16)
            io256 = load_const("io256", iota256[:, :], [128, G], bf16)
            idn = load_const("idn", ident[:, :], [128, 128])
            idnb = load_const("idnb", identb[:, :], [128, 128], bf16)
            l0w = load_const("l0w", lin0w[:, :], [NF, D1], bf16)
            l0b = load_const("l0b", lin0b[:, :], [D1, 1])
            wd = load_const("wd", wdst[:, :], [D1, L * 128], bf16)
            ws = load_const("ws", wsrc[:, :], [D1, L * 128], bf16)
            we = load_const("we", wea[:, :], [64, L * 128], bf16)
            gmt = load_const("gmt", bng[:, :], [D1, L])
            bbt = load_const("bbt", bnb[:, :], [D1, L])
            l1w = load_const("l1w", lin1w[:, :], [D1, D2])
            l1b = load_const("l1b", lin1b[:, :], [D2, 1])
            fw = load_const("fw", fcw[:, :], [D2, FC * D2])
            fb = load_const("fb", fcb[:, :], [D2, FC])
            l2w = load_const("l2w", lin2w[:, :], [D2, 1])
            l2b = load_const("l2b", lin2b[:, :], [1, 1])
            dlp = load_const("dlp", dstloc_p[:, :], [128, nchunk])
            rcp = load_const("rcp", rc_p[:, :], [128, nchunk])
            blc = load_const("blc", batchloc[:, :], [128, NWIN])
            rgp = load_const("rgp", rgc_pn[:, :], [128, NWIN])
            qsix = load_const("qsix", qs_idxD[:, :], [128, etot // 16], i16)
            onesb = load_const("onesb", onesD[:, :], [1, 128], bf16)
            iotap = load_const("iotap", iotapD[:, :], [128, 1])

            # ---------- resident state ----------
            hT_own = bigp.tile([D1, SHARD_P], f32, tag="hown")
            hb_own = bigp.tile([D1, SHARD_P], bf16, tag="hbown")
            aggr_sb = bigp.tile([D1, SHARD_P], f32, tag="aggr")
            qd_sb = bigp.tile([128, NWIN, 128], bf16, tag="qdsb")

            # ---------- lin0: hT_own = relu(lin0w.T @ xT + b) ----------
            for j in range(8):
                sl = slice(j * 480, (j + 1) * 480)
                xt = wp.tile([NF, 480], bf16, tag="xt")
                nc.sync.dma_start(out=xt[:], in_=xT[:, sl])
                ph = ppB.tile([D1, 480], f32, tag="bld")
                nc.tensor.matmul(out=ph[:], lhsT=l0w[:], rhs=xt[:],
                                 start=True, stop=True)
                nc.scalar.activation(out=hT_own[:, sl], in_=ph[:],
                                     func=AF.Relu, bias=l0b[:], scale=1.0)

            # ---------- layers ----------
            for l in range(L):
                wd_l = wd[:, l * 128:(l + 1) * 128]
                ws_l = ws[:, l * 128:(l + 1) * 128]
                we_l = we[:, l * 128:(l + 1) * 128]

                # --- bf16 copy of own h ---
                nc.scalar.activation(out=hb_own[:], in_=hT_own[:],
                                     func=AF.Identity, scale=1.0)

                # --- Qd table build (own shard) -> QdD ---
                for w0 in range(0, NWIN, 4):
                    kk = min(4, NWIN - w0)
                    qp = ppB.tile([128, 512], f32, tag="bld")
                    for k in range(kk):
                        w = w0 + k
                        nc.tensor.matmul(
                            out=qp[:, k * 128:(k + 1) * 128],
                            lhsT=hb_own[:, w * 128:(w + 1) * 128],
                            rhs=wd_l, start=True, stop=True)
                    nc.scalar.activation(
                        out=qd_sb[:, w0:w0 + kk, :].rearrange("p a b -> p (a b)"),
                        in_=qp[:, :kk * 128],
                        func=AF.Identity, scale=1.0)

                # --- AllGather h (bf16) ---
                nc.sync.dma_start(out=ag_in[:, :], in_=hb_own[:])
                nc.gpsimd.collective_compute(
                    "AllGather", OP.bypass, replica_groups=rg,
                    ins=[ag_in.ap().opt()], outs=[ag_out.ap().opt()])

                # --- Qs table build (all nodes, per gathered shard) -> QsD ---
                for s_ in range(NCORES):
                    hb_sh = stp.tile([D1, SHARD_P], bf16, tag="hbsh")
                    nc.sync.dma_start(out=hb_sh[:],
                                      in_=ag_out[s_ * D1:(s_ + 1) * D1, :])
                    for wB in range(0, NWIN, 16):
                        kB = min(16, NWIN - wB)
                        sg_t = stp.tile([128, 16, 128], bf16, tag="qsst")
                        for w0 in range(wB, wB + kB, 4):
                            kk = min(4, wB + kB - w0)
                            qp = ppB.tile([128, 512], f32, tag="bld")
                            for k in range(kk):
                                w = w0 + k
                                nc.tensor.matmul(
                                    out=qp[:, k * 128:(k + 1) * 128],
                                    lhsT=hb_sh[:, w * 128:(w + 1) * 128],
                                    rhs=ws_l, start=True, stop=True)
                            nc.scalar.activation(
                                out=sg_t[:, w0 - wB:w0 - wB + kk, :]
                                    .rearrange("p a b -> p (a b)"),
                                in_=qp[:, :kk * 128],
                                func=AF.Identity, scale=1.0)
                        W0 = s_ * NWIN + wB
                        nc.sync.dma_start(out=QsD3[:, W0:W0 + kB, :],
                                          in_=sg_t[:, :kB, :])

                # --- edge pipeline ---
                agg = None
                for t in range(ntile):
                    te = min(8, nchunk - t * 8)          # chunks this tile
                    ne = te * 128                        # edges this tile
                    qs_g = gp.tile([128, 8, 128], bf16, tag="qsg")
                    nc.gpsimd.dma_gather(
                        qs_g[:, :te, :], QsD[:, :],
                        qsix[:, t * 64: t * 64 + te * 8], ne, ne, 128)
                    et = gp.tile([64, 1024], bf16, tag="et")
                    nc.sync.dma_start(out=et[:, :ne],
                                      in_=eaT[:, t * 1024: t * 1024 + ne])
                    ohT = ohp.tile([128, 1024], bf16, tag="ohT")
                    for h0 in range(0, ne, 512):
                        hn = min(512, ne - h0)
                        bc = ppB.tile([128, 512], f32, tag="bld")
                        nc.tensor.matmul(out=bc[:, :hn], lhsT=onesb[:],
                                         rhs=et[0:1, h0:h0 + hn],
                                         start=True, stop=True)
                        nc.vector.tensor_scalar(
                            out=ohT[:, h0:h0 + hn], in0=bc[:, :hn],
                            scalar1=iotap[:], scalar2=None, op0=OP.is_equal)

                    pre = pp.tile([128, 1024], f32, tag="pre")
                    qs_f = qs_g[:].rearrange("p a b -> p (a b)")
                    for c in range(te):
                        gc = t * 8 + c
                        w = gc // cw
                        csl = slice(c * 128, (c + 1) * 128)
                        nc.tensor.matmul(out=pre[:, csl], lhsT=et[:, csl],
                                         rhs=we_l, start=True, stop=False)
                        nc.tensor.matmul(out=pre[:, csl], lhsT=idnb[:],
                                         rhs=qs_f[:, csl], start=False, stop=False)
                        nc.tensor.matmul(out=pre[:, csl], lhsT=ohT[:, csl],
                                         rhs=qd_sb[:, w, :], start=False, stop=True)

                    u = nlp.tile([128, 1024], f32, tag="u")
                    nc.scalar.activation(out=u[:, :ne], in_=pre[:, :ne],
                                         func=AF.Exp, scale=1.0)
                    v = nlp.tile([128, 8, 128], f32, tag="v")
                    nc.scalar.activation(
                        out=v[:, :te, :].rearrange("p a b -> p (a b)"),
                        in_=u[:, :ne], func=AF.Ln, bias=1.0, scale=1.0)
                    sg = nlp.tile([128, 8, 64], f32, tag="sg")
                    nc.scalar.activation(out=sg[:, :te, :], in_=v[:, :te, 0:64],
                                         func=AF.Exp, scale=-1.0)
                    m = nlp.tile([128, 8, 64], bf16, tag="m")
                    nc.vector.tensor_tensor(out=m[:, :te, :], in0=sg[:, :te, :],
                                            in1=v[:, :te, 64:128], op=OP.mult)

                    for c in range(te):
                        gc = t * 8 + c
                        w = gc // cw
                        if gc % (4 * cw) == 0:
                            agg = ppA.tile([D1, 512], f32, tag="agg")
                        ohS = ohp.tile([128, 128], bf16, tag="ohS")
                        nc.vector.tensor_scalar(
                            out=ohS[:], in0=io128[:],
                            scalar1=dlp[:, gc:gc + 1], scalar2=rcp[:, gc:gc + 1],
                            op0=OP.is_equal, op1=OP.mult)
                        nc.tensor.matmul(
                            out=agg[:, (w % 4) * 128:(w % 4 + 1) * 128],
                            lhsT=m[:, c, :], rhs=ohS[:],
                            start=(gc % cw == 0), stop=(gc % cw == cw - 1))
                        if gc % (4 * cw) == 4 * cw - 1 or gc == nchunk - 1:
                            grp = w // 4
                            lo = grp * 512
                            hi = min(lo + 512, SHARD_P)
                            nc.scalar.activation(
                                out=aggr_sb[:, lo:hi], in_=agg[:, :hi - lo],
                                func=AF.Identity, scale=1.0)

                # --- BN stats + AllReduce ---
                st = wp.tile([D1, 2], f32, tag="st")
                nc.vector.reduce_sum(out=st[:, 0:1], in_=aggr_sb[:],
                                     axis=mybir.AxisListType.X)
                sq = scp.tile([D1, SHARD_P], f32, tag="sq")
                nc.vector.tensor_tensor(out=sq[:], in0=aggr_sb[:],
                                        in1=aggr_sb[:], op=OP.mult)
                nc.vector.reduce_sum(out=st[:, 1:2], in_=sq[:],
                                     axis=mybir.AxisListType.X)
                nc.sync.dma_start(out=ar_in[:, :], in_=st[:])
                nc.gpsimd.collective_compute(
                    "AllGather", OP.bypass, replica_groups=rg,
                    ins=[ar_in.ap().opt()], outs=[ar_out.ap().opt()])
                stga = wp.tile([D1, 2, NCORES], f32, tag="stga")
                nc.sync.dma_start(
                    out=stga[:],
                    in_=ar_out[:, :].rearrange("(c p) s -> p s c", p=D1))
                stg = wp.tile([D1, 2], f32, tag="stg")
                nc.vector.reduce_sum(
                    out=stg[:].rearrange("p (s o) -> p s o", o=1),
                    in_=stga[:], axis=mybir.AxisListType.X)
                mu = wp.tile([D1, 1], f32, tag="mu")
                nc.vector.tensor_scalar(out=mu[:], in0=stg[:, 0:1],
                                        scalar1=1.0 / N, scalar2=None, op0=OP.mult)
                ex2 = wp.tile([D1, 1], f32, tag="ex2")
                nc.vector.tensor_scalar(out=ex2[:], in0=stg[:, 1:2],
                                        scalar1=1.0 / N, scalar2=None, op0=OP.mult)
                mu2 = wp.tile([D1, 1], f32, tag="mu2")
                nc.vector.tensor_tensor(out=mu2[:], in0=mu[:], in1=mu[:], op=OP.mult)
                var = wp.tile([D1, 1], f32, tag="var")
                nc.vector.tensor_tensor(out=var[:], in0=ex2[:], in1=mu2[:],
                                        op=OP.subtract)
                ve = wp.tile([D1, 1], f32, tag="ve")
                nc.vector.tensor_scalar(out=ve[:], in0=var[:], scalar1=EPS,
                                        scalar2=None, op0=OP.add)
                lv = wp.tile([D1, 1], f32, tag="lv")
                nc.scalar.activation(out=lv[:], in_=ve[:], func=AF.Ln, scale=1.0)
                isd = wp.tile([D1, 1], f32, tag="isd")
                nc.scalar.activation(out=isd[:], in_=lv[:], func=AF.Exp, scale=-0.5)
                scale = wp.tile([D1, 1], f32, tag="scale")
                nc.vector.tensor_tensor(out=scale[:], in0=isd[:],
                                        in1=gmt[:, l:l + 1], op=OP.mult)
                mshift = wp.tile([D1, 1], f32, tag="mshift")
                nc.vector.tensor_tensor(out=mshift[:], in0=mu[:], in1=scale[:],
                                        op=OP.mult)
                shift = wp.tile([D1, 1], f32, tag="shift")
                nc.vector.tensor_tensor(out=shift[:], in0=bbt[:, l:l + 1],
                                        in1=mshift[:], op=OP.subtract)
                # h = relu(h + aggr*scale + shift)
                asb = scp.tile([D1, SHARD_P], f32, tag="asb")
                nc.vector.tensor_scalar(out=asb[:], in0=aggr_sb[:],
                                        scalar1=scale[:], scalar2=shift[:],
                                        op0=OP.mult, op1=OP.add)
                nc.vector.tensor_tensor(out=asb[:], in0=asb[:], in1=hT_own[:],
                                        op=OP.add)
                nc.vector.tensor_scalar(out=hT_own[:], in0=asb[:],
                                        scalar1=0.0, scalar2=None, op0=OP.max)

            # ---------- global mean pool ----------
            pool_ps = pp.tile([D1, G], f32, tag="pre")
            for w in range(NWIN):
                tp = ppB.tile([128, D1], f32, tag="bld")
                nc.tensor.transpose(out=tp[:], in_=hT_own[:, w * 128:(w + 1) * 128],
                                    identity=idn[0:D1, 0:D1])
                rows = wp.tile([128, D1], bf16, tag="rows")
                nc.vector.tensor_copy(out=rows[:], in_=tp[:])
                ohg = ohp.tile([128, G], bf16, tag="ohg")
                nc.vector.tensor_scalar(
                    out=ohg[:], in0=io256[:],
                    scalar1=blc[:, w:w + 1], scalar2=rgp[:, w:w + 1],
                    op0=OP.is_equal, op1=OP.mult)
                nc.tensor.matmul(out=pool_ps[:], lhsT=rows[:], rhs=ohg[:],
                                 start=(w == 0), stop=(w == NWIN - 1))
            poolT = wp.tile([D1, G], f32, tag="poolT")
            nc.vector.tensor_copy(out=poolT[:], in_=pool_ps[:])
            nc.sync.dma_start(out=pl_in[:, :], in_=poolT[:])
            nc.gpsimd.collective_compute(
                "AllReduce", OP.add, replica_groups=rg,
                ins=[pl_in.ap().opt()], outs=[pl_out.ap().opt()])
            pg = wp.tile([D1, G], f32, tag="pg")
            nc.sync.dma_start(out=pg[:], in_=pl_out[:, :])

            # ---------- head ----------
            a = pg
            hw_ = [(l1w[:], l1b[:]), (fw[:, 0:D2], fb[:, 0:1]), (fw[:, D2:2 * D2], fb[:, 1:2])]
            for (wt, bt) in hw_:
                ps = ppB.tile([D2, G], f32, tag="bld")
                nc.tensor.matmul(out=ps[:, 0:G], lhsT=wt, rhs=a[:], start=True, stop=True)
                an = wp.tile([D2, G], f32, tag="an")
                nc.scalar.activation(out=an[:], in_=ps[:, 0:G], func=AF.Relu,
                                     bias=bt, scale=1.0)
                a = an
            ps = ppB.tile([1, G], f32, tag="bld")
            nc.tensor.matmul(out=ps[:, 0:G], lhsT=l2w[:], rhs=a[:], start=True, stop=True)
            yt = wp.tile([1, G], f32, tag="yt")
            nc.scalar.activation(out=yt[:], in_=ps[:, 0:G], func=AF.Identity,
                                 bias=l2b[:], scale=1.0)
            nc.sync.dma_start(out=yout[:, :], in_=yt[:])

    nc.compile()
    return nc


def _wrap16(idx):
    """Flat idx list -> [128, n/16] int16: slot i at [i%16, i//16], replicated
    across the 8 Q7 cores."""
    a = idx.reshape(-1, 16).T.astype(np.int16)
    return np.tile(a, (8, 1))


def _preprocess(inputs):
    x = np.asarray(inputs["x"], np.float32)
    ea = np.asarray(inputs["edge_attr"], np.float32)
    ei = np.asarray(inputs["edge_index"]).astype(np.int64)
    batch = np.asarray(inputs["batch"]).astype(np.int64)
    src, dst = ei[0], ei[1]

    cnt = np.bincount(dst, minlength=N).astype(np.float32)
    rc_node = 1.0 / np.maximum(cnt, 1.0)
    gcnt = np.bincount(batch, minlength=G).astype(np.float32)
    rgc = 1.0 / np.maximum(gcnt, 1.0)

    order = np.argsort(dst, kind="stable")
    src_s, dst_s, ea_idx = src[order], dst[order], order
    core_s = dst_s // SHARD

    bounds = []
    for c in range(NCORES):
        for w in range(NWIN):
            bounds.append(c * SHARD + min(w * 128, SHARD))
    bounds.append(N)
    bpos = np.searchsorted(dst_s, np.asarray(bounds), side="left")
    percw = {}
    maxcnt = 0
    k = 0
    for c in range(NCORES):
        for w in range(NWIN):
            lo, hi = bpos[k], bpos[k + 1]
            percw[(c, w)] = np.arange(lo, hi)
            maxcnt = max(maxcnt, hi - lo)
            k += 1
    cw = max(1, (maxcnt + 127) // 128)
    etot = NWIN * cw * 128

    per_core = []
    for c in range(NCORES):
        qs_idx = np.zeros(etot, np.int64)
        dl = np.full(etot, -1.0, np.float32)
        rc_e = np.ones(etot, np.float32)
        ea_e = np.zeros((etot, EF), np.float32)
        for w in range(NWIN):
            idxs = percw[(c, w)]
            o = w * cw * 128
            k = len(idxs)
            s = src_s[idxs]
            g = (s // SHARD) * SHARD_P + (s % SHARD)   # padded global id
            qs_idx[o:o + k] = (g % 128) * NWING + (g // 128)
            loc = dst_s[idxs] - c * SHARD              # 0..3749
            dl[o:o + k] = (loc - w * 128).astype(np.float32)
            rc_e[o:o + k] = rc_node[dst_s[idxs]]
            ea_e[o:o + k] = ea[ea_idx[idxs]]
        eaT = np.zeros((64, etot), np.float32)
        eaT[0] = dl
        eaT[1:EF + 1] = ea_e.T
        eaT[EF + 1] = 1.0
        eaT[EF + 1, dl < 0] = 0.0
        nch = etot // 128
        d = {
            "qs_idxD": _wrap16(qs_idx),
            "dstloc_p": dl.reshape(nch, 128).T.copy(),
            "rc_p": rc_e.reshape(nch, 128).T.copy(),
            "eaT": eaT.astype(ml_dtypes.bfloat16),
        }
        xp = np.zeros((NF, SHARD_P), np.float32)
        xp[:, :SHARD] = x[c * SHARD:(c + 1) * SHARD].T
        d["xT"] = xp.astype(ml_dtypes.bfloat16)
        bl = np.full(SHARD_P, -1.0, np.float32)
        bl[:SHARD] = batch[c * SHARD:(c + 1) * SHARD].astype(np.float32)
        rg_n = np.zeros(SHARD_P, np.float32)
        rg_n[:SHARD] = rgc[batch[c * SHARD:(c + 1) * SHARD]]
        d["batchloc"] = bl.reshape(NWIN, 128).T.copy()
        d["rgc_pn"] = rg_n.reshape(NWIN, 128).T.copy()
        per_core.append(d)

    # replicated weights; f-gate half negated so pre = [-a | b]
    wf = np.asarray(inputs["conv_wf"], np.float32)
    wsv = np.asarray(inputs["conv_ws"], np.float32)
    bf = np.asarray(inputs["conv_bf"], np.float32)
    bs = np.asarray(inputs["conv_bs"], np.float32)
    wdst = np.concatenate([-wf[:, 0:D1, :], wsv[:, 0:D1, :]], axis=2)
    wsrc = np.concatenate([-wf[:, D1:2 * D1, :], wsv[:, D1:2 * D1, :]], axis=2)
    wea = np.concatenate([-wf[:, 2 * D1:, :], wsv[:, 2 * D1:, :]], axis=2)
    bias = np.concatenate([-bf, bs], axis=1)[:, None, :]
    wea = np.concatenate([wea, bias], axis=1)
    shared = {
        "lin0w": np.asarray(inputs["lin0_w"], np.float32).astype(ml_dtypes.bfloat16),
        "lin0b": np.asarray(inputs["lin0_b"], np.float32).reshape(D1, 1),
        "wdst": np.transpose(wdst, (1, 0, 2)).reshape(D1, L * 128).astype(ml_dtypes.bfloat16),
        "wsrc": np.transpose(wsrc, (1, 0, 2)).reshape(D1, L * 128).astype(ml_dtypes.bfloat16),
        "wea": np.concatenate([
            np.zeros((1, L * 128), np.float32),
            np.transpose(wea, (1, 0, 2)).reshape(EF + 1, L * 128),
            np.zeros((64 - EF - 2, L * 128), np.float32),
        ], axis=0).astype(ml_dtypes.bfloat16),
        "bng": np.asarray(inputs["bn_gamma"], np.float32).T.copy(),
        "bnb": np.asarray(inputs["bn_beta"], np.float32).T.copy(),
        "lin1w": np.asarray(inputs["lin1_w"], np.float32),
        "lin1b": np.asarray(inputs["lin1_b"], np.float32).reshape(D2, 1),
        "fcw": np.transpose(np.asarray(inputs["fc_w"], np.float32), (1, 0, 2)).reshape(D2, FC * D2),
        "fcb": np.asarray(inputs["fc_b"], np.float32).T.copy(),
        "lin2w": np.asarray(inputs["lin2_w"], np.float32).reshape(D2, 1),
        "lin2b": np.asarray(inputs["lin2_b"], np.float32).reshape(1, 1),
        "iota128": np.broadcast_to(np.arange(128, dtype=np.float32)[None, :],
                                   (128, 128)).astype(ml_dtypes.bfloat16),
        "iota256": np.broadcast_to(np.arange(G, dtype=np.float32)[None, :],
                                   (128, G)).astype(ml_dtypes.bfloat16),
        "ident": np.eye(128, dtype=np.float32),
        "identb": np.eye(128, dtype=np.float32).astype(ml_dtypes.bfloat16),
        "onesD": np.ones((1, 128), np.float32).astype(ml_dtypes.bfloat16),
        "iotapD": np.arange(128, dtype=np.float32).reshape(128, 1),
    }
    in_maps = [dict(shared, **pc) for pc in per_core]
    return in_maps, cw


def kernel(**inputs):
    from concourse.bass_utils import run_bass_kernel_spmd

    in_maps, cw = _preprocess(inputs)
    key = ("nc", cw)
    if key not in _CACHE:
        _CACHE[key] = _build_nc(cw)
    nc = _CACHE[key]
    res = run_bass_kernel_spmd(nc, in_maps, core_ids=list(range(NCORES)))
    return res.results[0]["y"].reshape(G).astype(np.float32)


# revision 19
# speedup vs baseline: 4.3654x; 1.1161x over previous
"""CGCNN message-passing kernel for 8 Trainium2 NeuronCores (Bass/Tile), v2.

Strategy (data-parallel by dst-node range, gather-based edge pipeline):
- Nodes split into 8 shards of 3750 (padded 3840 = 30 windows x 128). Edges
  assigned to the core owning dst, grouped by 128-node dst window, padded to a
  uniform chunks-per-window count cw (SPMD-uniform program).
- Per layer, per core:
  * Qd table (own shard)  = h_own  @ Wdst  -> DRAM [3840, 128] bf16
  * AllGather h (bf16), then Qs table (all nodes) = h_full @ Wsrc
    -> DRAM [30720, 128] bf16 (partition-major row order for fat DMA runs)
  * per 1024-edge tile: dma_gather Qd rows + Qs rows (1024 descriptors each),
    Qe = ea @ Wea as matmul, summed in PSUM via identity-matmul adds.
    f-gate columns are negated at preprocessing, so one joint exp pass gives
    u=[e^-a | e^b]; v=ln(1+u)=[sp(-a) | sp(b)]; sigmoid(a)=e^(-sp(-a));
    m = sigmoid * softplus (bf16).
  * segment-mean via onehot matmul (is_equal(iota,dst)*1/cnt, bf16) into PSUM
    accumulated per dst window; BatchNorm batch stats via tiny AllReduce;
    residual + relu on the own shard.
- Global mean pool via onehot matmul, partial sums AllReduced, tiny head MLP
  computed redundantly on every core.
"""
import numpy as np
import ml_dtypes

N = 30000
E = 480000
NF = 92
EF = 50
D1 = 64
D2 = 64
L = 3
FC = 2
G = 256
EPS = 1e-5
NCORES = 8
SHARD = N // NCORES            # 3750
SHARD_P = 3840                 # padded shard (30 windows of 128)
NWIN = SHARD_P // 128          # 30
NWING = NCORES * NWIN          # 240 global windows
TBL = NCORES * SHARD_P         # 30720 table rows

_CACHE = {}


def _build_nc(cw):
    """Build the SPMD bass module. cw = chunks per dst window (uniform)."""
    import concourse.mybir as mybir
    from concourse import bacc
    from concourse.tile import TileContext

    f32 = mybir.dt.float32
    bf16 = mybir.dt.bfloat16
    f8 = mybir.dt.float8e4
    i16 = mybir.dt.int16
    AF = mybir.ActivationFunctionType
    OP = mybir.AluOpType

    nchunk = NWIN * cw                 # chunks per layer per core
    etot = nchunk * 128                # padded edges per core
    ntile = (nchunk + 7) // 8          # 8-chunk (1024-edge) tiles

    import concourse.hw_specs as _hw
    import concourse.bacc as _bacc_mod
    _real_tables = _hw.get_activation_tables("gen3")
    _combined = "natural_log_exp_and_others"
    if _combined in _real_tables:
        _patched = {
            k: (v if k == _combined else (v - {AF.Exp, AF.Ln}))
            for k, v in _real_tables.items()
        }
        _bacc_mod.get_activation_tables = lambda arch: _patched

    nc = bacc.Bacc(None, target_bir_lowering=False)

    # ---- inputs (per core) ----
    xT = nc.dram_tensor("xT", [NF, SHARD_P], bf16, kind="ExternalInput")
    eaT = nc.dram_tensor("eaT", [64, etot], bf16, kind="ExternalInput")
    qs_idxD = nc.dram_tensor("qs_idxD", [128, etot // 16], i16, kind="ExternalInput")
    onesD = nc.dram_tensor("onesD", [1, 128], bf16, kind="ExternalInput")
    iotapD = nc.dram_tensor("iotapD", [128, 1], f32, kind="ExternalInput")
    dstloc_p = nc.dram_tensor("dstloc_p", [128, nchunk], f32, kind="ExternalInput")
    rc_p = nc.dram_tensor("rc_p", [128, nchunk], f32, kind="ExternalInput")
    batchloc = nc.dram_tensor("batchloc", [128, NWIN], f32, kind="ExternalInput")
    rgc_pn = nc.dram_tensor("rgc_pn", [128, NWIN], f32, kind="ExternalInput")
    # weights (replicated; f-gate halves pre-negated)
    lin0w = nc.dram_tensor("lin0w", [NF, D1], bf16, kind="ExternalInput")
    lin0b = nc.dram_tensor("lin0b", [D1, 1], f32, kind="ExternalInput")
    wdst = nc.dram_tensor("wdst", [D1, L * 128], bf16, kind="ExternalInput")
    wsrc = nc.dram_tensor("wsrc", [D1, L * 128], bf16, kind="ExternalInput")
    wea = nc.dram_tensor("wea", [64, L * 128], bf16, kind="ExternalInput")
    bng = nc.dram_tensor("bng", [D1, L], f32, kind="ExternalInput")
    bnb = nc.dram_tensor("bnb", [D1, L], f32, kind="ExternalInput")
    lin1w = nc.dram_tensor("lin1w", [D1, D2], f32, kind="ExternalInput")
    lin1b = nc.dram_tensor("lin1b", [D2, 1], f32, kind="ExternalInput")
    fcw = nc.dram_tensor("fcw", [D2, FC * D2], f32, kind="ExternalInput")
    fcb = nc.dram_tensor("fcb", [D2, FC], f32, kind="ExternalInput")
    lin2w = nc.dram_tensor("lin2w", [D2, 1], f32, kind="ExternalInput")
    lin2b = nc.dram_tensor("lin2b", [1, 1], f32, kind="ExternalInput")
    iota128 = nc.dram_tensor("iota128", [128, 128], bf16, kind="ExternalInput")
    iota256 = nc.dram_tensor("iota256", [128, G], bf16, kind="ExternalInput")
    ident = nc.dram_tensor("ident", [128, 128], f32, kind="ExternalInput")
    identb = nc.dram_tensor("identb", [128, 128], bf16, kind="ExternalInput")

    yout = nc.dram_tensor("y", [1, G], f32, kind="ExternalOutput")

    # ---- DRAM scratch ----
    QsD = nc.dram_tensor("QsD", [TBL, 128], bf16)          # row p*NWING+W
    ag_in = nc.dram_tensor("ag_in", [D1, SHARD_P], f8)
    ag_out = nc.dram_tensor("ag_out", [NCORES * D1, SHARD_P], f8,
                            addr_space="Shared")
    ar_in = nc.dram_tensor("ar_in", [D1, 2], f32)
    ar_out = nc.dram_tensor("ar_out", [NCORES * D1, 2], f32, addr_space="Shared")
    pl_in = nc.dram_tensor("pl_in", [D1, G], f32)
    pl_out = nc.dram_tensor("pl_out", [D1, G], f32, addr_space="Shared")

    rg = [list(range(NCORES))]
    QsD3 = QsD[:, :].rearrange("(p w) f -> p w f", p=128)   # [128, NWING, 128]

    with TileContext(nc) as tc:
        with (
            tc.tile_pool(name="const", bufs=1) as cp,
            tc.tile_pool(name="big", bufs=1) as bigp,
            tc.tile_pool(name="work", bufs=2) as wp,
            tc.tile_pool(name="gat", bufs=3) as gp,
            tc.tile_pool(name="nl", bufs=2) as nlp,
            tc.tile_pool(name="oh", bufs=2) as ohp,
            tc.tile_pool(name="st", bufs=2) as stp,
            tc.tile_pool(name="scr", bufs=1) as scp,
            tc.tile_pool(name="pre", bufs=2, space="PSUM") as pp,
            tc.tile_pool(name="psB", bufs=2, space="PSUM") as ppB,
            tc.tile_pool(name="psA", bufs=2, space="PSUM") as ppA,
        ):
            # ---------- constants ----------
            def load_const(t, dram, shape, dtype=f32):
                tt = cp.tile(shape, dtype, tag=t)
                nc.sync.dma_start(out=tt[:], in_=dram)
                return tt

            io128 = load_const("io128", iota128[:, :], [128, 128], bf16)
            io256 = load_const("io256", iota256[:, :], [128, G], bf16)
            idn = load_const("idn", ident[:, :], [128, 128])
            idnb = load_const("idnb", identb[:, :], [128, 128], bf16)
            l0w = load_const("l0w", lin0w[:, :], [NF, D1], bf16)
            l0b = load_const("l0b", lin0b[:, :], [D1, 1])
            wd = load_const("wd", wdst[:, :], [D1, L * 128], bf16)
            ws = load_const("ws", wsrc[:, :], [D1, L * 128], bf16)
            we = load_const("we", wea[:, :], [64, L * 128], bf16)
            gmt = load_const("gmt", bng[:, :], [D1, L])
            bbt = load_const("bbt", bnb[:, :], [D1, L])
            l1w = load_const("l1w", lin1w[:, :], [D1, D2])
            l1b = load_const("l1b", lin1b[:, :], [D2, 1])
            fw = load_const("fw", fcw[:, :], [D2, FC * D2])
            fb = load_const("fb", fcb[:, :], [D2, FC])
            l2w = load_const("l2w", lin2w[:, :], [D2, 1])
            l2b = load_const("l2b", lin2b[:, :], [1, 1])
            dlp = load_const("dlp", dstloc_p[:, :], [128, nchunk])
            rcp = load_const("rcp", rc_p[:, :], [128, nchunk])
            blc = load_const("blc", batchloc[:, :], [128, NWIN])
            rgp = load_const("rgp", rgc_pn[:, :], [128, NWIN])
            qsix = load_const("qsix", qs_idxD[:, :], [128, etot // 16], i16)
            onesb = load_const("onesb", onesD[:, :], [1, 128], bf16)
            iotap = load_const("iotap", iotapD[:, :], [128, 1])

            # ---------- resident state ----------
            hT_own = bigp.tile([D1, SHARD_P], f32, tag="hown")
            hb_own = bigp.tile([D1, SHARD_P], bf16, tag="hbown")
            aggr_sb = bigp.tile([D1, SHARD_P], f32, tag="aggr")
            qd_sb = bigp.tile([128, NWIN, 128], bf16, tag="qdsb")

            # ---------- lin0: hT_own = relu(lin0w.T @ xT + b) ----------
            for j in range(8):
                sl = slice(j * 480, (j + 1) * 480)
                xt = wp.tile([NF, 480], bf16, tag="xt")
                nc.sync.dma_start(out=xt[:], in_=xT[:, sl])
                ph = ppB.tile([D1, 480], f32, tag="bld")
                nc.tensor.matmul(out=ph[:], lhsT=l0w[:], rhs=xt[:],
                                 start=True, stop=True)
                nc.scalar.activation(out=hT_own[:, sl], in_=ph[:],
                                     func=AF.Relu, bias=l0b[:], scale=1.0)

            # ---------- layers ----------
            for l in range(L):
                wd_l = wd[:, l * 128:(l + 1) * 128]
                ws_l = ws[:, l * 128:(l + 1) * 128]
                we_l = we[:, l * 128:(l + 1) * 128]

                # --- bf16 copy of own h ---
                nc.scalar.activation(out=hb_own[:], in_=hT_own[:],
                                     func=AF.Identity, scale=1.0)

                # --- Qd table build (own shard) -> QdD ---
                for w0 in range(0, NWIN, 4):
                    kk = min(4, NWIN - w0)
                    qp = ppB.tile([128, 512], f32, tag="bld")
                    for k in range(kk):
                        w = w0 + k
                        nc.tensor.matmul(
                            out=qp[:, k * 128:(k + 1) * 128],
                            lhsT=hb_own[:, w * 128:(w + 1) * 128],
                            rhs=wd_l, start=True, stop=True)
                    nc.scalar.activation(
                        out=qd_sb[:, w0:w0 + kk, :].rearrange("p a b -> p (a b)"),
                        in_=qp[:, :kk * 128],
                        func=AF.Identity, scale=1.0)

                # --- AllGather h (fp8) ---
                h8 = stp.tile([D1, SHARD_P], f8, tag="h8")
                nc.scalar.activation(out=h8[:], in_=hT_own[:],
                                     func=AF.Identity, scale=1.0)
                nc.sync.dma_start(out=ag_in[:, :], in_=h8[:])
                nc.gpsimd.collective_compute(
                    "AllGather", OP.bypass, replica_groups=rg,
                    ins=[ag_in.ap().opt()], outs=[ag_out.ap().opt()])

                # --- Qs table build (all nodes, per gathered shard) -> QsD ---
                ws8 = stp.tile([D1, 128], f8, tag="ws8")
                nc.scalar.activation(out=ws8[:], in_=ws_l,
                                     func=AF.Identity, scale=1.0)
                for s_ in range(NCORES):
                    hb_sh = stp.tile([D1, SHARD_P], f8, tag="hbsh")
                    nc.sync.dma_start(out=hb_sh[:],
                                      in_=ag_out[s_ * D1:(s_ + 1) * D1, :])
                    for wB in range(0, NWIN, 16):
                        kB = min(16, NWIN - wB)
                        sg_t = stp.tile([128, 16, 128], bf16, tag="qsst")
                        for w0 in range(wB, wB + kB, 4):
                            kk = min(4, wB + kB - w0)
                            qp = ppB.tile([128, 512], f32, tag="bld")
                            for k in range(kk):
                                w = w0 + k
                                nc.tensor.matmul(
                                    out=qp[:, k * 128:(k + 1) * 128],
                                    lhsT=hb_sh[:, w * 128:(w + 1) * 128],
                                    rhs=ws8[:], start=True, stop=True)
                            nc.scalar.activation(
                                out=sg_t[:, w0 - wB:w0 - wB + kk, :]
                                    .rearrange("p a b -> p (a b)"),
                                in_=qp[:, :kk * 128],
                                func=AF.Identity, scale=1.0)
                        W0 = s_ * NWIN + wB
                        nc.sync.dma_start(out=QsD3[:, W0:W0 + kB, :],
                                          in_=sg_t[:, :kB, :])

                # --- edge pipeline ---
                agg = None
                for t in range(ntile):
                    te = min(8, nchunk - t * 8)          # chunks this tile
                    ne = te * 128                        # edges this tile
                    qs_g = gp.tile([128, 8, 128], bf16, tag="qsg")
                    nc.gpsimd.dma_gather(
                        qs_g[:, :te, :], QsD[:, :],
                        qsix[:, t * 64: t * 64 + te * 8], ne, ne, 128)
                    et = gp.tile([64, 1024], bf16, tag="et")
                    nc.sync.dma_start(out=et[:, :ne],
                                      in_=eaT[:, t * 1024: t * 1024 + ne])
                    ohT = ohp.tile([128, 1024], bf16, tag="ohT")
                    for h0 in range(0, ne, 512):
                        hn = min(512, ne - h0)
                        bc = ppB.tile([128, 512], f32, tag="bld")
                        nc.tensor.matmul(out=bc[:, :hn], lhsT=onesb[:],
                                         rhs=et[0:1, h0:h0 + hn],
                                         start=True, stop=True)
                        nc.vector.tensor_scalar(
                            out=ohT[:, h0:h0 + hn], in0=bc[:, :hn],
                            scalar1=iotap[:], scalar2=None, op0=OP.is_equal)

                    pre = pp.tile([128, 1024], f32, tag="pre")
                    qs_f = qs_g[:].rearrange("p a b -> p (a b)")
                    for c in range(te):
                        gc = t * 8 + c
                        w = gc // cw
                        csl = slice(c * 128, (c + 1) * 128)
                        nc.tensor.matmul(out=pre[:, csl], lhsT=et[:, csl],
                                         rhs=we_l, start=True, stop=False)
                        nc.tensor.matmul(out=pre[:, csl], lhsT=idnb[:],
                                         rhs=qs_f[:, csl], start=False, stop=False)
                        nc.tensor.matmul(out=pre[:, csl], lhsT=ohT[:, csl],
                                         rhs=qd_sb[:, w, :], start=False, stop=True)

                    u = nlp.tile([128, 1024], f32, tag="u")
                    nc.scalar.activation(out=u[:, :ne], in_=pre[:, :ne],
                                         func=AF.Exp, scale=1.0)
                    v = nlp.tile([128, 8, 128], f32, tag="v")
                    nc.scalar.activation(
                        out=v[:, :te, :].rearrange("p a b -> p (a b)"),
                        in_=u[:, :ne], func=AF.Ln, bias=1.0, scale=1.0)
                    sg = nlp.tile([128, 8, 64], f32, tag="sg")
                    nc.scalar.activation(out=sg[:, :te, :], in_=v[:, :te, 0:64],
                                         func=AF.Exp, scale=-1.0)
                    m = nlp.tile([128, 8, 64], bf16, tag="m")
                    nc.vector.tensor_tensor(out=m[:, :te, :], in0=sg[:, :te, :],
                                            in1=v[:, :te, 64:128], op=OP.mult)

                    for c in range(te):
                        gc = t * 8 + c
                        w = gc // cw
                        if gc % (4 * cw) == 0:
                            agg = ppA.tile([D1, 512], f32, tag="agg")
                        ohS = ohp.tile([128, 128], bf16, tag="ohS")
                        nc.vector.tensor_scalar(
                            out=ohS[:], in0=io128[:],
                            scalar1=dlp[:, gc:gc + 1], scalar2=rcp[:, gc:gc + 1],
                            op0=OP.is_equal, op1=OP.mult)
                        nc.tensor.matmul(
                            out=agg[:, (w % 4) * 128:(w % 4 + 1) * 128],
                            lhsT=m[:, c, :], rhs=ohS[:],
                            start=(gc % cw == 0), stop=(gc % cw == cw - 1))
                        if gc % (4 * cw) == 4 * cw - 1 or gc == nchunk - 1:
                            grp = w // 4
                            lo = grp * 512
                            hi = min(lo + 512, SHARD_P)
                            nc.scalar.activation(
                                out=aggr_sb[:, lo:hi], in_=agg[:, :hi - lo],
                                func=AF.Identity, scale=1.0)

                # --- BN stats + AllReduce ---
                st = wp.tile([D1, 2], f32, tag="st")
                nc.vector.reduce_sum(out=st[:, 0:1], in_=aggr_sb[:],
                                     axis=mybir.AxisListType.X)
                sq = scp.tile([D1, SHARD_P], f32, tag="sq")
                nc.vector.tensor_tensor(out=sq[:], in0=aggr_sb[:],
                                        in1=aggr_sb[:], op=OP.mult)
                nc.vector.reduce_sum(out=st[:, 1:2], in_=sq[:],
                                     axis=mybir.AxisListType.X)
                nc.sync.dma_start(out=ar_in[:, :], in_=st[:])
                nc.gpsimd.collective_compute(
                    "AllGather", OP.bypass, replica_groups=rg,
                    ins=[ar_in.ap().opt()], outs=[ar_out.ap().opt()])
                stga = wp.tile([D1, 2, NCORES], f32, tag="stga")
                nc.sync.dma_start(
                    out=stga[:],
                    in_=ar_out[:, :].rearrange("(c p) s -> p s c", p=D1))
                stg = wp.tile([D1, 2], f32, tag="stg")
                nc.vector.reduce_sum(
                    out=stg[:].rearrange("p (s o) -> p s o", o=1),
                    in_=stga[:], axis=mybir.AxisListType.X)
                mu = wp.tile([D1, 1], f32, tag="mu")
                nc.vector.tensor_scalar(out=mu[:], in0=stg[:, 0:1],
                                        scalar1=1.0 / N, scalar2=None, op0=OP.mult)
                ex2 = wp.tile([D1, 1], f32, tag="ex2")
                nc.vector.tensor_scalar(out=ex2[:], in0=stg[:, 1:2],
                                        scalar1=1.0 / N, scalar2=None, op0=OP.mult)
                mu2 = wp.tile([D1, 1], f32, tag="mu2")
                nc.vector.tensor_tensor(out=mu2[:], in0=mu[:], in1=mu[:], op=OP.mult)
                var = wp.tile([D1, 1], f32, tag="var")
                nc.vector.tensor_tensor(out=var[:], in0=ex2[:], in1=mu2[:],
                                        op=OP.subtract)
                ve = wp.tile([D1, 1], f32, tag="ve")
                nc.vector.tensor_scalar(out=ve[:], in0=var[:], scalar1=EPS,
                                        scalar2=None, op0=OP.add)
                lv = wp.tile([D1, 1], f32, tag="lv")
                nc.scalar.activation(out=lv[:], in_=ve[:], func=AF.Ln, scale=1.0)
                isd = wp.tile([D1, 1], f32, tag="isd")
                nc.scalar.activation(out=isd[:], in_=lv[:], func=AF.Exp, scale=-0.5)
                scale = wp.tile([D1, 1], f32, tag="scale")
                nc.vector.tensor_tensor(out=scale[:], in0=isd[:],
                                        in1=gmt[:, l:l + 1], op=OP.mult)
                mshift = wp.tile([D1, 1], f32, tag="mshift")
                nc.vector.tensor_tensor(out=mshift[:], in0=mu[:], in1=scale[:],
                                        op=OP.mult)
                shift = wp.tile([D1, 1], f32, tag="shift")
                nc.vector.tensor_tensor(out=shift[:], in0=bbt[:, l:l + 1],
                                        in1=mshift[:], op=OP.subtract)
                # h = relu(h + aggr*scale + shift)
                asb = scp.tile([D1, SHARD_P], f32, tag="asb")
                nc.vector.tensor_scalar(out=asb[:], in0=aggr_sb[:],
                                        scalar1=scale[:], scalar2=shift[:],
                                        op0=OP.mult, op1=OP.add)
                nc.vector.tensor_tensor(out=asb[:], in0=asb[:], in1=hT_own[:],
                                        op=OP.add)
                nc.vector.tensor_scalar(out=hT_own[:], in0=asb[:],
                                        scalar1=0.0, scalar2=None, op0=OP.max)

            # ---------- global mean pool ----------
            pool_ps = pp.tile([D1, G], f32, tag="pre")
            for w in range(NWIN):
                tp = ppB.tile([128, D1], f32, tag="bld")
                nc.tensor.transpose(out=tp[:], in_=hT_own[:, w * 128:(w + 1) * 128],
                                    identity=idn[0:D1, 0:D1])
                rows = wp.tile([128, D1], bf16, tag="rows")
                nc.vector.tensor_copy(out=rows[:], in_=tp[:])
                ohg = ohp.tile([128, G], bf16, tag="ohg")
                nc.vector.tensor_scalar(
                    out=ohg[:], in0=io256[:],
                    scalar1=blc[:, w:w + 1], scalar2=rgp[:, w:w + 1],
                    op0=OP.is_equal, op1=OP.mult)
                nc.tensor.matmul(out=pool_ps[:], lhsT=rows[:], rhs=ohg[:],
                                 start=(w == 0), stop=(w == NWIN - 1))
            poolT = wp.tile([D1, G], f32, tag="poolT")
            nc.vector.tensor_copy(out=poolT[:], in_=pool_ps[:])
            nc.sync.dma_start(out=pl_in[:, :], in_=poolT[:])
            nc.gpsimd.collective_compute(
                "AllReduce", OP.add, replica_groups=rg,
                ins=[pl_in.ap().opt()], outs=[pl_out.ap().opt()])
            pg = wp.tile([D1, G], f32, tag="pg")
            nc.sync.dma_start(out=pg[:], in_=pl_out[:, :])

            # ---------- head ----------
            a = pg
            hw_ = [(l1w[:], l1b[:]), (fw[:, 0:D2], fb[:, 0:1]), (fw[:, D2:2 * D2], fb[:, 1:2])]
            for (wt, bt) in hw_:
                ps = ppB.tile([D2, G], f32, tag="bld")
                nc.tensor.matmul(out=ps[:, 0:G], lhsT=wt, rhs=a[:], start=True, stop=True)
                an = wp.tile([D2, G], f32, tag="an")
                nc.scalar.activation(out=an[:], in_=ps[:, 0:G], func=AF.Relu,
                                     bias=bt, scale=1.0)
                a = an
            ps = ppB.tile([1, G], f32, tag="bld")
            nc.tensor.matmul(out=ps[:, 0:G], lhsT=l2w[:], rhs=a[:], start=True, stop=True)
            yt = wp.tile([1, G], f32, tag="yt")
            nc.scalar.activation(out=yt[:], in_=ps[:, 0:G], func=AF.Identity,
                                 bias=l2b[:], scale=1.0)
            nc.sync.dma_start(out=yout[:, :], in_=yt[:])

    nc.compile()
    return nc


def _wrap16(idx):
    """Flat idx list -> [128, n/16] int16: slot i at [i%16, i//16], replicated
    across the 8 Q7 cores."""
    a = idx.reshape(-1, 16).T.astype(np.int16)
    return np.tile(a, (8, 1))


def _preprocess(inputs):
    x = np.asarray(inputs["x"], np.float32)
    ea = np.asarray(inputs["edge_attr"], np.float32)
    ei = np.asarray(inputs["edge_index"]).astype(np.int64)
    batch = np.asarray(inputs["batch"]).astype(np.int64)
    src, dst = ei[0], ei[1]

    cnt = np.bincount(dst, minlength=N).astype(np.float32)
    rc_node = 1.0 / np.maximum(cnt, 1.0)
    gcnt = np.bincount(batch, minlength=G).astype(np.float32)
    rgc = 1.0 / np.maximum(gcnt, 1.0)

    order = np.argsort(dst, kind="stable")
    src_s, dst_s, ea_idx = src[order], dst[order], order
    core_s = dst_s // SHARD

    bounds = []
    for c in range(NCORES):
        for w in range(NWIN):
            bounds.append(c * SHARD + min(w * 128, SHARD))
    bounds.append(N)
    bpos = np.searchsorted(dst_s, np.asarray(bounds), side="left")
    percw = {}
    maxcnt = 0
    k = 0
    for c in range(NCORES):
        for w in range(NWIN):
            lo, hi = bpos[k], bpos[k + 1]
            percw[(c, w)] = np.arange(lo, hi)
            maxcnt = max(maxcnt, hi - lo)
            k += 1
    cw = max(1, (maxcnt + 127) // 128)
    etot = NWIN * cw * 128

    per_core = []
    for c in range(NCORES):
        qs_idx = np.zeros(etot, np.int64)
        dl = np.full(etot, -1.0, np.float32)
        rc_e = np.ones(etot, np.float32)
        ea_e = np.zeros((etot, EF), np.float32)
        for w in range(NWIN):
            idxs = percw[(c, w)]
            o = w * cw * 128
            k = len(idxs)
            s = src_s[idxs]
            g = (s // SHARD) * SHARD_P + (s % SHARD)   # padded global id
            qs_idx[o:o + k] = (g % 128) * NWING + (g // 128)
            loc = dst_s[idxs] - c * SHARD              # 0..3749
            dl[o:o + k] = (loc - w * 128).astype(np.float32)
            rc_e[o:o + k] = rc_node[dst_s[idxs]]
            ea_e[o:o + k] = ea[ea_idx[idxs]]
        eaT = np.zeros((64, etot), np.float32)
        eaT[0] = dl
        eaT[1:EF + 1] = ea_e.T
        eaT[EF + 1] = 1.0
        eaT[EF + 1, dl < 0] = 0.0
        nch = etot // 128
        d = {
            "qs_idxD": _wrap16(qs_idx),
            "dstloc_p": dl.reshape(nch, 128).T.copy(),
            "rc_p": rc_e.reshape(nch, 128).T.copy(),
            "eaT": eaT.astype(ml_dtypes.bfloat16),
        }
        xp = np.zeros((NF, SHARD_P), np.float32)
        xp[:, :SHARD] = x[c * SHARD:(c + 1) * SHARD].T
        d["xT"] = xp.astype(ml_dtypes.bfloat16)
        bl = np.full(SHARD_P, -1.0, np.float32)
        bl[:SHARD] = batch[c * SHARD:(c + 1) * SHARD].astype(np.float32)
        rg_n = np.zeros(SHARD_P, np.float32)
        rg_n[:SHARD] = rgc[batch[c * SHARD:(c + 1) * SHARD]]
        d["batchloc"] = bl.reshape(NWIN, 128).T.copy()
        d["rgc_pn"] = rg_n.reshape(NWIN, 128).T.copy()
        per_core.append(d)

    # replicated weights; f-gate half negated so pre = [-a | b]
    wf = np.asarray(inputs["conv_wf"], np.float32)
    wsv = np.asarray(inputs["conv_ws"], np.float32)
    bf = np.asarray(inputs["conv_bf"], np.float32)
    bs = np.asarray(inputs["conv_bs"], np.float32)
    wdst = np.concatenate([-wf[:, 0:D1, :], wsv[:, 0:D1, :]], axis=2)
    wsrc = np.concatenate([-wf[:, D1:2 * D1, :], wsv[:, D1:2 * D1, :]], axis=2)
    wea = np.concatenate([-wf[:, 2 * D1:, :], wsv[:, 2 * D1:, :]], axis=2)
    bias = np.concatenate([-bf, bs], axis=1)[:, None, :]
    wea = np.concatenate([wea, bias], axis=1)
    shared = {
        "lin0w": np.asarray(inputs["lin0_w"], np.float32).astype(ml_dtypes.bfloat16),
        "lin0b": np.asarray(inputs["lin0_b"], np.float32).reshape(D1, 1),
        "wdst": np.transpose(wdst, (1, 0, 2)).reshape(D1, L * 128).astype(ml_dtypes.bfloat16),
        "wsrc": np.transpose(wsrc, (1, 0, 2)).reshape(D1, L * 128).astype(ml_dtypes.bfloat16),
        "wea": np.concatenate([
            np.zeros((1, L * 128), np.float32),
            np.transpose(wea, (1, 0, 2)).reshape(EF + 1, L * 128),
            np.zeros((64 - EF - 2, L * 128), np.float32),
        ], axis=0).astype(ml_dtypes.bfloat16),
        "bng": np.asarray(inputs["bn_gamma"], np.float32).T.copy(),
        "bnb": np.asarray(inputs["bn_beta"], np.float32).T.copy(),
        "lin1w": np.asarray(inputs["lin1_w"], np.float32),
        "lin1b": np.asarray(inputs["lin1_b"], np.float32).reshape(D2, 1),
        "fcw": np.transpose(np.asarray(inputs["fc_w"], np.float32), (1, 0, 2)).reshape(D2, FC * D2),
        "fcb": np.asarray(inputs["fc_b"], np.float32).T.copy(),
        "lin2w": np.asarray(inputs["lin2_w"], np.float32).reshape(D2, 1),
        "lin2b": np.asarray(inputs["lin2_b"], np.float32).reshape(1, 1),
        "iota128": np.broadcast_to(np.arange(128, dtype=np.float32)[None, :],
                                   (128, 128)).astype(ml_dtypes.bfloat16),
        "iota256": np.broadcast_to(np.arange(G, dtype=np.float32)[None, :],
                                   (128, G)).astype(ml_dtypes.bfloat16),
        "ident": np.eye(128, dtype=np.float32),
        "identb": np.eye(128, dtype=np.float32).astype(ml_dtypes.bfloat16),
        "onesD": np.ones((1, 128), np.float32).astype(ml_dtypes.bfloat16),
        "iotapD": np.arange(128, dtype=np.float32).reshape(128, 1),
    }
    in_maps = [dict(shared, **pc) for pc in per_core]
    return in_maps, cw


def kernel(**inputs):
    from concourse.bass_utils import run_bass_kernel_spmd

    in_maps, cw = _preprocess(inputs)
    key = ("nc", cw)
    if key not in _CACHE:
        _CACHE[key] = _build_nc(cw)
    nc = _CACHE[key]
    res = run_bass_kernel_spmd(nc, in_maps, core_ids=list(range(NCORES)))
    return res.results[0]["y"].reshape(G).astype(np.float32)


# revision 22
# speedup vs baseline: 4.5234x; 1.0362x over previous
"""CGCNN message-passing kernel for 8 Trainium2 NeuronCores (Bass/Tile), v2.

Strategy (data-parallel by dst-node range, gather-based edge pipeline):
- Nodes split into 8 shards of 3750 (padded 3840 = 30 windows x 128). Edges
  assigned to the core owning dst, grouped by 128-node dst window, padded to a
  uniform chunks-per-window count cw (SPMD-uniform program).
- Per layer, per core:
  * Qd table (own shard)  = h_own  @ Wdst  -> DRAM [3840, 128] bf16
  * AllGather h (bf16), then Qs table (all nodes) = h_full @ Wsrc
    -> DRAM [30720, 128] bf16 (partition-major row order for fat DMA runs)
  * per 1024-edge tile: dma_gather Qd rows + Qs rows (1024 descriptors each),
    Qe = ea @ Wea as matmul, summed in PSUM via identity-matmul adds.
    f-gate columns are negated at preprocessing, so one joint exp pass gives
    u=[e^-a | e^b]; v=ln(1+u)=[sp(-a) | sp(b)]; sigmoid(a)=e^(-sp(-a));
    m = sigmoid * softplus (bf16).
  * segment-mean via onehot matmul (is_equal(iota,dst)*1/cnt, bf16) into PSUM
    accumulated per dst window; BatchNorm batch stats via tiny AllReduce;
    residual + relu on the own shard.
- Global mean pool via onehot matmul, partial sums AllReduced, tiny head MLP
  computed redundantly on every core.
"""
import numpy as np
import ml_dtypes

N = 30000
E = 480000
NF = 92
EF = 50
D1 = 64
D2 = 64
L = 3
FC = 2
G = 256
EPS = 1e-5
NCORES = 8
SHARD = N // NCORES            # 3750
SHARD_P = 3840                 # padded shard (30 windows of 128)
NWIN = SHARD_P // 128          # 30
NWING = NCORES * NWIN          # 240 global windows
TBL = NCORES * SHARD_P         # 30720 table rows

_CACHE = {}


def _build_nc(cw):
    """Build the SPMD bass module. cw = chunks per dst window (uniform)."""
    import concourse.mybir as mybir
    from concourse import bacc
    from concourse.tile import TileContext

    f32 = mybir.dt.float32
    bf16 = mybir.dt.bfloat16
    f8 = mybir.dt.float8e4
    i16 = mybir.dt.int16
    AF = mybir.ActivationFunctionType
    OP = mybir.AluOpType

    nchunk = NWIN * cw                 # chunks per layer per core
    etot = nchunk * 128                # padded edges per core
    ntile = (nchunk + 7) // 8          # 8-chunk (1024-edge) tiles

    import concourse.hw_specs as _hw
    import concourse.bacc as _bacc_mod
    _real_tables = _hw.get_activation_tables("gen3")
    _combined = "natural_log_exp_and_others"
    if _combined in _real_tables:
        _patched = {
            k: (v if k == _combined else (v - {AF.Exp, AF.Ln}))
            for k, v in _real_tables.items()
        }
        _bacc_mod.get_activation_tables = lambda arch: _patched

    nc = bacc.Bacc(None, target_bir_lowering=False)

    # ---- inputs (per core) ----
    xT = nc.dram_tensor("xT", [NF, SHARD_P], bf16, kind="ExternalInput")
    eaT = nc.dram_tensor("eaT", [64, etot], bf16, kind="ExternalInput")
    qs_idxD = nc.dram_tensor("qs_idxD", [128, etot // 16], i16, kind="ExternalInput")
    onesD = nc.dram_tensor("onesD", [1, 128], bf16, kind="ExternalInput")
    iotapD = nc.dram_tensor("iotapD", [128, 1], f32, kind="ExternalInput")
    dstloc_p = nc.dram_tensor("dstloc_p", [128, nchunk], f32, kind="ExternalInput")
    rc_p = nc.dram_tensor("rc_p", [128, nchunk], f32, kind="ExternalInput")
    batchloc = nc.dram_tensor("batchloc", [128, NWIN], f32, kind="ExternalInput")
    rgc_pn = nc.dram_tensor("rgc_pn", [128, NWIN], f32, kind="ExternalInput")
    # weights (replicated; f-gate halves pre-negated)
    lin0w = nc.dram_tensor("lin0w", [NF, D1], bf16, kind="ExternalInput")
    lin0b = nc.dram_tensor("lin0b", [D1, 1], f32, kind="ExternalInput")
    wdst = nc.dram_tensor("wdst", [D1, L * 128], bf16, kind="ExternalInput")
    wsrc = nc.dram_tensor("wsrc", [D1, L * 128], bf16, kind="ExternalInput")
    wea = nc.dram_tensor("wea", [64, L * 128], bf16, kind="ExternalInput")
    bng = nc.dram_tensor("bng", [D1, L], f32, kind="ExternalInput")
    bnb = nc.dram_tensor("bnb", [D1, L], f32, kind="ExternalInput")
    lin1w = nc.dram_tensor("lin1w", [D1, D2], f32, kind="ExternalInput")
    lin1b = nc.dram_tensor("lin1b", [D2, 1], f32, kind="ExternalInput")
    fcw = nc.dram_tensor("fcw", [D2, FC * D2], f32, kind="ExternalInput")
    fcb = nc.dram_tensor("fcb", [D2, FC], f32, kind="ExternalInput")
    lin2w = nc.dram_tensor("lin2w", [D2, 1], f32, kind="ExternalInput")
    lin2b = nc.dram_tensor("lin2b", [1, 1], f32, kind="ExternalInput")
    iota128 = nc.dram_tensor("iota128", [128, 128], bf16, kind="ExternalInput")
    iota256 = nc.dram_tensor("iota256", [128, G], bf16, kind="ExternalInput")
    ident = nc.dram_tensor("ident", [128, 128], f32, kind="ExternalInput")
    identb = nc.dram_tensor("identb", [128, 128], bf16, kind="ExternalInput")

    yout = nc.dram_tensor("y", [1, G], f32, kind="ExternalOutput")

    # ---- DRAM scratch ----
    QsD = nc.dram_tensor("QsD", [TBL, 128], bf16)          # row p*NWING+W
    ag_in = nc.dram_tensor("ag_in", [D1, SHARD_P], f8)
    ag_out = nc.dram_tensor("ag_out", [NCORES * D1, SHARD_P], f8,
                            addr_space="Shared")
    ar_in = nc.dram_tensor("ar_in", [D1, 2], f32)
    ar_out = nc.dram_tensor("ar_out", [NCORES * D1, 2], f32, addr_space="Shared")
    pl_in = nc.dram_tensor("pl_in", [D1, G], f32)
    pl_out = nc.dram_tensor("pl_out", [D1, G], f32, addr_space="Shared")

    rg = [list(range(NCORES))]
    QsD3 = QsD[:, :].rearrange("(p w) f -> p w f", p=128)   # [128, NWING, 128]

    with TileContext(nc) as tc:
        with (
            tc.tile_pool(name="const", bufs=1) as cp,
            tc.tile_pool(name="big", bufs=1) as bigp,
            tc.tile_pool(name="work", bufs=2) as wp,
            tc.tile_pool(name="gat", bufs=4) as gp,
            tc.tile_pool(name="nl", bufs=3) as nlp,
            tc.tile_pool(name="oh", bufs=3) as ohp,
            tc.tile_pool(name="st", bufs=2) as stp,
            tc.tile_pool(name="scr", bufs=1) as scp,
            tc.tile_pool(name="pre", bufs=2, space="PSUM") as pp,
            tc.tile_pool(name="psB", bufs=2, space="PSUM") as ppB,
            tc.tile_pool(name="psA", bufs=2, space="PSUM") as ppA,
        ):
            # ---------- constants ----------
            def load_const(t, dram, shape, dtype=f32):
                tt = cp.tile(shape, dtype, tag=t)
                nc.sync.dma_start(out=tt[:], in_=dram)
                return tt

            io128 = load_const("io128", iota128[:, :], [128, 128], bf16)
            io256 = load_const("io256", iota256[:, :], [128, G], bf16)
            idn = load_const("idn", ident[:, :], [128, 128])
            idnb = load_const("idnb", identb[:, :], [128, 128], bf16)
            l0w = load_const("l0w", lin0w[:, :], [NF, D1], bf16)
            l0b = load_const("l0b", lin0b[:, :], [D1, 1])
            wd = load_const("wd", wdst[:, :], [D1, L * 128], bf16)
            ws = load_const("ws", wsrc[:, :], [D1, L * 128], bf16)
            we = load_const("we", wea[:, :], [64, L * 128], bf16)
            gmt = load_const("gmt", bng[:, :], [D1, L])
            bbt = load_const("bbt", bnb[:, :], [D1, L])
            l1w = load_const("l1w", lin1w[:, :], [D1, D2])
            l1b = load_const("l1b", lin1b[:, :], [D2, 1])
            fw = load_const("fw", fcw[:, :], [D2, FC * D2])
            fb = load_const("fb", fcb[:, :], [D2, FC])
            l2w = load_const("l2w", lin2w[:, :], [D2, 1])
            l2b = load_const("l2b", lin2b[:, :], [1, 1])
            dlp = load_const("dlp", dstloc_p[:, :], [128, nchunk])
            rcp = load_const("rcp", rc_p[:, :], [128, nchunk])
            blc = load_const("blc", batchloc[:, :], [128, NWIN])
            rgp = load_const("rgp", rgc_pn[:, :], [128, NWIN])
            qsix = load_const("qsix", qs_idxD[:, :], [128, etot // 16], i16)
            onesb = load_const("onesb", onesD[:, :], [1, 128], bf16)
            iotap = load_const("iotap", iotapD[:, :], [128, 1])

            # ---------- resident state ----------
            hT_own = bigp.tile([D1, SHARD_P], f32, tag="hown")
            hb_own = bigp.tile([D1, SHARD_P], bf16, tag="hbown")
            aggr_sb = bigp.tile([D1, SHARD_P], f32, tag="aggr")
            qd_sb = bigp.tile([128, NWIN, 128], bf16, tag="qdsb")

            # ---------- lin0: hT_own = relu(lin0w.T @ xT + b) ----------
            for j in range(8):
                sl = slice(j * 480, (j + 1) * 480)
                xt = wp.tile([NF, 480], bf16, tag="xt")
                nc.sync.dma_start(out=xt[:], in_=xT[:, sl])
                ph = ppB.tile([D1, 480], f32, tag="bld")
                nc.tensor.matmul(out=ph[:], lhsT=l0w[:], rhs=xt[:],
                                 start=True, stop=True)
                nc.scalar.activation(out=hT_own[:, sl], in_=ph[:],
                                     func=AF.Relu, bias=l0b[:], scale=1.0)

            # ---------- layers ----------
            for l in range(L):
                wd_l = wd[:, l * 128:(l + 1) * 128]
                ws_l = ws[:, l * 128:(l + 1) * 128]
                we_l = we[:, l * 128:(l + 1) * 128]

                # --- bf16 copy of own h ---
                nc.scalar.activation(out=hb_own[:], in_=hT_own[:],
                                     func=AF.Identity, scale=1.0)

                # --- Qd table build (own shard) -> QdD ---
                for w0 in range(0, NWIN, 4):
                    kk = min(4, NWIN - w0)
                    qp = ppB.tile([128, 512], f32, tag="bld")
                    for k in range(kk):
                        w = w0 + k
                        nc.tensor.matmul(
                            out=qp[:, k * 128:(k + 1) * 128],
                            lhsT=hb_own[:, w * 128:(w + 1) * 128],
                            rhs=wd_l, start=True, stop=True)
                    nc.scalar.activation(
                        out=qd_sb[:, w0:w0 + kk, :].rearrange("p a b -> p (a b)"),
                        in_=qp[:, :kk * 128],
                        func=AF.Identity, scale=1.0)

                # --- AllGather h (fp8) ---
                h8 = stp.tile([D1, SHARD_P], f8, tag="h8")
                nc.scalar.activation(out=h8[:], in_=hT_own[:],
                                     func=AF.Identity, scale=1.0)
                nc.sync.dma_start(out=ag_in[:, :], in_=h8[:])
                nc.gpsimd.collective_compute(
                    "AllGather", OP.bypass, replica_groups=rg,
                    ins=[ag_in.ap().opt()], outs=[ag_out.ap().opt()])

                # --- Qs table build (all nodes, per gathered shard) -> QsD ---
                ws8 = stp.tile([D1, 128], f8, tag="ws8")
                nc.scalar.activation(out=ws8[:], in_=ws_l,
                                     func=AF.Identity, scale=1.0)
                for s_ in range(NCORES):
                    hb_sh = stp.tile([D1, SHARD_P], f8, tag="hbsh")
                    nc.sync.dma_start(out=hb_sh[:],
                                      in_=ag_out[s_ * D1:(s_ + 1) * D1, :])
                    for wB in range(0, NWIN, 16):
                        kB = min(16, NWIN - wB)
                        sg_t = stp.tile([128, 16, 128], bf16, tag="qsst")
                        for w0 in range(wB, wB + kB, 4):
                            kk = min(4, wB + kB - w0)
                            qp = ppB.tile([128, 512], f32, tag="bld")
                            for k in range(kk):
                                w = w0 + k
                                nc.tensor.matmul(
                                    out=qp[:, k * 128:(k + 1) * 128],
                                    lhsT=hb_sh[:, w * 128:(w + 1) * 128],
                                    rhs=ws8[:], start=True, stop=True)
                            nc.scalar.activation(
                                out=sg_t[:, w0 - wB:w0 - wB + kk, :]
                                    .rearrange("p a b -> p (a b)"),
                                in_=qp[:, :kk * 128],
                                func=AF.Identity, scale=1.0)
                        W0 = s_ * NWIN + wB
                        nc.sync.dma_start(out=QsD3[:, W0:W0 + kB, :],
                                          in_=sg_t[:, :kB, :])

                # --- edge pipeline ---
                agg = None
                for t in range(ntile):
                    te = min(8, nchunk - t * 8)          # chunks this tile
                    ne = te * 128                        # edges this tile
                    qs_g = gp.tile([128, 8, 128], bf16, tag="qsg")
                    nc.gpsimd.dma_gather(
                        qs_g[:, :te, :], QsD[:, :],
                        qsix[:, t * 64: t * 64 + te * 8], ne, ne, 128)
                    et = gp.tile([64, 1024], bf16, tag="et")
                    nc.sync.dma_start(out=et[:, :ne],
                                      in_=eaT[:, t * 1024: t * 1024 + ne])
                    ohT = ohp.tile([128, 1024], bf16, tag="ohT")
                    for h0 in range(0, ne, 512):
                        hn = min(512, ne - h0)
                        bc = ppB.tile([128, 512], f32, tag="bld")
                        nc.tensor.matmul(out=bc[:, :hn], lhsT=onesb[:],
                                         rhs=et[0:1, h0:h0 + hn],
                                         start=True, stop=True)
                        nc.vector.tensor_scalar(
                            out=ohT[:, h0:h0 + hn], in0=bc[:, :hn],
                            scalar1=iotap[:], scalar2=None, op0=OP.is_equal)

                    pre = pp.tile([128, 1024], f32, tag="pre")
                    qs_f = qs_g[:].rearrange("p a b -> p (a b)")
                    for c in range(te):
                        gc = t * 8 + c
                        w = gc // cw
                        csl = slice(c * 128, (c + 1) * 128)
                        nc.tensor.matmul(out=pre[:, csl], lhsT=et[:, csl],
                                         rhs=we_l, start=True, stop=False)
                        nc.tensor.matmul(out=pre[:, csl], lhsT=idnb[:],
                                         rhs=qs_f[:, csl], start=False, stop=False)
                        nc.tensor.matmul(out=pre[:, csl], lhsT=ohT[:, csl],
                                         rhs=qd_sb[:, w, :], start=False, stop=True)

                    u = nlp.tile([128, 1024], f32, tag="u")
                    nc.scalar.activation(out=u[:, :ne], in_=pre[:, :ne],
                                         func=AF.Exp, scale=1.0)
                    v = nlp.tile([128, 8, 128], f32, tag="v")
                    nc.scalar.activation(
                        out=v[:, :te, :].rearrange("p a b -> p (a b)"),
                        in_=u[:, :ne], func=AF.Ln, bias=1.0, scale=1.0)
                    sg = nlp.tile([128, 8, 64], f32, tag="sg")
                    nc.scalar.activation(out=sg[:, :te, :], in_=v[:, :te, 0:64],
                                         func=AF.Exp, scale=-1.0)
                    m = nlp.tile([128, 8, 64], bf16, tag="m")
                    nc.vector.tensor_tensor(out=m[:, :te, :], in0=sg[:, :te, :],
                                            in1=v[:, :te, 64:128], op=OP.mult)

                    for c in range(te):
                        gc = t * 8 + c
                        w = gc // cw
                        if gc % (4 * cw) == 0:
                            agg = ppA.tile([D1, 512], f32, tag="agg")
                        ohS = ohp.tile([128, 128], bf16, tag="ohS")
                        nc.vector.tensor_scalar(
                            out=ohS[:], in0=io128[:],
                            scalar1=dlp[:, gc:gc + 1], scalar2=rcp[:, gc:gc + 1],
                            op0=OP.is_equal, op1=OP.mult)
                        nc.tensor.matmul(
                            out=agg[:, (w % 4) * 128:(w % 4 + 1) * 128],
                            lhsT=m[:, c, :], rhs=ohS[:],
                            start=(gc % cw == 0), stop=(gc % cw == cw - 1))
                        if gc % (4 * cw) == 4 * cw - 1 or gc == nchunk - 1:
                            grp = w // 4
                            lo = grp * 512
                            hi = min(lo + 512, SHARD_P)
                            nc.scalar.activation(
                                out=aggr_sb[:, lo:hi], in_=agg[:, :hi - lo],
                                func=AF.Identity, scale=1.0)

                # --- BN stats + AllReduce ---
                st = wp.tile([D1, 2], f32, tag="st")
                nc.vector.reduce_sum(out=st[:, 0:1], in_=aggr_sb[:],
                                     axis=mybir.AxisListType.X)
                sq = scp.tile([D1, SHARD_P], f32, tag="sq")
                nc.vector.tensor_tensor(out=sq[:], in0=aggr_sb[:],
                                        in1=aggr_sb[:], op=OP.mult)
                nc.vector.reduce_sum(out=st[:, 1:2], in_=sq[:],
                                     axis=mybir.AxisListType.X)
                nc.sync.dma_start(out=ar_in[:, :], in_=st[:])
                nc.gpsimd.collective_compute(
                    "AllGather", OP.bypass, replica_groups=rg,
                    ins=[ar_in.ap().opt()], outs=[ar_out.ap().opt()])
                stga = wp.tile([D1, 2, NCORES], f32, tag="stga")
                nc.sync.dma_start(
                    out=stga[:],
                    in_=ar_out[:, :].rearrange("(c p) s -> p s c", p=D1))
                stg = wp.tile([D1, 2], f32, tag="stg")
                nc.vector.reduce_sum(
                    out=stg[:].rearrange("p (s o) -> p s o", o=1),
                    in_=stga[:], axis=mybir.AxisListType.X)
                mu = wp.tile([D1, 1], f32, tag="mu")
                nc.vector.tensor_scalar(out=mu[:], in0=stg[:, 0:1],
                                        scalar1=1.0 / N, scalar2=None, op0=OP.mult)
                ex2 = wp.tile([D1, 1], f32, tag="ex2")
                nc.vector.tensor_scalar(out=ex2[:], in0=stg[:, 1:2],
                                        scalar1=1.0 / N, scalar2=None, op0=OP.mult)
                mu2 = wp.tile([D1, 1], f32, tag="mu2")
                nc.vector.tensor_tensor(out=mu2[:], in0=mu[:], in1=mu[:], op=OP.mult)
                var = wp.tile([D1, 1], f32, tag="var")
                nc.vector.tensor_tensor(out=var[:], in0=ex2[:], in1=mu2[:],
                                        op=OP.subtract)
                ve = wp.tile([D1, 1], f32, tag="ve")
                nc.vector.tensor_scalar(out=ve[:], in0=var[:], scalar1=EPS,
                                        scalar2=None, op0=OP.add)
                lv = wp.tile([D1, 1], f32, tag="lv")
                nc.scalar.activation(out=lv[:], in_=ve[:], func=AF.Ln, scale=1.0)
                isd = wp.tile([D1, 1], f32, tag="isd")
                nc.scalar.activation(out=isd[:], in_=lv[:], func=AF.Exp, scale=-0.5)
                scale = wp.tile([D1, 1], f32, tag="scale")
                nc.vector.tensor_tensor(out=scale[:], in0=isd[:],
                                        in1=gmt[:, l:l + 1], op=OP.mult)
                mshift = wp.tile([D1, 1], f32, tag="mshift")
                nc.vector.tensor_tensor(out=mshift[:], in0=mu[:], in1=scale[:],
                                        op=OP.mult)
                shift = wp.tile([D1, 1], f32, tag="shift")
                nc.vector.tensor_tensor(out=shift[:], in0=bbt[:, l:l + 1],
                                        in1=mshift[:], op=OP.subtract)
                # h = relu(h + aggr*scale + shift)
                asb = scp.tile([D1, SHARD_P], f32, tag="asb")
                nc.vector.tensor_scalar(out=asb[:], in0=aggr_sb[:],
                                        scalar1=scale[:], scalar2=shift[:],
                                        op0=OP.mult, op1=OP.add)
                nc.vector.tensor_tensor(out=asb[:], in0=asb[:], in1=hT_own[:],
                                        op=OP.add)
                nc.vector.tensor_scalar(out=hT_own[:], in0=asb[:],
                                        scalar1=0.0, scalar2=None, op0=OP.max)

            # ---------- global mean pool ----------
            pool_ps = pp.tile([D1, G], f32, tag="pre")
            for w in range(NWIN):
                tp = ppB.tile([128, D1], f32, tag="bld")
                nc.tensor.transpose(out=tp[:], in_=hT_own[:, w * 128:(w + 1) * 128],
                                    identity=idn[0:D1, 0:D1])
                rows = wp.tile([128, D1], bf16, tag="rows")
                nc.vector.tensor_copy(out=rows[:], in_=tp[:])
                ohg = ohp.tile([128, G], bf16, tag="ohg")
                nc.vector.tensor_scalar(
                    out=ohg[:], in0=io256[:],
                    scalar1=blc[:, w:w + 1], scalar2=rgp[:, w:w + 1],
                    op0=OP.is_equal, op1=OP.mult)
                nc.tensor.matmul(out=pool_ps[:], lhsT=rows[:], rhs=ohg[:],
                                 start=(w == 0), stop=(w == NWIN - 1))
            poolT = wp.tile([D1, G], f32, tag="poolT")
            nc.vector.tensor_copy(out=poolT[:], in_=pool_ps[:])
            nc.sync.dma_start(out=pl_in[:, :], in_=poolT[:])
            nc.gpsimd.collective_compute(
                "AllReduce", OP.add, replica_groups=rg,
                ins=[pl_in.ap().opt()], outs=[pl_out.ap().opt()])
            pg = wp.tile([D1, G], f32, tag="pg")
            nc.sync.dma_start(out=pg[:], in_=pl_out[:, :])

            # ---------- head ----------
            a = pg
            hw_ = [(l1w[:], l1b[:]), (fw[:, 0:D2], fb[:, 0:1]), (fw[:, D2:2 * D2], fb[:, 1:2])]
            for (wt, bt) in hw_:
                ps = ppB.tile([D2, G], f32, tag="bld")
                nc.tensor.matmul(out=ps[:, 0:G], lhsT=wt, rhs=a[:], start=True, stop=True)
                an = wp.tile([D2, G], f32, tag="an")
                nc.scalar.activation(out=an[:], in_=ps[:, 0:G], func=AF.Relu,
                                     bias=bt, scale=1.0)
                a = an
            ps = ppB.tile([1, G], f32, tag="bld")
            nc.tensor.matmul(out=ps[:, 0:G], lhsT=l2w[:], rhs=a[:], start=True, stop=True)
            yt = wp.tile([1, G], f32, tag="yt")
            nc.scalar.activation(out=yt[:], in_=ps[:, 0:G], func=AF.Identity,
                                 bias=l2b[:], scale=1.0)
            nc.sync.dma_start(out=yout[:, :], in_=yt[:])

    nc.compile()
    return nc


def _wrap16(idx):
    """Flat idx list -> [128, n/16] int16: slot i at [i%16, i//16], replicated
    across the 8 Q7 cores."""
    a = idx.reshape(-1, 16).T.astype(np.int16)
    return np.tile(a, (8, 1))


def _preprocess(inputs):
    x = np.asarray(inputs["x"], np.float32)
    ea = np.asarray(inputs["edge_attr"], np.float32)
    ei = np.asarray(inputs["edge_index"]).astype(np.int64)
    batch = np.asarray(inputs["batch"]).astype(np.int64)
    src, dst = ei[0], ei[1]

    cnt = np.bincount(dst, minlength=N).astype(np.float32)
    rc_node = 1.0 / np.maximum(cnt, 1.0)
    gcnt = np.bincount(batch, minlength=G).astype(np.float32)
    rgc = 1.0 / np.maximum(gcnt, 1.0)

    order = np.argsort(dst, kind="stable")
    src_s, dst_s, ea_idx = src[order], dst[order], order
    core_s = dst_s // SHARD

    bounds = []
    for c in range(NCORES):
        for w in range(NWIN):
            bounds.append(c * SHARD + min(w * 128, SHARD))
    bounds.append(N)
    bpos = np.searchsorted(dst_s, np.asarray(bounds), side="left")
    percw = {}
    maxcnt = 0
    k = 0
    for c in range(NCORES):
        for w in range(NWIN):
            lo, hi = bpos[k], bpos[k + 1]
            percw[(c, w)] = np.arange(lo, hi)
            maxcnt = max(maxcnt, hi - lo)
            k += 1
    cw = max(1, (maxcnt + 127) // 128)
    etot = NWIN * cw * 128

    per_core = []
    for c in range(NCORES):
        qs_idx = np.zeros(etot, np.int64)
        dl = np.full(etot, -1.0, np.float32)
        rc_e = np.ones(etot, np.float32)
        ea_e = np.zeros((etot, EF), np.float32)
        for w in range(NWIN):
            idxs = percw[(c, w)]
            o = w * cw * 128
            k = len(idxs)
            s = src_s[idxs]
            g = (s // SHARD) * SHARD_P + (s % SHARD)   # padded global id
            qs_idx[o:o + k] = (g % 128) * NWING + (g // 128)
            loc = dst_s[idxs] - c * SHARD              # 0..3749
            dl[o:o + k] = (loc - w * 128).astype(np.float32)
            rc_e[o:o + k] = rc_node[dst_s[idxs]]
            ea_e[o:o + k] = ea[ea_idx[idxs]]
        eaT = np.zeros((64, etot), np.float32)
        eaT[0] = dl
        eaT[1:EF + 1] = ea_e.T
        eaT[EF + 1] = 1.0
        eaT[EF + 1, dl < 0] = 0.0
        nch = etot // 128
        d = {
            "qs_idxD": _wrap16(qs_idx),
            "dstloc_p": dl.reshape(nch, 128).T.copy(),
            "rc_p": rc_e.reshape(nch, 128).T.copy(),
            "eaT": eaT.astype(ml_dtypes.bfloat16),
        }
        xp = np.zeros((NF, SHARD_P), np.float32)
        xp[:, :SHARD] = x[c * SHARD:(c + 1) * SHARD].T
        d["xT"] = xp.astype(ml_dtypes.bfloat16)
        bl = np.full(SHARD_P, -1.0, np.float32)
        bl[:SHARD] = batch[c * SHARD:(c + 1) * SHARD].astype(np.float32)
        rg_n = np.zeros(SHARD_P, np.float32)
        rg_n[:SHARD] = rgc[batch[c * SHARD:(c + 1) * SHARD]]
        d["batchloc"] = bl.reshape(NWIN, 128).T.copy()
        d["rgc_pn"] = rg_n.reshape(NWIN, 128).T.copy()
        per_core.append(d)

    # replicated weights; f-gate half negated so pre = [-a | b]
    wf = np.asarray(inputs["conv_wf"], np.float32)
    wsv = np.asarray(inputs["conv_ws"], np.float32)
    bf = np.asarray(inputs["conv_bf"], np.float32)
    bs = np.asarray(inputs["conv_bs"], np.float32)
    wdst = np.concatenate([-wf[:, 0:D1, :], wsv[:, 0:D1, :]], axis=2)
    wsrc = np.concatenate([-wf[:, D1:2 * D1, :], wsv[:, D1:2 * D1, :]], axis=2)
    wea = np.concatenate([-wf[:, 2 * D1:, :], wsv[:, 2 * D1:, :]], axis=2)
    bias = np.concatenate([-bf, bs], axis=1)[:, None, :]
    wea = np.concatenate([wea, bias], axis=1)
    shared = {
        "lin0w": np.asarray(inputs["lin0_w"], np.float32).astype(ml_dtypes.bfloat16),
        "lin0b": np.asarray(inputs["lin0_b"], np.float32).reshape(D1, 1),
        "wdst": np.transpose(wdst, (1, 0, 2)).reshape(D1, L * 128).astype(ml_dtypes.bfloat16),
        "wsrc": np.transpose(wsrc, (1, 0, 2)).reshape(D1, L * 128).astype(ml_dtypes.bfloat16),
        "wea": np.concatenate([
            np.zeros((1, L * 128), np.float32),
            np.transpose(wea, (1, 0, 2)).reshape(EF + 1, L * 128),
            np.zeros((64 - EF - 2, L * 128), np.float32),
        ], axis=0).astype(ml_dtypes.bfloat16),
        "bng": np.asarray(inputs["bn_gamma"], np.float32).T.copy(),
        "bnb": np.asarray(inputs["bn_beta"], np.float32).T.copy(),
        "lin1w": np.asarray(inputs["lin1_w"], np.float32),
        "lin1b": np.asarray(inputs["lin1_b"], np.float32).reshape(D2, 1),
        "fcw": np.transpose(np.asarray(inputs["fc_w"], np.float32), (1, 0, 2)).reshape(D2, FC * D2),
        "fcb": np.asarray(inputs["fc_b"], np.float32).T.copy(),
        "lin2w": np.asarray(inputs["lin2_w"], np.float32).reshape(D2, 1),
        "lin2b": np.asarray(inputs["lin2_b"], np.float32).reshape(1, 1),
        "iota128": np.broadcast_to(np.arange(128, dtype=np.float32)[None, :],
                                   (128, 128)).astype(ml_dtypes.bfloat16),
        "iota256": np.broadcast_to(np.arange(G, dtype=np.float32)[None, :],
                                   (128, G)).astype(ml_dtypes.bfloat16),
        "ident": np.eye(128, dtype=np.float32),
        "identb": np.eye(128, dtype=np.float32).astype(ml_dtypes.bfloat16),
        "onesD": np.ones((1, 128), np.float32).astype(ml_dtypes.bfloat16),
        "iotapD": np.arange(128, dtype=np.float32).reshape(128, 1),
    }
    in_maps = [dict(shared, **pc) for pc in per_core]
    return in_maps, cw


def kernel(**inputs):
    from concourse.bass_utils import run_bass_kernel_spmd

    in_maps, cw = _preprocess(inputs)
    key = ("nc", cw)
    if key not in _CACHE:
        _CACHE[key] = _build_nc(cw)
    nc = _CACHE[key]
    res = run_bass_kernel_spmd(nc, in_maps, core_ids=list(range(NCORES)))
    return res.results[0]["y"].reshape(G).astype(np.float32)


# revision 28
# speedup vs baseline: 4.7579x; 1.0519x over previous
"""CGCNN message-passing kernel for 8 Trainium2 NeuronCores (Bass/Tile), v2.

Strategy (data-parallel by dst-node range, gather-based edge pipeline):
- Nodes split into 8 shards of 3750 (padded 3840 = 30 windows x 128). Edges
  assigned to the core owning dst, grouped by 128-node dst window, padded to a
  uniform chunks-per-window count cw (SPMD-uniform program).
- Per layer, per core:
  * Qd table (own shard)  = h_own  @ Wdst  -> DRAM [3840, 128] bf16
  * AllGather h (bf16), then Qs table (all nodes) = h_full @ Wsrc
    -> DRAM [30720, 128] bf16 (partition-major row order for fat DMA runs)
  * per 1024-edge tile: dma_gather Qd rows + Qs rows (1024 descriptors each),
    Qe = ea @ Wea as matmul, summed in PSUM via identity-matmul adds.
    f-gate columns are negated at preprocessing, so one joint exp pass gives
    u=[e^-a | e^b]; v=ln(1+u)=[sp(-a) | sp(b)]; sigmoid(a)=e^(-sp(-a));
    m = sigmoid * softplus (bf16).
  * segment-mean via onehot matmul (is_equal(iota,dst)*1/cnt, bf16) into PSUM
    accumulated per dst window; BatchNorm batch stats via tiny AllReduce;
    residual + relu on the own shard.
- Global mean pool via onehot matmul, partial sums AllReduced, tiny head MLP
  computed redundantly on every core.
"""
import numpy as np
import ml_dtypes

N = 30000
E = 480000
NF = 92
EF = 50
D1 = 64
D2 = 64
L = 3
FC = 2
G = 256
EPS = 1e-5
NCORES = 8
SHARD = N // NCORES            # 3750
SHARD_P = 3840                 # padded shard (30 windows of 128)
NWIN = SHARD_P // 128          # 30
NWING = NCORES * NWIN          # 240 global windows
TBL = NCORES * SHARD_P         # 30720 table rows

_CACHE = {}


def _build_nc(cw):
    """Build the SPMD bass module. cw = chunks per dst window (uniform)."""
    import concourse.mybir as mybir
    from concourse import bacc
    from concourse.tile import TileContext

    f32 = mybir.dt.float32
    bf16 = mybir.dt.bfloat16
    f8 = mybir.dt.float8e4
    i16 = mybir.dt.int16
    AF = mybir.ActivationFunctionType
    OP = mybir.AluOpType

    nchunk = NWIN * cw                 # chunks per layer per core
    etot = nchunk * 128                # padded edges per core
    ntile = (nchunk + 7) // 8          # 8-chunk (1024-edge) tiles

    import concourse.hw_specs as _hw
    import concourse.bacc as _bacc_mod
    _real_tables = _hw.get_activation_tables("gen3")
    _combined = "natural_log_exp_and_others"
    if _combined in _real_tables:
        _patched = {
            k: (v if k == _combined else (v - {AF.Exp, AF.Ln}))
            for k, v in _real_tables.items()
        }
        _bacc_mod.get_activation_tables = lambda arch: _patched

    nc = bacc.Bacc(None, target_bir_lowering=False)

    # ---- inputs (per core) ----
    xT = nc.dram_tensor("xT", [NF, SHARD_P], bf16, kind="ExternalInput")
    eaT = nc.dram_tensor("eaT", [64, etot], bf16, kind="ExternalInput")
    qs_idxD = nc.dram_tensor("qs_idxD", [128, etot // 16], i16, kind="ExternalInput")
    onesD = nc.dram_tensor("onesD", [1, 128], bf16, kind="ExternalInput")
    iotapD = nc.dram_tensor("iotapD", [128, 1], f32, kind="ExternalInput")
    dstloc_p = nc.dram_tensor("dstloc_p", [128, nchunk], f32, kind="ExternalInput")
    rc_p = nc.dram_tensor("rc_p", [128, nchunk], f32, kind="ExternalInput")
    batchloc = nc.dram_tensor("batchloc", [128, NWIN], f32, kind="ExternalInput")
    rgc_pn = nc.dram_tensor("rgc_pn", [128, NWIN], f32, kind="ExternalInput")
    # weights (replicated; f-gate halves pre-negated)
    lin0w = nc.dram_tensor("lin0w", [NF, D1], bf16, kind="ExternalInput")
    lin0b = nc.dram_tensor("lin0b", [D1, 1], f32, kind="ExternalInput")
    wdst = nc.dram_tensor("wdst", [D1, L * 128], bf16, kind="ExternalInput")
    wsrc = nc.dram_tensor("wsrc", [D1, L * 128], bf16, kind="ExternalInput")
    wea = nc.dram_tensor("wea", [64, L * 128], bf16, kind="ExternalInput")
    bng = nc.dram_tensor("bng", [D1, L], f32, kind="ExternalInput")
    bnb = nc.dram_tensor("bnb", [D1, L], f32, kind="ExternalInput")
    lin1w = nc.dram_tensor("lin1w", [D1, D2], f32, kind="ExternalInput")
    lin1b = nc.dram_tensor("lin1b", [D2, 1], f32, kind="ExternalInput")
    fcw = nc.dram_tensor("fcw", [D2, FC * D2], f32, kind="ExternalInput")
    fcb = nc.dram_tensor("fcb", [D2, FC], f32, kind="ExternalInput")
    lin2w = nc.dram_tensor("lin2w", [D2, 1], f32, kind="ExternalInput")
    lin2b = nc.dram_tensor("lin2b", [1, 1], f32, kind="ExternalInput")
    iota128 = nc.dram_tensor("iota128", [128, 128], bf16, kind="ExternalInput")
    iota256 = nc.dram_tensor("iota256", [128, G], bf16, kind="ExternalInput")
    ident = nc.dram_tensor("ident", [128, 128], f32, kind="ExternalInput")
    identb = nc.dram_tensor("identb", [128, 128], bf16, kind="ExternalInput")

    yout = nc.dram_tensor("y", [1, G], f32, kind="ExternalOutput")

    # ---- DRAM scratch ----
    QsD = nc.dram_tensor("QsD", [TBL, 128], bf16)          # row p*NWING+W
    ag_in = nc.dram_tensor("ag_in", [D1, SHARD_P], f8)
    ag_out = nc.dram_tensor("ag_out", [NCORES * D1, SHARD_P], f8,
                            addr_space="Shared")
    ar_in = nc.dram_tensor("ar_in", [D1, 2], f32)
    ar_out = nc.dram_tensor("ar_out", [NCORES * D1, 2], f32, addr_space="Shared")
    pl_in = nc.dram_tensor("pl_in", [D1, G], f32)
    pl_out = nc.dram_tensor("pl_out", [D1, G], f32, addr_space="Shared")

    rg = [list(range(NCORES))]
    QsD3 = QsD[:, :].rearrange("(p w) f -> p w f", p=128)   # [128, NWING, 128]

    with TileContext(nc) as tc:
        with (
            tc.tile_pool(name="const", bufs=1) as cp,
            tc.tile_pool(name="big", bufs=1) as bigp,
            tc.tile_pool(name="work", bufs=2) as wp,
            tc.tile_pool(name="gat", bufs=4) as gp,
            tc.tile_pool(name="nl", bufs=3) as nlp,
            tc.tile_pool(name="oh", bufs=3) as ohp,
            tc.tile_pool(name="st", bufs=2) as stp,
            tc.tile_pool(name="scr", bufs=1) as scp,
            tc.tile_pool(name="pre", bufs=2, space="PSUM") as pp,
            tc.tile_pool(name="psB", bufs=2, space="PSUM") as ppB,
            tc.tile_pool(name="psA", bufs=2, space="PSUM") as ppA,
        ):
            # ---------- constants ----------
            def load_const(t, dram, shape, dtype=f32):
                tt = cp.tile(shape, dtype, tag=t)
                nc.sync.dma_start(out=tt[:], in_=dram)
                return tt

            io128 = load_const("io128", iota128[:, :], [128, 128], bf16)
            io256 = load_const("io256", iota256[:, :], [128, G], bf16)
            idn = load_const("idn", ident[:, :], [128, 128])
            idnb = load_const("idnb", identb[:, :], [128, 128], bf16)
            l0w = load_const("l0w", lin0w[:, :], [NF, D1], bf16)
            l0b = load_const("l0b", lin0b[:, :], [D1, 1])
            wd = load_const("wd", wdst[:, :], [D1, L * 128], bf16)
            ws = load_const("ws", wsrc[:, :], [D1, L * 128], bf16)
            we = load_const("we", wea[:, :], [64, L * 128], bf16)
            gmt = load_const("gmt", bng[:, :], [D1, L])
            bbt = load_const("bbt", bnb[:, :], [D1, L])
            l1w = load_const("l1w", lin1w[:, :], [D1, D2])
            l1b = load_const("l1b", lin1b[:, :], [D2, 1])
            fw = load_const("fw", fcw[:, :], [D2, FC * D2])
            fb = load_const("fb", fcb[:, :], [D2, FC])
            l2w = load_const("l2w", lin2w[:, :], [D2, 1])
            l2b = load_const("l2b", lin2b[:, :], [1, 1])
            dlp = load_const("dlp", dstloc_p[:, :], [128, nchunk])
            rcp = load_const("rcp", rc_p[:, :], [128, nchunk])
            blc = load_const("blc", batchloc[:, :], [128, NWIN])
            rgp = load_const("rgp", rgc_pn[:, :], [128, NWIN])
            qsix = load_const("qsix", qs_idxD[:, :], [128, etot // 16], i16)
            onesb = load_const("onesb", onesD[:, :], [1, 128], bf16)
            iotap = load_const("iotap", iotapD[:, :], [128, 1])

            # ---------- resident state ----------
            hT_own = bigp.tile([D1, SHARD_P], f32, tag="hown")
            hb_own = bigp.tile([D1, SHARD_P], bf16, tag="hbown")
            aggr_sb = bigp.tile([D1, SHARD_P], f32, tag="aggr")
            qd_sb = bigp.tile([128, NWIN, 128], bf16, tag="qdsb")

            # ---------- lin0: hT_own = relu(lin0w.T @ xT + b) ----------
            for j in range(8):
                sl = slice(j * 480, (j + 1) * 480)
                xt = wp.tile([NF, 480], bf16, tag="xt")
                nc.sync.dma_start(out=xt[:], in_=xT[:, sl])
                ph = ppB.tile([D1, 480], f32, tag="bld")
                nc.tensor.matmul(out=ph[:], lhsT=l0w[:], rhs=xt[:],
                                 start=True, stop=True)
                nc.scalar.activation(out=hT_own[:, sl], in_=ph[:],
                                     func=AF.Relu, bias=l0b[:], scale=1.0)

            # ---------- layers ----------
            for l in range(L):
                wd_l = wd[:, l * 128:(l + 1) * 128]
                ws_l = ws[:, l * 128:(l + 1) * 128]
                we_l = we[:, l * 128:(l + 1) * 128]

                # --- bf16 copy of own h ---
                nc.scalar.activation(out=hb_own[:], in_=hT_own[:],
                                     func=AF.Identity, scale=1.0)

                # --- Qd table build (own shard) -> QdD ---
                for w0 in range(0, NWIN, 4):
                    kk = min(4, NWIN - w0)
                    qp = ppB.tile([128, 512], f32, tag="bld")
                    for k in range(kk):
                        w = w0 + k
                        nc.tensor.matmul(
                            out=qp[:, k * 128:(k + 1) * 128],
                            lhsT=hb_own[:, w * 128:(w + 1) * 128],
                            rhs=wd_l, start=True, stop=True)
                    nc.scalar.activation(
                        out=qd_sb[:, w0:w0 + kk, :].rearrange("p a b -> p (a b)"),
                        in_=qp[:, :kk * 128],
                        func=AF.Identity, scale=1.0)

                # --- AllGather h (fp8) ---
                h8 = stp.tile([D1, SHARD_P], f8, tag="h8")
                nc.scalar.activation(out=h8[:], in_=hT_own[:],
                                     func=AF.Identity, scale=1.0)
                nc.sync.dma_start(out=ag_in[:, :], in_=h8[:])
                nc.gpsimd.collective_compute(
                    "AllGather", OP.bypass, replica_groups=rg,
                    ins=[ag_in.ap().opt()], outs=[ag_out.ap().opt()])

                # --- Qs table build (all nodes, per gathered shard) -> QsD ---
                ws8 = stp.tile([D1, 128], f8, tag="ws8")
                nc.scalar.activation(out=ws8[:], in_=ws_l,
                                     func=AF.Identity, scale=1.0)
                for s_ in range(NCORES):
                    hb_sh = stp.tile([D1, SHARD_P], f8, tag="hbsh")
                    nc.sync.dma_start(out=hb_sh[:],
                                      in_=ag_out[s_ * D1:(s_ + 1) * D1, :])
                    for wB in range(0, NWIN, 16):
                        kB = min(16, NWIN - wB)
                        sg_t = stp.tile([128, 16, 128], bf16, tag="qsst")
                        for w0 in range(wB, wB + kB, 4):
                            kk = min(4, wB + kB - w0)
                            qp = ppB.tile([128, 512], f32, tag="bld")
                            for k in range(kk):
                                w = w0 + k
                                nc.tensor.matmul(
                                    out=qp[:, k * 128:(k + 1) * 128],
                                    lhsT=hb_sh[:, w * 128:(w + 1) * 128],
                                    rhs=ws8[:], start=True, stop=True)
                            nc.scalar.activation(
                                out=sg_t[:, w0 - wB:w0 - wB + kk, :]
                                    .rearrange("p a b -> p (a b)"),
                                in_=qp[:, :kk * 128],
                                func=AF.Identity, scale=1.0)
                        W0 = s_ * NWIN + wB
                        nc.sync.dma_start(out=QsD3[:, W0:W0 + kB, :],
                                          in_=sg_t[:, :kB, :])

                # --- edge pipeline ---
                agg = None
                for t in range(ntile):
                    te = min(8, nchunk - t * 8)          # chunks this tile
                    ne = te * 128                        # edges this tile
                    qs_g = gp.tile([128, 8, 128], bf16, tag="qsg")
                    nc.gpsimd.dma_gather(
                        qs_g[:, :te, :], QsD[:, :],
                        qsix[:, t * 64: t * 64 + te * 8], ne, ne, 128)
                    et = gp.tile([64, 1024], bf16, tag="et")
                    nc.sync.dma_start(out=et[:, :ne],
                                      in_=eaT[:, t * 1024: t * 1024 + ne])
                    ohT = ohp.tile([128, 1024], bf16, tag="ohT")
                    for h0 in range(0, ne, 512):
                        hn = min(512, ne - h0)
                        bc = ppB.tile([128, 512], f32, tag="bld")
                        nc.tensor.matmul(out=bc[:, :hn], lhsT=onesb[:],
                                         rhs=et[0:1, h0:h0 + hn],
                                         start=True, stop=True)
                        nc.vector.tensor_scalar(
                            out=ohT[:, h0:h0 + hn], in0=bc[:, :hn],
                            scalar1=iotap[:], scalar2=None, op0=OP.is_equal)

                    pre = pp.tile([128, 1024], f32, tag="pre")
                    qs_f = qs_g[:].rearrange("p a b -> p (a b)")
                    for c in range(te):
                        gc = t * 8 + c
                        w = gc // cw
                        csl = slice(c * 128, (c + 1) * 128)
                        nc.tensor.matmul(out=pre[:, csl], lhsT=et[:, csl],
                                         rhs=we_l, start=True, stop=False)
                        nc.tensor.matmul(out=pre[:, csl], lhsT=idnb[:],
                                         rhs=qs_f[:, csl], start=False, stop=False)
                        nc.tensor.matmul(out=pre[:, csl], lhsT=ohT[:, csl],
                                         rhs=qd_sb[:, w, :], start=False, stop=True)

                    u = nlp.tile([128, 8, 128], f32, tag="u")
                    uf = u[:].rearrange("p a b -> p (a b)")
                    nc.scalar.activation(out=uf[:, :ne], in_=pre[:, :ne],
                                         func=AF.Exp, scale=1.0)
                    v = nlp.tile([128, 8, 128], f32, tag="v")
                    m = nlp.tile([128, 8, 64], bf16, tag="m")
                    if True:
                        # ACT path: v=ln(1+u) both halves; sg=exp(-v_f)
                        nc.scalar.activation(
                            out=v[:, :te, :].rearrange("p a b -> p (a b)"),
                            in_=uf[:, :ne], func=AF.Ln, bias=1.0, scale=1.0)
                        sg = nlp.tile([128, 8, 64], f32, tag="sg")
                        nc.scalar.activation(out=sg[:, :te, :],
                                             in_=v[:, :te, 0:64],
                                             func=AF.Exp, scale=-1.0)
                        nc.vector.tensor_tensor(out=m[:, :te, :],
                                                in0=sg[:, :te, :],
                                                in1=v[:, :te, 64:128], op=OP.mult)
                    else:
                        # DVE path: v_s only; sigma = 1/(1+u_f) on DVE
                        nc.scalar.activation(
                            out=v[:, :te, 64:128], in_=u[:, :te, 64:128],
                            func=AF.Ln, bias=1.0, scale=1.0)
                        w1 = nlp.tile([128, 8, 64], f32, tag="w1")
                        nc.vector.tensor_scalar(out=w1[:, :te, :],
                                                in0=u[:, :te, 0:64],
                                                scalar1=1.0, scalar2=None,
                                                op0=OP.add)
                        nc.vector.reciprocal(out=w1[:, :te, :], in_=w1[:, :te, :])
                        nc.vector.tensor_tensor(out=m[:, :te, :],
                                                in0=w1[:, :te, :],
                                                in1=v[:, :te, 64:128], op=OP.mult)

                    for c in range(te):
                        gc = t * 8 + c
                        w = gc // cw
                        if gc % (4 * cw) == 0:
                            agg = ppA.tile([D1, 512], f32, tag="agg")
                        ohS = ohp.tile([128, 128], bf16, tag="ohS")
                        nc.vector.tensor_scalar(
                            out=ohS[:], in0=io128[:],
                            scalar1=dlp[:, gc:gc + 1], scalar2=rcp[:, gc:gc + 1],
                            op0=OP.is_equal, op1=OP.mult)
                        nc.tensor.matmul(
                            out=agg[:, (w % 4) * 128:(w % 4 + 1) * 128],
                            lhsT=m[:, c, :], rhs=ohS[:],
                            start=(gc % cw == 0), stop=(gc % cw == cw - 1))
                        if gc % (4 * cw) == 4 * cw - 1 or gc == nchunk - 1:
                            grp = w // 4
                            lo = grp * 512
                            hi = min(lo + 512, SHARD_P)
                            nc.scalar.activation(
                                out=aggr_sb[:, lo:hi], in_=agg[:, :hi - lo],
                                func=AF.Identity, scale=1.0)

                # --- BN stats + AllReduce ---
                st = wp.tile([D1, 2], f32, tag="st")
                nc.vector.reduce_sum(out=st[:, 0:1], in_=aggr_sb[:],
                                     axis=mybir.AxisListType.X)
                sq = scp.tile([D1, SHARD_P], f32, tag="sq")
                nc.vector.tensor_tensor(out=sq[:], in0=aggr_sb[:],
                                        in1=aggr_sb[:], op=OP.mult)
                nc.vector.reduce_sum(out=st[:, 1:2], in_=sq[:],
                                     axis=mybir.AxisListType.X)
                nc.sync.dma_start(out=ar_in[:, :], in_=st[:])
                nc.gpsimd.collective_compute(
                    "AllGather", OP.bypass, replica_groups=rg,
                    ins=[ar_in.ap().opt()], outs=[ar_out.ap().opt()])
                stga = wp.tile([D1, 2, NCORES], f32, tag="stga")
                nc.sync.dma_start(
                    out=stga[:],
                    in_=ar_out[:, :].rearrange("(c p) s -> p s c", p=D1))
                stg = wp.tile([D1, 2], f32, tag="stg")
                nc.vector.reduce_sum(
                    out=stg[:].rearrange("p (s o) -> p s o", o=1),
                    in_=stga[:], axis=mybir.AxisListType.X)
                mu = wp.tile([D1, 1], f32, tag="mu")
                nc.vector.tensor_scalar(out=mu[:], in0=stg[:, 0:1],
                                        scalar1=1.0 / N, scalar2=None, op0=OP.mult)
                ex2 = wp.tile([D1, 1], f32, tag="ex2")
                nc.vector.tensor_scalar(out=ex2[:], in0=stg[:, 1:2],
                                        scalar1=1.0 / N, scalar2=None, op0=OP.mult)
                mu2 = wp.tile([D1, 1], f32, tag="mu2")
                nc.vector.tensor_tensor(out=mu2[:], in0=mu[:], in1=mu[:], op=OP.mult)
                var = wp.tile([D1, 1], f32, tag="var")
                nc.vector.tensor_tensor(out=var[:], in0=ex2[:], in1=mu2[:],
                                        op=OP.subtract)
                ve = wp.tile([D1, 1], f32, tag="ve")
                nc.vector.tensor_scalar(out=ve[:], in0=var[:], scalar1=EPS,
                                        scalar2=None, op0=OP.add)
                lv = wp.tile([D1, 1], f32, tag="lv")
                nc.scalar.activation(out=lv[:], in_=ve[:], func=AF.Ln, scale=1.0)
                isd = wp.tile([D1, 1], f32, tag="isd")
                nc.scalar.activation(out=isd[:], in_=lv[:], func=AF.Exp, scale=-0.5)
                scale = wp.tile([D1, 1], f32, tag="scale")
                nc.vector.tensor_tensor(out=scale[:], in0=isd[:],
                                        in1=gmt[:, l:l + 1], op=OP.mult)
                mshift = wp.tile([D1, 1], f32, tag="mshift")
                nc.vector.tensor_tensor(out=mshift[:], in0=mu[:], in1=scale[:],
                                        op=OP.mult)
                shift = wp.tile([D1, 1], f32, tag="shift")
                nc.vector.tensor_tensor(out=shift[:], in0=bbt[:, l:l + 1],
                                        in1=mshift[:], op=OP.subtract)
                # h = relu(h + aggr*scale + shift)
                asb = scp.tile([D1, SHARD_P], f32, tag="asb")
                nc.vector.tensor_scalar(out=asb[:], in0=aggr_sb[:],
                                        scalar1=scale[:], scalar2=shift[:],
                                        op0=OP.mult, op1=OP.add)
                nc.vector.tensor_tensor(out=asb[:], in0=asb[:], in1=hT_own[:],
                                        op=OP.add)
                nc.vector.tensor_scalar(out=hT_own[:], in0=asb[:],
                                        scalar1=0.0, scalar2=None, op0=OP.max)

            # ---------- global mean pool ----------
            pool_ps = pp.tile([D1, G], f32, tag="pre")
            for w in range(NWIN):
                tp = ppB.tile([128, D1], f32, tag="bld")
                nc.tensor.transpose(out=tp[:], in_=hT_own[:, w * 128:(w + 1) * 128],
                                    identity=idn[0:D1, 0:D1])
                rows = wp.tile([128, D1], bf16, tag="rows")
                nc.vector.tensor_copy(out=rows[:], in_=tp[:])
                ohg = ohp.tile([128, G], bf16, tag="ohg")
                nc.vector.tensor_scalar(
                    out=ohg[:], in0=io256[:],
                    scalar1=blc[:, w:w + 1], scalar2=rgp[:, w:w + 1],
                    op0=OP.is_equal, op1=OP.mult)
                nc.tensor.matmul(out=pool_ps[:], lhsT=rows[:], rhs=ohg[:],
                                 start=(w == 0), stop=(w == NWIN - 1))
            poolT = wp.tile([D1, G], f32, tag="poolT")
            nc.vector.tensor_copy(out=poolT[:], in_=pool_ps[:])
            nc.sync.dma_start(out=pl_in[:, :], in_=poolT[:])
            nc.gpsimd.collective_compute(
                "AllReduce", OP.add, replica_groups=rg,
                ins=[pl_in.ap().opt()], outs=[pl_out.ap().opt()])
            pg = wp.tile([D1, G], f32, tag="pg")
            nc.sync.dma_start(out=pg[:], in_=pl_out[:, :])

            # ---------- head ----------
            a = pg
            hw_ = [(l1w[:], l1b[:]), (fw[:, 0:D2], fb[:, 0:1]), (fw[:, D2:2 * D2], fb[:, 1:2])]
            for (wt, bt) in hw_:
                ps = ppB.tile([D2, G], f32, tag="bld")
                nc.tensor.matmul(out=ps[:, 0:G], lhsT=wt, rhs=a[:], start=True, stop=True)
                an = wp.tile([D2, G], f32, tag="an")
                nc.scalar.activation(out=an[:], in_=ps[:, 0:G], func=AF.Relu,
                                     bias=bt, scale=1.0)
                a = an
            ps = ppB.tile([1, G], f32, tag="bld")
            nc.tensor.matmul(out=ps[:, 0:G], lhsT=l2w[:], rhs=a[:], start=True, stop=True)
            yt = wp.tile([1, G], f32, tag="yt")
            nc.scalar.activation(out=yt[:], in_=ps[:, 0:G], func=AF.Identity,
                                 bias=l2b[:], scale=1.0)
            nc.sync.dma_start(out=yout[:, :], in_=yt[:])

    nc.compile()
    return nc


def _wrap16(idx):
    """Flat idx list -> [128, n/16] int16: slot i at [i%16, i//16], replicated
    across the 8 Q7 cores."""
    a = idx.reshape(-1, 16).T.astype(np.int16)
    return np.tile(a, (8, 1))


def _preprocess(inputs):
    x = np.asarray(inputs["x"], np.float32)
    ea = np.asarray(inputs["edge_attr"], np.float32)
    ei = np.asarray(inputs["edge_index"]).astype(np.int64)
    batch = np.asarray(inputs["batch"]).astype(np.int64)
    src, dst = ei[0], ei[1]

    cnt = np.bincount(dst, minlength=N).astype(np.float32)
    rc_node = 1.0 / np.maximum(cnt, 1.0)
    gcnt = np.bincount(batch, minlength=G).astype(np.float32)
    rgc = 1.0 / np.maximum(gcnt, 1.0)

    # Degree-balanced node -> (window, slot) assignment: snake-deal nodes in
    # descending-degree order across the 240 global windows, minimizing the
    # max per-window edge count (which sets the uniform chunk pad cw).
    deg_order = np.argsort(-cnt, kind="stable")       # node ids, deg desc
    nwin_g = NCORES * NWIN                            # 240
    perm_loc = np.empty(N, np.int64)                  # node -> global padded id
    wslot = np.zeros(nwin_g, np.int64)
    for i0 in range(0, N, nwin_g):
        blk = deg_order[i0:i0 + nwin_g]
        j = i0 // nwin_g
        wins = np.arange(len(blk)) if j % 2 == 0 else (len(blk) - 1 - np.arange(len(blk)))
        # rows fill slot j of each window; window w -> core w//NWIN,
        # local (w%NWIN)*128 + slot
        w_ids = wins
        perm_loc[blk] = (w_ids // NWIN) * SHARD_P + (w_ids % NWIN) * 128 + j
    # padded global id for every node
    gperm = perm_loc
    srcg = gperm[src]
    dstg = gperm[dst]
    order = np.argsort(dstg, kind="stable")
    srcg_s, dstg_s, ea_idx = srcg[order], dstg[order], order

    bounds = []
    for c in range(NCORES):
        for w in range(NWIN):
            bounds.append(c * SHARD_P + w * 128)
    bounds.append(NCORES * SHARD_P)
    bpos = np.searchsorted(dstg_s, np.asarray(bounds), side="left")
    percw = {}
    maxcnt = 0
    k = 0
    for c in range(NCORES):
        for w in range(NWIN):
            lo, hi = bpos[k], bpos[k + 1]
            percw[(c, w)] = np.arange(lo, hi)
            maxcnt = max(maxcnt, hi - lo)
            k += 1
    cw = max(1, (maxcnt + 127) // 128)
    etot = NWIN * cw * 128

    per_core = []
    for c in range(NCORES):
        qs_idx = np.zeros(etot, np.int64)
        dl = np.full(etot, -1.0, np.float32)
        rc_e = np.ones(etot, np.float32)
        ea_e = np.zeros((etot, EF), np.float32)
        for w in range(NWIN):
            idxs = percw[(c, w)]
            o = w * cw * 128
            k = len(idxs)
            g = srcg_s[idxs]                           # padded global id
            qs_idx[o:o + k] = (g % 128) * NWING + (g // 128)
            loc = dstg_s[idxs] - c * SHARD_P           # 0..3839
            dl[o:o + k] = (loc - w * 128).astype(np.float32)
            rc_e[o:o + k] = rc_node[dst[ea_idx[idxs]]]
            ea_e[o:o + k] = ea[ea_idx[idxs]]
        eaT = np.zeros((64, etot), np.float32)
        eaT[0] = dl
        eaT[1:EF + 1] = ea_e.T
        eaT[EF + 1] = 1.0
        eaT[EF + 1, dl < 0] = 0.0
        nch = etot // 128
        d = {
            "qs_idxD": _wrap16(qs_idx),
            "dstloc_p": dl.reshape(nch, 128).T.copy(),
            "rc_p": rc_e.reshape(nch, 128).T.copy(),
            "eaT": eaT.astype(ml_dtypes.bfloat16),
        }
        nodes_c = np.nonzero((gperm // SHARD_P) == c)[0]
        locs_c = gperm[nodes_c] - c * SHARD_P
        xp = np.zeros((NF, SHARD_P), np.float32)
        xp[:, locs_c] = x[nodes_c].T
        d["xT"] = xp.astype(ml_dtypes.bfloat16)
        bl = np.full(SHARD_P, -1.0, np.float32)
        bl[locs_c] = batch[nodes_c].astype(np.float32)
        rg_n = np.zeros(SHARD_P, np.float32)
        rg_n[locs_c] = rgc[batch[nodes_c]]
        d["batchloc"] = bl.reshape(NWIN, 128).T.copy()
        d["rgc_pn"] = rg_n.reshape(NWIN, 128).T.copy()
        per_core.append(d)

    # replicated weights; f-gate half negated so pre = [-a | b]
    wf = np.asarray(inputs["conv_wf"], np.float32)
    wsv = np.asarray(inputs["conv_ws"], np.float32)
    bf = np.asarray(inputs["conv_bf"], np.float32)
    bs = np.asarray(inputs["conv_bs"], np.float32)
    wdst = np.concatenate([-wf[:, 0:D1, :], wsv[:, 0:D1, :]], axis=2)
    wsrc = np.concatenate([-wf[:, D1:2 * D1, :], wsv[:, D1:2 * D1, :]], axis=2)
    wea = np.concatenate([-wf[:, 2 * D1:, :], wsv[:, 2 * D1:, :]], axis=2)
    bias = np.concatenate([-bf, bs], axis=1)[:, None, :]
    wea = np.concatenate([wea, bias], axis=1)
    shared = {
        "lin0w": np.asarray(inputs["lin0_w"], np.float32).astype(ml_dtypes.bfloat16),
        "lin0b": np.asarray(inputs["lin0_b"], np.float32).reshape(D1, 1),
        "wdst": np.transpose(wdst, (1, 0, 2)).reshape(D1, L * 128).astype(ml_dtypes.bfloat16),
        "wsrc": np.transpose(wsrc, (1, 0, 2)).reshape(D1, L * 128).astype(ml_dtypes.bfloat16),
        "wea": np.concatenate([
            np.zeros((1, L * 128), np.float32),
            np.transpose(wea, (1, 0, 2)).reshape(EF + 1, L * 128),
            np.zeros((64 - EF - 2, L * 128), np.float32),
        ], axis=0).astype(ml_dtypes.bfloat16),
        "bng": np.asarray(inputs["bn_gamma"], np.float32).T.copy(),
        "bnb": np.asarray(inputs["bn_beta"], np.float32).T.copy(),
        "lin1w": np.asarray(inputs["lin1_w"], np.float32),
        "lin1b": np.asarray(inputs["lin1_b"], np.float32).reshape(D2, 1),
        "fcw": np.transpose(np.asarray(inputs["fc_w"], np.float32), (1, 0, 2)).reshape(D2, FC * D2),
        "fcb": np.asarray(inputs["fc_b"], np.float32).T.copy(),
        "lin2w": np.asarray(inputs["lin2_w"], np.float32).reshape(D2, 1),
        "lin2b": np.asarray(inputs["lin2_b"], np.float32).reshape(1, 1),
        "iota128": np.broadcast_to(np.arange(128, dtype=np.float32)[None, :],
                                   (128, 128)).astype(ml_dtypes.bfloat16),
        "iota256": np.broadcast_to(np.arange(G, dtype=np.float32)[None, :],
                                   (128, G)).astype(ml_dtypes.bfloat16),
        "ident": np.eye(128, dtype=np.float32),
        "identb": np.eye(128, dtype=np.float32).astype(ml_dtypes.bfloat16),
        "onesD": np.ones((1, 128), np.float32).astype(ml_dtypes.bfloat16),
        "iotapD": np.arange(128, dtype=np.float32).reshape(128, 1),
    }
    in_maps = [dict(shared, **pc) for pc in per_core]
    return in_maps, cw


def kernel(**inputs):
    from concourse.bass_utils import run_bass_kernel_spmd

    in_maps, cw = _preprocess(inputs)
    key = ("nc", cw)
    if key not in _CACHE:
        _CACHE[key] = _build_nc(cw)
    nc = _CACHE[key]
    res = run_bass_kernel_spmd(nc, in_maps, core_ids=list(range(NCORES)))
    return res.results[0]["y"].reshape(G).astype(np.float32)
